# revision 1
# baseline (speedup 1.0000x reference)
import sys, os
sys.path.insert(0, '/opt/trn_rl_repo')
import numpy as np
import ml_dtypes
import concourse.bass as bass
import concourse.bacc as bacc
import concourse.mybir as mybir
import concourse.tile as tile
from concourse.tile import add_dep_helper
from concourse.bass_utils import run_bass_kernel_spmd

BF = mybir.dt.bfloat16
F32 = mybir.dt.float32
AF = mybir.ActivationFunctionType
ALU = mybir.AluOpType
AX = mybir.AxisListType

N_CORES = 8
B = 512
BL = B // N_CORES      # 64 batch rows per core
T = 365
TP = 368               # t padded to 8-multiple
D = 256
FA = 128               # attention MLP hidden
NF = 16                # forecast steps
NR = 256               # decoder hidden


def _bf(x):
    return np.ascontiguousarray(x).astype(ml_dtypes.bfloat16)


def _f32(x):
    return np.ascontiguousarray(x).astype(np.float32)


def _ktile(w, k_total, pad_to):
    """[K, M] -> [ntiles, 128, M] zero-padded on K."""
    K, M = w.shape
    assert K == k_total
    nt = (pad_to + 127) // 128
    out = np.zeros((nt, 128, M), w.dtype)
    for i in range(nt):
        lo = i * 128
        hi = min(K, lo + 128)
        if lo < K:
            out[i, :hi - lo] = w[lo:hi]
    return out


def build_bass():
    nc = bacc.Bacc("TRN2", target_bir_lowering=False, debug=False,
                   num_devices=N_CORES)

    def inp(name, shape, dt=BF):
        return nc.dram_tensor(name, shape, dt, kind="ExternalInput")

    # per-core sharded tensors
    xdt = inp("xdt", [2, 128, T, BL])          # X^T: xdt[k,p,t,b] = X[t,b,128k+p]
    xtb = inp("xtb", [3, 128, BL, D])          # X:  xtb[k,p,b,d] = X[128k+p,b,d]
    st0 = inp("st0", [4, 128, BL])             # [h0;c0]^T k-tiled
    idxr = inp("idxr", [128, 3, BL], F32)      # indices replicated over partitions
    # replicated weights
    iota = inp("iota", [128, 1], F32)
    emb_s = inp("emb_s", [54, 16])
    emb_i = inp("emb_i", [32, 128, 64])
    emb_f = inp("emb_f", [33, 16])
    wc1p = inp("wc1p", [7, 128, 512])
    bc1t = inp("bc1t", [128, 4], F32)
    wc2 = inp("wc2", [4, 128, 384])
    bc2t = inp("bc2t", [128, 3], F32)
    wc3 = inp("wc3", [3, 128, 512])
    bc3t = inp("bc3t", [128, 4], F32)
    wa1a = inp("wa1a", [2, 128, 128])
    wa1b = inp("wa1b", [4, 128, 128])
    ba1t = inp("ba1t", [128, 1], F32)
    wa2 = inp("wa2", [128, 1])
    wiht = inp("wiht", [2, 128, 1024])
    whht = inp("whht", [2, 128, 1024])
    bgt = inp("bgt", [128, 8], F32)
    wt1 = inp("wt1", [2, 128, 128])
    bt1t = inp("bt1t", [128, 1], F32)
    wt2 = inp("wt2", [128, 1])
    bt2r = inp("bt2r", [128, 1], F32)
    id16 = inp("id16", [128, 128])             # bf16 identity
    id32 = inp("id32", [128, 128], F32)        # f32 identity
    out = nc.dram_tensor("out", [NF, BL], F32, kind="ExternalOutput")
    DBG = os.environ.get("KDBG", "") == "1"
    if DBG:
        dbg_ct3 = nc.dram_tensor("dbg_ct3", [128, 4, BL], F32, kind="ExternalOutput")
        dbg_q = nc.dram_tensor("dbg_q", [128, BL], F32, kind="ExternalOutput")
        dbg_sc = nc.dram_tensor("dbg_sc", [128, 183], F32, kind="ExternalOutput")
        dbg_e = nc.dram_tensor("dbg_e", [128, 183], F32, kind="ExternalOutput")
        dbg_inpT = nc.dram_tensor("dbg_inpT", [128, 2, BL], F32, kind="ExternalOutput")
        dbg_h = nc.dram_tensor("dbg_h", [128, 2, BL], F32, kind="ExternalOutput")
        dbg_whi = nc.dram_tensor("dbg_whi", [128, 128], F32, kind="ExternalOutput")
        dbg_wlo = nc.dram_tensor("dbg_wlo", [55, 128], F32, kind="ExternalOutput")
        dbg_wsh = nc.dram_tensor("dbg_wsh", [128, 64], F32, kind="ExternalOutput")
        dbg_xtb = nc.dram_tensor("dbg_xtb", [128, 3, BL, D], BF, kind="ExternalOutput")
        dbg_v = [nc.dram_tensor(f"dbg_v{i}", [128, 2, BL], F32,
                                kind="ExternalOutput") for i in range(4)]

    with tile.TileContext(nc) as tc:
        with (
            tc.tile_pool(name="const", bufs=1) as cpool,
            tc.tile_pool(name="big", bufs=1) as bigpool,
            tc.tile_pool(name="xchunk", bufs=3) as xchunk,
            tc.tile_pool(name="rchunk", bufs=3) as rchunk,
            tc.tile_pool(name="dbgp", bufs=1) as dbgp,
            tc.tile_pool(name="work", bufs=2) as work,
            tc.tile_pool(name="state", bufs=1) as state,
            tc.tile_pool(name="ps", bufs=4, space="PSUM") as ps,
            tc.tile_pool(name="psf", bufs=2, space="PSUM") as psf,
        ):
            # ---- load constants ----
            def ld(ap_dram, shape, dt=BF, tag=None):
                t_ = cpool.tile(shape, dt, tag=tag or ap_dram.name)
                nc.sync.dma_start(t_[:], ap_dram)
                return t_

            # k-tiled weights go to [128, ntile, M] sbuf layout
            def ldk(dram, nt, m, dt=BF):
                t_ = cpool.tile([128, nt, m], dt, tag=dram.name)
                for k in range(nt):
                    nc.sync.dma_start(t_[:, k, :], dram[k])
                return t_

            wc1p_s = ldk(wc1p, 7, 512)
            wc2_s = ldk(wc2, 4, 384)
            wc3_s = ldk(wc3, 3, 512)
            wa1a_s = ldk(wa1a, 2, 128)
            wa1b_s = ldk(wa1b, 4, 128)
            wiht_s = ldk(wiht, 2, 1024)
            whht_s = ldk(whht, 2, 1024)
            wt1_s = ldk(wt1, 2, 128)
            embi_s = ldk(emb_i, 32, 64)
            ba1_s = ld(ba1t[:, :], [128, 1], F32)
            wa2_s = ld(wa2[:, :], [128, 1])
            bgt_s = ld(bgt[:, :], [128, 8], F32)
            bt1_s = ld(bt1t[:, :], [128, 1], F32)
            wt2_s = ld(wt2[:, :], [128, 1])
            bt2_s = ld(bt2r[:, :], [128, 1], F32)
            id16_s = ld(id16[:, :], [128, 128])
            id32_s = ld(id32[:, :], [128, 128], F32)
            bc1_s = ld(bc1t[:, :], [128, 4], F32)
            bc2_s = ld(bc2t[:, :], [128, 3], F32)
            bc3_s = ld(bc3t[:, :], [128, 4], F32)
            embs_s = ld(emb_s[:, :], [54, 16])
            embf_s = ld(emb_f[:, :], [33, 16])
            idx_s = ld(idxr[:, :, :], [128, 3, BL], F32)
            iota_s = ld(iota[:, :], [128, 1], F32)
            st0_s = ldk(st0, 4, BL)

            # ---- persistent big tensors ----
            xtb_s = bigpool.tile([128, 3, BL, D], BF, tag="xtb")
            for k in range(3):
                nc.sync.dma_start(xtb_s[:, k, :, :], xtb[k])
            if DBG:
                nc.sync.dma_start(dbg_xtb[:, :, :, :], xtb_s[:, :, :, :])
            P_s = bigpool.tile([128, TP, BL], BF, tag="P")
            # zero the 3 padded t-columns of P so R never reads uninit data
            nc.vector.memset(P_s[:, T:TP, :], 0.0)

            # ---- embeddings -> xcond tiles 0..2; thought -> 3..6 ----
            xcond = state.tile([128, 7, BL], BF, tag="xcond")
            nc.vector.memset(xcond[:, 0:3, :], 0.0)
            for k in range(4):
                nc.vector.tensor_copy(xcond[:, 3 + k, :], st0_s[:, k, :])

            def onehot_embed(col, table_s, nt, width, out_slice):
                # out_slice [width, BL] = table^T @ onehot
                pe_out = ps.tile([width, BL], F32, tag="sm")
                for k in range(nt):
                    oh = work.tile([128, BL], BF, tag="oh")
                    # oh[p, b] = 1.0 if idx[b] - 128k == p else 0
                    nc.vector.scalar_tensor_tensor(
                        oh[:], idx_s[:, col, :], float(-128 * k),
                        iota_s[:, :].broadcast_to((128, BL)),
                        op0=ALU.add, op1=ALU.is_equal)
                    kk = table_s.shape[0] if nt == 1 else 128
                    lhs = table_s[:, k, :] if nt > 1 else table_s[:, :]
                    nc.tensor.matmul(pe_out[:], lhs[:kk] if nt == 1 else lhs,
                                     oh[:kk] if nt == 1 else oh[:],
                                     start=(k == 0), stop=(k == nt - 1))
                nc.scalar.activation(out_slice, pe_out[:], AF.Copy)

            onehot_embed(0, embs_s, 1, 16, xcond[0:16, 0, :])
            onehot_embed(1, embi_s, 32, 64, xcond[0:64, 1, :])
            onehot_embed(2, embf_s, 1, 16, xcond[0:16, 2, :])

            # ---- conditioning MLP ----
            def mlp_layer(w_s, nk, x_s, nx, mt, bias_s, relu, out_t):
                # out [mt*128, BL] = act(w^T x + b): w_s [128, nk, mt*128]
                for m in range(mt):
                    pe_o = ps.tile([128, BL], F32, tag="sm")
                    for k in range(nk):
                        nc.tensor.matmul(
                            pe_o[:], w_s[:, k, m * 128:(m + 1) * 128],
                            x_s[:, k, :],
                            start=(k == 0), stop=(k == nk - 1))
                    nc.scalar.activation(
                        out_t[:, m, :], pe_o[:],
                        AF.Relu if relu else AF.Identity,
                        bias=bias_s[:, m:m + 1])

            ct1 = state.tile([128, 4, BL], BF, tag="ct1")
            mlp_layer(wc1p_s, 7, xcond, 7, 4, bc1_s, True, ct1)
            ct2 = state.tile([128, 3, BL], BF, tag="ct2")
            mlp_layer(wc2_s, 4, ct1, 4, 3, bc2_s, True, ct2)
            ct3f = state.tile([128, 4, BL], F32, tag="ct3f")
            mlp_layer(wc3_s, 3, ct2, 3, 4, bc3_s, False, ct3f)

            if DBG:
                nc.sync.dma_start(dbg_ct3[:, :, :], ct3f[:])
            # state: stT bf16 [128, 4, BL] (h tiles 0-1, c tiles 2-3), c f32 master
            stT = state.tile([128, 4, BL], BF, tag="stT")
            c32 = state.tile([128, 2, BL], F32, tag="c32")
            nc.vector.tensor_copy(stT[:], ct3f[:])
            nc.vector.tensor_copy(c32[:], ct3f[:, 2:4, :])

            # ---- P precompute: P[f, t, b] = sum_d X[t,b,d] Wa1a[d,f] ----
            CH = 8  # t per chunk
            nch = (T + CH - 1) // CH  # 46; last chunk covers t 360..364 (5)
            for c in range(nch):
                t0 = c * CH
                tn = min(CH, T - t0)
                xc = xchunk.tile([128, 2, CH, BL], BF, tag="xc")
                for k in range(2):
                    nc.sync.dma_start(xc[:, k, :tn, :], xdt[k, :, t0:t0 + tn, :])
                pe_p = ps.tile([128, CH * BL], F32, tag="sm")
                for k in range(2):
                    nc.tensor.matmul(pe_p[:, :tn * BL], wa1a_s[:, k, :],
                                     xc[:, k, :tn, :].rearrange("p t b -> p (t b)"),
                                     start=(k == 0), stop=(k == 1))
                eng = nc.scalar if c % 2 == 0 else nc.vector
                if c % 2 == 0:
                    nc.scalar.activation(
                        P_s[:, t0:t0 + tn, :].rearrange("p t b -> p (t b)"),
                        pe_p[:, :tn * BL], AF.Copy)
                else:
                    nc.vector.tensor_copy(
                        P_s[:, t0:t0 + tn, :].rearrange("p t b -> p (t b)"),
                        pe_p[:, :tn * BL])

            # ---- output accumulator ----
            outs_s = state.tile([128, 2, NF, BL], BF, tag="outs")
            # combined lo-weights: rows 0:55 even-lo, 64:118 odd-lo (via DMA),
            # rows 55:64 stay zero to match the zero rows of xtb group 1
            w_cmb = state.tile([128, 64], BF, tag="wcmb")
            nc.vector.memset(w_cmb[32:64, :], 0.0)

            # ================= decoder steps =================
            for s in range(NF):
                # q[f, b] = Wa1b^T @ stT + ba1
                q_ps = ps.tile([128, BL], F32, tag="sm")
                for k in range(4):
                    nc.tensor.matmul(q_ps[:], wa1b_s[:, k, :], stT[:, k, :],
                                     start=(k == 0), stop=(k == 3))
                q_sb = work.tile([128, BL], BF, tag="qsb")
                nc.scalar.activation(q_sb[:], q_ps[:], AF.Identity, bias=ba1_s[:, :])
                if DBG and s == 0:
                    qdb = dbgp.tile([128, BL], F32, tag="dbgst")
                    nc.vector.tensor_copy(qdb[:], q_sb[:])
                    nc.sync.dma_start(dbg_q[:, :], qdb[:])

                # R = relu(P + q) chunked over t (16 t per chunk), fused with
                # the scores matmuls: R-chunk flat 128-slices as stationary,
                # wa2 [128, 1] moving -> psum col [128, 1]. Col j covers flat
                # (t*64+b) in [128j, 128j+128) = (t=2j, all b; t=2j+1, all b).
                # Partition p of col j: t = 2j + (p>=64), b = p%64.
                NJ = 183  # col 183 would be all-pad (t=366/367)
                TCH = 16
                sc_ps = ps.tile([128, NJ], F32, tag="sm")
                for c in range(TP // TCH):
                    t0 = c * TCH
                    rt = rchunk.tile([128, TCH, BL], BF, tag="rt")
                    nc.vector.tensor_tensor(
                        rt[:], P_s[:, t0:t0 + TCH, :],
                        q_sb[:].unsqueeze(1).broadcast_to((128, TCH, BL)),
                        op=ALU.add)
                    rflat = rt[:, :, :].rearrange("p t b -> p (t b)")
                    if c % 2 == 0:
                        nc.scalar.activation(rflat, rflat, AF.Relu)
                    else:
                        nc.vector.tensor_scalar_max(rflat, rflat, 0.0)
                    for jj in range(TCH * BL // 128):
                        j = c * (TCH * BL // 128) + jj
                        if j >= NJ:
                            continue
                        nc.tensor.matmul(sc_ps[:, j:j + 1],
                                         rflat[:, 128 * jj:128 * (jj + 1)],
                                         wa2_s[:, :], start=True, stop=True)
                sc_tb = work.tile([128, NJ], F32, tag="sctb")
                nc.vector.tensor_copy(sc_tb[:], sc_ps[:])
                # col 182 rows 64.. is t=365 (pad): mask for the max/exp
                nc.vector.memset(sc_tb[64:128, NJ - 1:NJ], -1e30)

                if DBG and s == 0:
                    nc.sync.dma_start(dbg_sc[:, :], sc_tb[:])
                # softmax over t: rows p and p+64 hold the same b
                mx = work.tile([128, 1], F32, tag="mx")
                nc.vector.tensor_reduce(mx[:], sc_tb[:], axis=AX.X, op=ALU.max)
                mx_lo = work.tile([64, 1], F32, tag="mxlo")
                nc.sync.dma_start(mx_lo[:], mx[64:128, :])
                mneg = work.tile([128, 1], F32, tag="mneg")
                mpair = work.tile([64, 1], F32, tag="mpair")
                nc.vector.tensor_tensor(mpair[:], mx[0:64, :], mx_lo[:],
                                        op=ALU.max)
                nc.vector.tensor_scalar_mul(mneg[0:64, :], mpair[:], -1.0)
                nc.sync.dma_start(mneg[64:128, :], mneg[0:64, :])
                e_tb = work.tile([128, NJ], BF, tag="etb")
                z2 = work.tile([128, 1], F32, tag="z2")
                nc.scalar.activation(e_tb[:], sc_tb[:], AF.Exp,
                                     bias=mneg[:, :], accum_out=z2[:])
                z_lo = work.tile([64, 1], F32, tag="zlo")
                nc.sync.dma_start(z_lo[:], z2[64:128, :])
                zsum = work.tile([64, 1], F32, tag="zsum")
                nc.vector.tensor_tensor(zsum[:], z2[0:64, :], z_lo[:],
                                        op=ALU.add)
                zrec = work.tile([128, 1], F32, tag="zrec")
                nc.vector.reciprocal(zrec[0:64, :], zsum[:])
                nc.sync.dma_start(zrec[64:128, :], zrec[0:64, :])
                # normalize e in place (per-partition scalar = 1/Z of that b)
                nc.vector.tensor_scalar_mul(e_tb[:], e_tb[:], zrec[:, :])

                if DBG and s == 0:
                    edb = dbgp.tile([128, 183], F32, tag="dbgst")
                    nc.vector.tensor_copy(edb[:], e_tb[:])
                    nc.sync.dma_start(dbg_e[:, :], edb[:])
                # transpose e_tb -> w_hi [128, 128] (t-pairs 0..127),
                # w_lo [55, 128] (t-pairs 128..182); col = (t%2)*64 + b
                w_hi = work.tile([128, 128], BF, tag="whi")
                w_lo = work.tile([55, 128], BF, tag="wlo")
                tr1 = ps.tile([128, 128], BF, tag="smb")
                nc.tensor.transpose(tr1[:], e_tb[:, 0:128], id16_s[:, :])
                nc.vector.tensor_copy(w_hi[:], tr1[:])
                tr2 = ps.tile([55, 128], BF, tag="smb")
                nc.tensor.transpose(tr2[:], e_tb[:, 128:NJ], id16_s[:, :])
                nc.vector.tensor_copy(w_lo[:], tr2[:])

                if DBG and s == 0:
                    t1 = dbgp.tile([128, 128], F32, tag="dbg2")
                    nc.vector.tensor_copy(t1[:], w_hi[:])
                    nc.sync.dma_start(dbg_whi[:, :], t1[:])
                    t2 = dbgp.tile([55, 128], F32, tag="dbg3")
                    nc.vector.tensor_copy(t2[:], w_lo[:])
                    nc.sync.dma_start(dbg_wlo[:, :], t2[:])
                # odd-lo weights must sit at partition base 64 to match the
                # xtb group-1 packing (stationary base must be 0/32/64)
                nc.vector.tensor_copy(w_cmb[0:55, :], w_lo[0:55, 0:64])
                nc.sync.dma_start(w_cmb[64:118, :], w_lo[0:54, 64:128])
                w_sh = w_cmb
                if DBG and s == 0:
                    t3 = dbgp.tile([128, 64], F32, tag="dbg4")
                    nc.vector.tensor_copy(t3[:], w_sh[:])
                    nc.sync.dma_start(dbg_wsh[:, :], t3[:])

                # weighted sum -> inpT[d, b] directly:
                # X-slice [K_t, 128 dcols] stationary, w col [K_t, 1] moving.
                # xtb groups: 0 = even-hi t=0,2..254 | 1 = even-lo t=256..364
                # at rows 0:55 + odd-lo t=257..363 at rows 64:118 | 2 = odd-hi
                v_ps = ps.tile([128, 2, BL], F32, tag="sm")
                prev_mm = None
                for b in range(BL):
                    # order matters: the two partial-row groups (rows 0:55
                    # and 64:118) occupy disjoint PE row-groups and would run
                    # concurrently, racing on the accumulating PSUM column.
                    # Interleave with full-K matmuls to force serialization.
                    # group 1 rows 55:64 are zero in both xtb and w_cmb, so
                    # one K=118 matmul covers even-lo + odd-lo together
                    mms = [(1, slice(0, 118), w_cmb, b),
                           (0, slice(0, 128), w_hi, b),
                           (2, slice(0, 128), w_hi, 64 + b)]
                    for dm in range(2):
                        for g2, (grp, ksl, wt, col) in enumerate(mms):
                            mm = nc.tensor.matmul(
                                v_ps[:, dm, b:b + 1],
                                xtb_s[ksl, grp, b, dm * 128:(dm + 1) * 128],
                                wt[ksl, col:col + 1],
                                start=(g2 == 0), stop=(g2 == 2))
                            # accumulation groups share one PSUM bank; keep
                            # PE order so start=True can't clear a sibling
                            # group's has_written mid-accumulation
                            if prev_mm is not None:
                                add_dep_helper(mm.ins, prev_mm.ins, sync=False,
                                               reason="psum accum order")
                            prev_mm = mm
                if DBG and s == 0:
                    allg = [(0, slice(0, 128), w_hi, lambda b: b),
                            (2, slice(0, 128), w_hi, lambda b: 64 + b),
                            (1, slice(0, 55), w_lo, lambda b: b),
                            (1, slice(64, 118), w_sh, lambda b: b)]
                    # probe 0: accumulate groups 0+1 (both K=128 base0)
                    # probe 1: accumulate groups 0+2 (K=128 + K=55)
                    # probe 2: accumulate groups 0+3 (base0 + base64)
                    # probe 3: all 4 accumulated (same as main)
                    sets = [[0, 1, 2], [0, 1, 3], [0, 2, 3], [3, 0, 1]]
                    for pi, gset in enumerate(sets):
                        v0_ps = psf.tile([128, 2, BL], F32, tag="smdbg")
                        for b in range(BL):
                            for dm in range(2):
                                for gi, g2 in enumerate(gset):
                                    grp, ksl, wt, colf = allg[g2]
                                    nc.tensor.matmul(
                                        v0_ps[:, dm, b:b + 1],
                                        xtb_s[ksl, grp, b,
                                              dm * 128:(dm + 1) * 128],
                                        wt[ksl, colf(b):colf(b) + 1],
                                        start=(gi == 0),
                                        stop=(gi == len(gset) - 1))
                        v0db = dbgp.tile([128, 2, BL], F32, tag="dbg5")
                        nc.vector.tensor_copy(v0db[:], v0_ps[:])
                        nc.sync.dma_start(dbg_v[pi][:, :, :], v0db[:])
                inpT = work.tile([128, 2, BL], BF, tag="inpT")
                nc.vector.tensor_copy(inpT[:], v_ps[:])
                if DBG and s == 0:
                    idb = dbgp.tile([128, 2, BL], F32, tag="dbgst")
                    nc.vector.tensor_copy(idb[:], v_ps[:])
                    nc.sync.dma_start(dbg_inpT[:, :, :], idb[:])

                # gates = Wih^T inp + Whh^T h + b  -> [1024, b] as 8 m-tiles
                g_ps = ps.tile([128, 512], F32, tag="sm")
                prev_mm = None
                for m in range(8):
                    sl = g_ps[:, m * 64:(m + 1) * 64]
                    gmms = [(wiht_s, inpT, 0), (wiht_s, inpT, 1),
                            (whht_s, stT, 0), (whht_s, stT, 1)]
                    for gi, (wm, xm, k) in enumerate(gmms):
                        mm = nc.tensor.matmul(
                            sl, wm[:, k, m * 128:(m + 1) * 128], xm[:, k, :],
                            start=(gi == 0), stop=(gi == 3))
                        if prev_mm is not None:
                            add_dep_helper(mm.ins, prev_mm.ins, sync=False,
                                           reason="psum accum order")
                        prev_mm = mm
                gi = work.tile([128, 2, BL], F32, tag="gi")
                gf = work.tile([128, 2, BL], F32, tag="gf")
                gg = work.tile([128, 2, BL], F32, tag="gg")
                go = work.tile([128, 2, BL], F32, tag="go")
                dest = [gi, gi, gf, gf, gg, gg, go, go]
                for m in range(8):
                    fn = AF.Tanh if m in (4, 5) else AF.Sigmoid
                    nc.scalar.activation(dest[m][:, m % 2, :],
                                         g_ps[:, m * 64:(m + 1) * 64],
                                         fn, bias=bgt_s[:, m:m + 1])
                # c = f*c + i*g ; h = o*tanh(c)
                t1 = work.tile([128, 2, BL], F32, tag="t1")
                nc.vector.tensor_tensor(t1[:], gf[:], c32[:], op=ALU.mult)
                t2 = work.tile([128, 2, BL], F32, tag="t2")
                nc.vector.tensor_tensor(t2[:], gi[:], gg[:], op=ALU.mult)
                nc.vector.tensor_tensor(c32[:], t1[:], t2[:], op=ALU.add)
                tc_ = work.tile([128, 2, BL], F32, tag="tc_")
                nc.scalar.activation(tc_[:], c32[:], AF.Tanh)
                nc.vector.tensor_tensor(outs_s[:, :, s, :], go[:], tc_[:],
                                        op=ALU.mult)
                nc.vector.tensor_copy(stT[:, 0:2, :], outs_s[:, :, s, :])
                nc.vector.tensor_copy(stT[:, 2:4, :], c32[:])
                if DBG and s == 0:
                    hdb = dbgp.tile([128, 2, BL], F32, tag="dbgst")
                    nc.vector.tensor_copy(hdb[:], outs_s[:, :, s, :])
                    nc.sync.dma_start(dbg_h[:, :, :], hdb[:])

            # ---- final MLP: td = relu(outs@Wt1+bt1)@Wt2 + bt2 ----
            h1 = state.tile([128, 2, 512], BF, tag="h1")
            for half in range(2):
                f_ps = ps.tile([128, 512], F32, tag="sm")
                for k in range(2):
                    nc.tensor.matmul(
                        f_ps[:],
                        wt1_s[:, k, :],
                        outs_s[:, k, half * 8:(half + 1) * 8, :].rearrange(
                            "p s b -> p (s b)"),
                        start=(k == 0), stop=(k == 1))
                nc.scalar.activation(h1[:, half, :], f_ps[:], AF.Relu,
                                     bias=bt1_s[:, :])
            # td via transpose trick: h1-chunks stationary, wt2 moving.
            # col j partition p -> flat = 128j+p, s = 2j + (p>=64), b = p%64
            td_ps = ps.tile([128, 8], F32, tag="sm")
            h1f = h1[:, :, :].rearrange("p h x -> p (h x)")
            for j in range(8):
                nc.tensor.matmul(td_ps[:, j:j + 1],
                                 h1f[:, 128 * j:128 * (j + 1)],
                                 wt2_s[:, :], start=True, stop=True)
            td_sb = work.tile([128, 8], F32, tag="tdsb")
            nc.vector.tensor_scalar_add(td_sb[:], td_ps[:], bt2_s[:, :])
            nc.sync.dma_start(
                out.rearrange("(j two) b -> (two b) j", two=2), td_sb[:, :])

    nc.compile()
    return nc


_NC = None


def _get_nc():
    global _NC
    if _NC is None:
        _NC = build_bass()
    return _NC


def kernel(x_cat_static, state_h, state_c, outputs_encoder,
           emb_store, emb_item, emb_family,
           Wc1, bc1, Wc2, bc2, Wc3, bc3,
           Wa1, ba1, Wa2, ba2,
           Wt1, bt1, Wt2, bt2,
           Wih, Whh, bih, bhh):
    nc = _get_nc()

    # ---- replicated (weight) tensors, shared across cores ----
    # Wc1 rows: [store 16][item 64][family 16][thought 512] -> pad each
    # embedding block to its own 128-row K-tile, thought gets 4 tiles.
    wc1p = np.zeros((7, 128, 512), ml_dtypes.bfloat16)
    wc1p[0, :16] = _bf(Wc1[0:16])
    wc1p[1, :64] = _bf(Wc1[16:80])
    wc1p[2, :16] = _bf(Wc1[80:96])
    for k in range(4):
        wc1p[3 + k] = _bf(Wc1[96 + k * 128:96 + (k + 1) * 128])

    emb_i_t = np.zeros((32, 128, 64), ml_dtypes.bfloat16)
    ei = _bf(emb_item)
    for k in range(32):
        lo = k * 128
        hi = min(4036, lo + 128)
        if lo < 4036:
            emb_i_t[k, :hi - lo] = ei[lo:hi]

    bias_g = _f32(bih + bhh)

    common = {
        "iota": np.arange(128, dtype=np.float32).reshape(128, 1),
        "emb_s": _bf(emb_store),
        "emb_i": emb_i_t,
        "emb_f": _bf(emb_family),
        "wc1p": wc1p,
        "bc1t": _f32(bc1).reshape(4, 128).T.copy(),
        "wc2": _ktile(_bf(Wc2), 512, 512),
        "bc2t": _f32(bc2).reshape(3, 128).T.copy(),
        "wc3": _ktile(_bf(Wc3), 384, 384),
        "bc3t": _f32(bc3).reshape(4, 128).T.copy(),
        "wa1a": _ktile(_bf(Wa1[:256]), 256, 256),
        "wa1b": _ktile(_bf(Wa1[256:]), 512, 512),
        "ba1t": _f32(ba1).reshape(128, 1),
        "wa2": _bf(Wa2),
        "wiht": _ktile(_bf(np.asarray(Wih).T), 256, 256),
        "whht": _ktile(_bf(np.asarray(Whh).T), 256, 256),
        "bgt": bias_g.reshape(8, 128).T.copy(),
        "wt1": _ktile(_bf(Wt1), 256, 256),
        "bt1t": _f32(bt1).reshape(128, 1),
        "wt2": _bf(Wt2),
        "bt2r": np.full((128, 1), float(np.asarray(bt2).reshape(-1)[0]),
                        np.float32),
        "id16": np.eye(128, dtype=ml_dtypes.bfloat16),
        "id32": np.eye(128, dtype=np.float32),
    }

    oe = np.asarray(outputs_encoder)
    th = np.concatenate([np.asarray(state_h)[0], np.asarray(state_c)[0]],
                        axis=-1)  # [B, 512]
    xc = np.asarray(x_cat_static)

    in_maps = []
    for c in range(N_CORES):
        b0 = c * BL
        sh = oe[:, b0:b0 + BL, :]                      # [T, BL, D]
        shb = _bf(sh)
        xtb_t = np.zeros((3, 128, BL, D), ml_dtypes.bfloat16)
        xtb_t[0] = shb[0:256:2]                        # even-hi t=0,2..254
        xtb_t[1, 0:55] = shb[256:365:2]                # even-lo t=256..364
        xtb_t[1, 64:118] = shb[257:365:2]              # odd-lo  t=257..363
        xtb_t[2] = shb[1:256:2]                        # odd-hi  t=1,3..255
        xdt_t = np.ascontiguousarray(
            shb.transpose(2, 0, 1).reshape(2, 128, T, BL))
        st_t = np.ascontiguousarray(
            _bf(th[b0:b0 + BL]).T.reshape(4, 128, BL))
        idxr = np.broadcast_to(
            xc[b0:b0 + BL].T.astype(np.float32)[None, :, :],
            (128, 3, BL)).copy()
        m = dict(common)
        m.update({"xdt": xdt_t, "xtb": xtb_t, "st0": st_t, "idxr": idxr})
        in_maps.append(m)

    kw = {}
    if os.environ.get("KTRACE", "") == "1":
        kw = dict(trace=True, trace_cores=[0])
    res = run_bass_kernel_spmd(nc, in_maps, list(range(N_CORES)), **kw)
    if res.exec_time_ns is not None:
        print("HW exec time:", res.exec_time_ns, "ns  (mean",
              res.mean_exec_time_ns, ")", flush=True)
    if res.instructions_and_trace is not None:
        import json
        insts, tracefile = res.instructions_and_trace
        print("trace file:", tracefile, flush=True)
    if res.profile_json:
        try:
            open(os.path.join(os.getcwd(), "profile.json"), "w").write(
                res.profile_json)
        except OSError:
            pass
    outs = [res.results[c]["out"] for c in range(N_CORES)]
    return np.concatenate(outs, axis=1).astype(np.float32)



# revision 11
# speedup vs baseline: 1.5374x; 1.5374x over previous
import sys, os
sys.path.insert(0, '/opt/trn_rl_repo')
import numpy as np
import ml_dtypes
import concourse.bass as bass
import concourse.bacc as bacc
import concourse.mybir as mybir
import concourse.tile as tile
from concourse.tile import add_dep_helper
from concourse.bass_utils import run_bass_kernel_spmd

BF = mybir.dt.bfloat16
F32 = mybir.dt.float32
AF = mybir.ActivationFunctionType
ALU = mybir.AluOpType
AX = mybir.AxisListType

N_CORES = 8
B = 512
BL = B // N_CORES      # 64 batch rows per core
T = 365
D = 256
FA = 128               # attention MLP hidden
NF = 16                # forecast steps
NR = 256               # decoder hidden

# max-phase split: DVE handles t[0:TDVE) in 4 chunks, Pool t[TDVE:365)
TDVE = 264
DCH = TDVE // 4        # 66


def _bf(x):
    return np.ascontiguousarray(x).astype(ml_dtypes.bfloat16)


def _f32(x):
    return np.ascontiguousarray(x).astype(np.float32)


def _ktile(w, k_total, pad_to):
    """[K, M] -> [ntiles, 128, M] zero-padded on K."""
    K, M = w.shape
    assert K == k_total
    nt = (pad_to + 127) // 128
    out = np.zeros((nt, 128, M), w.dtype)
    for i in range(nt):
        lo = i * 128
        hi = min(K, lo + 128)
        if lo < K:
            out[i, :hi - lo] = w[lo:hi]
    return out


def build_bass():
    nc = bacc.Bacc("TRN2", target_bir_lowering=False, debug=False,
                   num_devices=N_CORES)

    def inp(name, shape, dt=BF):
        return nc.dram_tensor(name, shape, dt, kind="ExternalInput")

    # per-core sharded tensors
    xdt = inp("xdt", [2, 128, T, BL])          # X^T: xdt[k,p,t,b] = X[t,b,128k+p]
    xtb = inp("xtb", [3, 128, BL, D])          # X t-grouped: [t0:128|128:256|256:365]
    st0 = inp("st0", [4, 128, BL])             # [h0;c0]^T k-tiled
    idxr = inp("idxr", [128, 3, BL], F32)      # indices replicated over partitions
    # replicated weights
    iota = inp("iota", [128, 1], F32)
    emb_s = inp("emb_s", [54, 16])
    emb_i = inp("emb_i", [32, 128, 64])
    emb_f = inp("emb_f", [33, 16])
    wc1p = inp("wc1p", [7, 128, 512])
    bc1t = inp("bc1t", [128, 4], F32)
    wc2 = inp("wc2", [4, 128, 384])
    bc2t = inp("bc2t", [128, 3], F32)
    wc3 = inp("wc3", [3, 128, 512])
    bc3t = inp("bc3t", [128, 4], F32)
    wa1a = inp("wa1a", [2, 128, 128])
    nwa1b = inp("nwa1b", [4, 128, 128])        # -Wa1[256:768] k-tiled
    nba1r = inp("nba1r", [1, 128])             # -ba1 as a row
    wa2 = inp("wa2", [128, 1])
    wihg = inp("wihg", [2, 128, 1024])         # Wih^T k-tiled, gate-scaled
    whhg = inp("whhg", [2, 128, 1024])         # Whh^T k-tiled, gate-scaled
    bgr = inp("bgr", [1, 1024])                # (bih+bhh) gate-scaled row
    wt1 = inp("wt1", [2, 128, 128])
    bt1t = inp("bt1t", [128, 1], F32)
    wt2 = inp("wt2", [128, 1])
    bt2r = inp("bt2r", [128, 1], F32)
    id16 = inp("id16", [128, 128])             # bf16 identity
    out = nc.dram_tensor("out", [NF, BL], F32, kind="ExternalOutput")

    with tile.TileContext(nc) as tc:
        with (
            tc.tile_pool(name="const", bufs=1) as cpool,
            tc.tile_pool(name="big", bufs=1) as bigpool,
            tc.tile_pool(name="state", bufs=1) as state,
            tc.tile_pool(name="ps", bufs=4, space="PSUM") as ps,
            tc.tile_pool(name="psg", bufs=1, space="PSUM") as psg,
            tc.tile_pool(name="pssc", bufs=1, space="PSUM") as pssc,
            tc.tile_pool(name="pswt", bufs=1, space="PSUM") as pswt,
        ):
            # ---- persistent constants ----
            def ld(ap_dram, shape, dt=BF, tag=None):
                t_ = cpool.tile(shape, dt, tag=tag or ap_dram.name)
                nc.sync.dma_start(t_[:], ap_dram)
                return t_

            def ldk(dram, nt, m, pool, dt=BF):
                t_ = pool.tile([128, nt, m], dt, tag=dram.name)
                for k in range(nt):
                    nc.sync.dma_start(t_[:, k, :], dram[k])
                return t_

            wa1b_s = ldk(nwa1b, 4, 128, cpool)
            wihg_s = ldk(wihg, 2, 1024, cpool)
            whhg_s = ldk(whhg, 2, 1024, cpool)
            wt1_s = ldk(wt1, 2, 128, cpool)
            nba1_s = ld(nba1r[:, :], [1, 128])
            wa2_s = ld(wa2[:, :], [128, 1])
            bgr_s = ld(bgr[:, :], [1, 1024])
            bt1_s = ld(bt1t[:, :], [128, 1], F32)
            wt2_s = ld(wt2[:, :], [128, 1])
            bt2_s = ld(bt2r[:, :], [128, 1], F32)
            id16_s = ld(id16[:, :], [128, 128])
            ones1 = cpool.tile([1, BL], BF, tag="ones1")
            nc.vector.memset(ones1[:], 1.0)

            # ---- persistent big tensors ----
            xtb_s = bigpool.tile([128, 3, BL, D], BF, tag="xtb")
            for k in range(3):
                nc.sync.dma_start(xtb_s[:, k, :, :], xtb[k])
            P_s = bigpool.tile([128, T, BL], BF, tag="P")

            # ---- persistent state ----
            outs17 = state.tile([128, 2, NF + 1, BL], BF, tag="outs")
            c32 = state.tile([128, 2, BL], F32, tag="c32")
            cb = state.tile([128, 2, BL], BF, tag="cb")
            e_sb = state.tile([64, 368], BF, tag="esb")
            nc.vector.memset(e_sb[:, T:368], 0.0)
            w_t = state.tile([128, 3, BL], BF, tag="wt")
            h1 = state.tile([128, 2, 512], BF, tag="h1")

            # ================= prologue =================
            with (
                tc.tile_pool(name="pro", bufs=1) as pro,
                tc.tile_pool(name="proh", bufs=2) as work,
                tc.tile_pool(name="xchunk", bufs=3) as xchunk,
            ):
                wa1a_s = ldk(wa1a, 2, 128, pro)
                wc1p_s = ldk(wc1p, 7, 512, pro)
                wc2_s = ldk(wc2, 4, 384, pro)
                wc3_s = ldk(wc3, 3, 512, pro)
                embi_s = ldk(emb_i, 32, 64, pro)
                bc1_s = pro.tile([128, 4], F32, tag="bc1")
                nc.sync.dma_start(bc1_s[:], bc1t[:, :])
                bc2_s = pro.tile([128, 3], F32, tag="bc2")
                nc.sync.dma_start(bc2_s[:], bc2t[:, :])
                bc3_s = pro.tile([128, 4], F32, tag="bc3")
                nc.sync.dma_start(bc3_s[:], bc3t[:, :])
                embs_s = pro.tile([54, 16], BF, tag="embs")
                nc.sync.dma_start(embs_s[:], emb_s[:, :])
                embf_s = pro.tile([33, 16], BF, tag="embf")
                nc.sync.dma_start(embf_s[:], emb_f[:, :])
                idx_s = pro.tile([128, 3, BL], F32, tag="idx")
                nc.sync.dma_start(idx_s[:], idxr[:, :, :])
                iota_s = pro.tile([128, 1], F32, tag="iota")
                nc.sync.dma_start(iota_s[:], iota[:, :])
                st0_s = ldk(st0, 4, BL, pro)

                # embeddings -> xcond tiles 0..2; thought -> 3..6
                xcond = pro.tile([128, 7, BL], BF, tag="xcond")
                nc.vector.memset(xcond[:, 0:3, :], 0.0)
                for k in range(4):
                    nc.vector.tensor_copy(xcond[:, 3 + k, :], st0_s[:, k, :])

                def onehot_embed(col, table_s, nt, width, out_slice):
                    pe_out = ps.tile([width, BL], F32, tag="sm")
                    for k in range(nt):
                        oh = work.tile([128, BL], BF, tag="oh")
                        nc.vector.scalar_tensor_tensor(
                            oh[:], idx_s[:, col, :], float(-128 * k),
                            iota_s[:, :].broadcast_to((128, BL)),
                            op0=ALU.add, op1=ALU.is_equal)
                        kk = table_s.shape[0] if nt == 1 else 128
                        lhs = table_s[:, k, :] if nt > 1 else table_s[:, :]
                        nc.tensor.matmul(pe_out[:], lhs[:kk] if nt == 1 else lhs,
                                         oh[:kk] if nt == 1 else oh[:],
                                         start=(k == 0), stop=(k == nt - 1))
                    nc.scalar.activation(out_slice, pe_out[:], AF.Copy)

                onehot_embed(0, embs_s, 1, 16, xcond[0:16, 0, :])
                onehot_embed(1, embi_s, 32, 64, xcond[0:64, 1, :])
                onehot_embed(2, embf_s, 1, 16, xcond[0:16, 2, :])

                # conditioning MLP
                def mlp_layer(w_s, nk, x_s, mt, bias_s, relu, out_t):
                    for m in range(mt):
                        pe_o = ps.tile([128, BL], F32, tag="sm")
                        for k in range(nk):
                            nc.tensor.matmul(
                                pe_o[:], w_s[:, k, m * 128:(m + 1) * 128],
                                x_s[:, k, :],
                                start=(k == 0), stop=(k == nk - 1))
                        nc.scalar.activation(
                            out_t[:, m, :], pe_o[:],
                            AF.Relu if relu else AF.Identity,
                            bias=bias_s[:, m:m + 1])

                ct1 = pro.tile([128, 4, BL], BF, tag="ct1")
                mlp_layer(wc1p_s, 7, xcond, 4, bc1_s, True, ct1)
                ct2 = pro.tile([128, 3, BL], BF, tag="ct2")
                mlp_layer(wc2_s, 4, ct1, 3, bc2_s, True, ct2)
                ct3f = pro.tile([128, 4, BL], F32, tag="ct3f")
                mlp_layer(wc3_s, 3, ct2, 4, bc3_s, False, ct3f)

                # initial state: h0 -> outs17 slot 0, c0 -> c32 + cb
                nc.scalar.activation(outs17[:, :, 0, :], ct3f[:, 0:2, :],
                                     AF.Copy)
                nc.scalar.activation(cb[:, :, :], ct3f[:, 2:4, :], AF.Copy)
                nc.vector.tensor_copy(c32[:, :, :], ct3f[:, 2:4, :])

                # P precompute: P[f, t, b] = sum_d X[t,b,d] Wa1a[d,f]
                CH = 8
                nch = (T + CH - 1) // CH  # 46; last covers 5 t
                for c in range(nch):
                    t0 = c * CH
                    tn = min(CH, T - t0)
                    xc = xchunk.tile([128, 2, CH, BL], BF, tag="xc")
                    for k in range(2):
                        nc.sync.dma_start(xc[:, k, :tn, :],
                                          xdt[k, :, t0:t0 + tn, :])
                    pe_p = ps.tile([128, CH * BL], F32, tag="sm")
                    for k in range(2):
                        nc.tensor.matmul(
                            pe_p[:, :tn * BL], wa1a_s[:, k, :],
                            xc[:, k, :tn, :].rearrange("p t b -> p (t b)"),
                            start=(k == 0), stop=(k == 1))
                    dst = P_s[:, t0:t0 + tn, :].rearrange("p t b -> p (t b)")
                    if c % 2 == 0:
                        nc.scalar.activation(dst, pe_p[:, :tn * BL], AF.Copy)
                    else:
                        nc.vector.tensor_copy(dst, pe_p[:, :tn * BL])

            # ================= decoder steps =================
            _dec_cm = tc.tile_pool(name="dec", bufs=1)
            _mch_cm = tc.tile_pool(name="mch", bufs=3)
            work = _dec_cm.__enter__()
            mpool = _mch_cm.__enter__()
            for s in range(NF):
                hs = [outs17[:, 0, s, :], outs17[:, 1, s, :],
                      cb[:, 0, :], cb[:, 1, :]]
                # nq = -(Wa1b^T [h;c] + ba1) via negated weights + bias fold
                nq_ps = ps.tile([128, BL], F32, tag="sm")
                prev = None
                for k in range(4):
                    mm = nc.tensor.matmul(nq_ps[:], wa1b_s[:, k, :], hs[k],
                                          start=(k == 0), stop=False)
                    if prev is not None:
                        add_dep_helper(mm.ins, prev.ins, sync=False,
                                       reason="psum accum order")
                    prev = mm
                mm = nc.tensor.matmul(nq_ps[:], nba1_s[0:1, :], ones1[0:1, :],
                                      start=False, stop=True)
                add_dep_helper(mm.ins, prev.ins, sync=False,
                               reason="psum accum order")
                nq_sb = work.tile([128, BL], BF, tag="nqsb")
                nc.scalar.activation(nq_sb[:], nq_ps[:], AF.Copy)

                # max phase: M = max(P, nq) ; scores[t] = wa2^T M[:, t, :]
                sc_ps = pssc.tile([64, T], F32, tag="sc")
                chunks = [(i * DCH, DCH, nc.vector) for i in range(4)]
                chunks.append((TDVE, DCH, nc.vector))
                chunks.append((TDVE + DCH, T - TDVE - DCH, nc.vector))
                for t0, tn, eng in chunks:
                    mc = mpool.tile([128, DCH, BL], BF, tag="mc")
                    eng.tensor_tensor(
                        mc[:, 0:tn, :], P_s[:, t0:t0 + tn, :],
                        nq_sb[:].unsqueeze(1).broadcast_to((128, tn, BL)),
                        op=ALU.max)
                    for jj in range(tn):
                        nc.tensor.matmul(sc_ps[:, t0 + jj:t0 + jj + 1],
                                         mc[:, jj, :], wa2_s[:, :],
                                         start=True, stop=True)

                # softmax over t (scores are O(1), skip max subtraction)
                zz = work.tile([64, 1], F32, tag="zz")
                nc.scalar.activation(e_sb[:, 0:T], sc_ps[:, :], AF.Exp,
                                     accum_out=zz[:])
                zrec = work.tile([64, 1], F32, tag="zrec")
                nc.vector.reciprocal(zrec[:], zz[:])
                nc.vector.tensor_scalar_mul(e_sb[:, 0:T], e_sb[:, 0:T],
                                            zrec[:, 0:1])

                # transpose e -> w_t [t-part, b]
                wt_ps = pswt.tile([128, 3, BL], BF, tag="wtp")
                nc.tensor.transpose(wt_ps[:, 0, :], e_sb[:, 0:128],
                                    id16_s[0:64, 0:64])
                nc.tensor.transpose(wt_ps[:, 1, :], e_sb[:, 128:256],
                                    id16_s[0:64, 0:64])
                nc.tensor.transpose(wt_ps[0:112, 2, :], e_sb[:, 256:368],
                                    id16_s[0:64, 0:64])
                nc.scalar.activation(w_t[:, 0:2, :], wt_ps[:, 0:2, :],
                                     AF.Copy)
                nc.scalar.activation(w_t[0:112, 2, :], wt_ps[0:112, 2, :],
                                     AF.Copy)

                # context: inp[d, b] = sum_t X[t, b, d] * w_t[t, b]
                v_ps = ps.tile([128, 2, BL], F32, tag="sm")
                prev = None
                groups = [(0, 128), (1, 128), (2, 109)]
                for b in range(BL):
                    for dm in range(2):
                        for gi, (g, kn) in enumerate(groups):
                            mm = nc.tensor.matmul(
                                v_ps[:, dm, b:b + 1],
                                xtb_s[0:kn, g, b, dm * 128:(dm + 1) * 128],
                                w_t[0:kn, g, b:b + 1],
                                start=(gi == 0), stop=(gi == 2))
                            if prev is not None:
                                add_dep_helper(mm.ins, prev.ins, sync=False,
                                               reason="psum accum order")
                            prev = mm
                inpT = work.tile([128, 2, BL], BF, tag="inpT")
                nc.scalar.activation(inpT[:], v_ps[:], AF.Copy)

                # gates (gate-scaled weights; bias folded): g_ps = -x|-2x
                g_ps = psg.tile([128, 512], F32, tag="g")
                prev = None
                for m in range(8):
                    sl = g_ps[:, m * 64:(m + 1) * 64]
                    gmms = [(wihg_s[:, 0, m * 128:(m + 1) * 128], inpT[:, 0, :]),
                            (wihg_s[:, 1, m * 128:(m + 1) * 128], inpT[:, 1, :]),
                            (whhg_s[:, 0, m * 128:(m + 1) * 128], hs[0]),
                            (whhg_s[:, 1, m * 128:(m + 1) * 128], hs[1]),
                            (bgr_s[0:1, m * 128:(m + 1) * 128], ones1[0:1, :])]
                    for gi, (wm, xm) in enumerate(gmms):
                        kk = 1 if gi == 4 else 128
                        mm = nc.tensor.matmul(sl, wm[0:kk] if gi == 4 else wm,
                                              xm, start=(gi == 0),
                                              stop=(gi == 4))
                        if prev is not None:
                            add_dep_helper(mm.ins, prev.ins, sync=False,
                                           reason="psum accum order")
                        prev = mm

                # ey = exp(g_ps); sigma = 1/(1+ey); tanh = 2/(1+ey)-1
                ey = work.tile([128, 512], F32, tag="ey")
                nc.scalar.activation(ey[:], g_ps[:], AF.Exp)
                sif = work.tile([128, 256], F32, tag="sif")   # i then f
                nc.vector.tensor_scalar_add(sif[:], ey[:, 0:256], 1.0)
                nc.vector.reciprocal(sif[:], sif[:])
                gt = work.tile([128, 2, BL], F32, tag="gt")
                nc.vector.tensor_scalar_add(
                    gt[:], ey[:, 256:384].rearrange("p (k b) -> p k b", k=2),
                    1.0)
                nc.vector.reciprocal(gt[:], gt[:])
                nc.vector.tensor_scalar(out=gt[:], in0=gt[:], scalar1=2.0,
                                        scalar2=-1.0, op0=ALU.mult,
                                        op1=ALU.add)
                so = work.tile([128, 2, BL], F32, tag="so")
                nc.vector.tensor_scalar_add(
                    so[:], ey[:, 384:512].rearrange("p (k b) -> p k b", k=2),
                    1.0)
                nc.vector.reciprocal(so[:], so[:])

                # c = sigma_f * c + sigma_i * tanh_g ; h = sigma_o * tanh(c)
                t1 = work.tile([128, 2, BL], F32, tag="t1")
                nc.vector.tensor_tensor(
                    t1[:], sif[:, 128:256].rearrange("p (k b) -> p k b", k=2),
                    c32[:], op=ALU.mult)
                t2 = work.tile([128, 2, BL], F32, tag="t2")
                nc.vector.tensor_tensor(
                    t2[:], sif[:, 0:128].rearrange("p (k b) -> p k b", k=2),
                    gt[:], op=ALU.mult)
                nc.vector.tensor_tensor(c32[:], t1[:], t2[:], op=ALU.add)
                ec = work.tile([128, 2, BL], F32, tag="ec")
                nc.scalar.activation(ec[:], c32[:], AF.Exp, scale=-2.0)
                nc.vector.tensor_scalar_add(ec[:], ec[:], 1.0)
                nc.vector.reciprocal(ec[:], ec[:])
                nc.vector.tensor_scalar(out=ec[:], in0=ec[:], scalar1=2.0,
                                        scalar2=-1.0, op0=ALU.mult,
                                        op1=ALU.add)
                nc.vector.tensor_tensor(outs17[:, :, s + 1, :], so[:], ec[:],
                                        op=ALU.mult)
                nc.scalar.activation(cb[:], c32[:], AF.Copy)

            # ---- final MLP: td = relu(outs@Wt1+bt1)@Wt2 + bt2 ----
            for half in range(2):
                f_ps = psg.tile([128, 512], F32, tag="g")
                for k in range(2):
                    nc.tensor.matmul(
                        f_ps[:],
                        wt1_s[:, k, :],
                        outs17[:, k, 1 + half * 8:1 + (half + 1) * 8,
                               :].rearrange("p s b -> p (s b)"),
                        start=(k == 0), stop=(k == 1))
                nc.scalar.activation(h1[:, half, :], f_ps[:], AF.Relu,
                                     bias=bt1_s[:, :])
            td_ps = ps.tile([128, 8], F32, tag="sm")
            h1f = h1[:, :, :].rearrange("p h x -> p (h x)")
            for j in range(8):
                nc.tensor.matmul(td_ps[:, j:j + 1],
                                 h1f[:, 128 * j:128 * (j + 1)],
                                 wt2_s[:, :], start=True, stop=True)
            td_sb = work.tile([128, 8], F32, tag="tdsb")
            nc.vector.tensor_scalar_add(td_sb[:], td_ps[:], bt2_s[:, 0:1])
            nc.sync.dma_start(
                out.rearrange("(j two) b -> (two b) j", two=2), td_sb[:, :])
            _mch_cm.__exit__(None, None, None)
            _dec_cm.__exit__(None, None, None)

    nc.compile()
    return nc


_NC = None


def _get_nc():
    global _NC
    if _NC is None:
        _NC = build_bass()
    return _NC


def kernel(x_cat_static, state_h, state_c, outputs_encoder,
           emb_store, emb_item, emb_family,
           Wc1, bc1, Wc2, bc2, Wc3, bc3,
           Wa1, ba1, Wa2, ba2,
           Wt1, bt1, Wt2, bt2,
           Wih, Whh, bih, bhh):
    nc = _get_nc()

    # Wc1 rows: [store 16][item 64][family 16][thought 512] -> pad each
    # embedding block to its own 128-row K-tile, thought gets 4 tiles.
    wc1p = np.zeros((7, 128, 512), ml_dtypes.bfloat16)
    wc1p[0, :16] = _bf(Wc1[0:16])
    wc1p[1, :64] = _bf(Wc1[16:80])
    wc1p[2, :16] = _bf(Wc1[80:96])
    for k in range(4):
        wc1p[3 + k] = _bf(Wc1[96 + k * 128:96 + (k + 1) * 128])

    emb_i_t = np.zeros((32, 128, 64), ml_dtypes.bfloat16)
    ei = _bf(emb_item)
    for k in range(32):
        lo = k * 128
        hi = min(4036, lo + 128)
        if lo < 4036:
            emb_i_t[k, :hi - lo] = ei[lo:hi]

    # gate scaling: sigmoid gates (i,f,o) need exp(-x); tanh gate (g) exp(-2x)
    gsc = np.full(1024, -1.0, np.float32)
    gsc[512:768] = -2.0
    wih_g = (np.asarray(Wih, np.float32) * gsc[:, None]).T   # [256, 1024]
    whh_g = (np.asarray(Whh, np.float32) * gsc[:, None]).T
    bias_g = (_f32(bih) + _f32(bhh)) * gsc

    common = {
        "iota": np.arange(128, dtype=np.float32).reshape(128, 1),
        "emb_s": _bf(emb_store),
        "emb_i": emb_i_t,
        "emb_f": _bf(emb_family),
        "wc1p": wc1p,
        "bc1t": _f32(bc1).reshape(4, 128).T.copy(),
        "wc2": _ktile(_bf(Wc2), 512, 512),
        "bc2t": _f32(bc2).reshape(3, 128).T.copy(),
        "wc3": _ktile(_bf(Wc3), 384, 384),
        "bc3t": _f32(bc3).reshape(4, 128).T.copy(),
        "wa1a": _ktile(_bf(Wa1[:256]), 256, 256),
        "nwa1b": _ktile(_bf(-np.asarray(Wa1[256:], np.float32)), 512, 512),
        "nba1r": _bf(-_f32(ba1)).reshape(1, 128),
        "wa2": _bf(Wa2),
        "wihg": _ktile(_bf(wih_g), 256, 256),
        "whhg": _ktile(_bf(whh_g), 256, 256),
        "bgr": _bf(bias_g).reshape(1, 1024),
        "wt1": _ktile(_bf(Wt1), 256, 256),
        "bt1t": _f32(bt1).reshape(128, 1),
        "wt2": _bf(Wt2),
        "bt2r": np.full((128, 1), float(np.asarray(bt2).reshape(-1)[0]),
                        np.float32),
        "id16": np.eye(128, dtype=ml_dtypes.bfloat16),
    }

    oe = np.asarray(outputs_encoder)
    th = np.concatenate([np.asarray(state_h)[0], np.asarray(state_c)[0]],
                        axis=-1)  # [B, 512]
    xc = np.asarray(x_cat_static)

    in_maps = []
    for c in range(N_CORES):
        b0 = c * BL
        shb = _bf(oe[:, b0:b0 + BL, :])                # [T, BL, D]
        xtb_t = np.zeros((3, 128, BL, D), ml_dtypes.bfloat16)
        xtb_t[0] = shb[0:128]
        xtb_t[1] = shb[128:256]
        xtb_t[2, 0:109] = shb[256:365]
        xdt_t = np.ascontiguousarray(
            shb.transpose(2, 0, 1).reshape(2, 128, T, BL))
        st_t = np.ascontiguousarray(
            _bf(th[b0:b0 + BL]).T.reshape(4, 128, BL))
        idxr = np.broadcast_to(
            xc[b0:b0 + BL].T.astype(np.float32)[None, :, :],
            (128, 3, BL)).copy()
        m = dict(common)
        m.update({"xdt": xdt_t, "xtb": xtb_t, "st0": st_t, "idxr": idxr})
        in_maps.append(m)

    kw = {}
    if os.environ.get("KTRACE", "") == "1":
        kw = dict(trace=True, trace_cores=[0])
    res = run_bass_kernel_spmd(nc, in_maps, list(range(N_CORES)), **kw)
    if res.exec_time_ns is not None:
        print("HW exec time:", res.exec_time_ns, "ns  (mean",
              res.mean_exec_time_ns, ")", flush=True)
    outs = [res.results[c]["out"] for c in range(N_CORES)]
    return np.concatenate(outs, axis=1).astype(np.float32)


# revision 23
# speedup vs baseline: 1.7400x; 1.1318x over previous
import sys, os
sys.path.insert(0, '/opt/trn_rl_repo')
import numpy as np
import ml_dtypes
import concourse.bass as bass
import concourse.bacc as bacc
import concourse.mybir as mybir
import concourse.tile as tile
from concourse.tile import add_dep_helper
from concourse.bass_utils import run_bass_kernel_spmd

BF = mybir.dt.bfloat16
F32 = mybir.dt.float32
AF = mybir.ActivationFunctionType
ALU = mybir.AluOpType
AX = mybir.AxisListType

N_CORES = 8
B = 512
BL = B // N_CORES      # 64 batch rows per core
T = 365
D = 256
FA = 128               # attention MLP hidden
NF = 16                # forecast steps
NR = 256               # decoder hidden

# max-phase split: DVE handles t[0:TDVE) in 4 chunks, Pool t[TDVE:365)
TDVE = 264
DCH = TDVE // 4        # 66


def _bf(x):
    return np.ascontiguousarray(x).astype(ml_dtypes.bfloat16)


def _f32(x):
    return np.ascontiguousarray(x).astype(np.float32)


def _ktile(w, k_total, pad_to):
    """[K, M] -> [ntiles, 128, M] zero-padded on K."""
    K, M = w.shape
    assert K == k_total
    nt = (pad_to + 127) // 128
    out = np.zeros((nt, 128, M), w.dtype)
    for i in range(nt):
        lo = i * 128
        hi = min(K, lo + 128)
        if lo < K:
            out[i, :hi - lo] = w[lo:hi]
    return out


def build_bass():
    nc = bacc.Bacc("TRN2", target_bir_lowering=False, debug=False,
                   num_devices=N_CORES)

    def inp(name, shape, dt=BF):
        return nc.dram_tensor(name, shape, dt, kind="ExternalInput")

    # per-core sharded tensors
    xdt = inp("xdt", [2, 128, T, BL])          # X^T: xdt[k,p,t,b] = X[t,b,128k+p]
    xtb = inp("xtb", [3, 128, BL, D])          # X t-grouped: [t0:128|128:256|256:365]
    st0 = inp("st0", [4, 128, BL])             # [h0;c0]^T k-tiled
    idxr = inp("idxr", [128, 3, BL], F32)      # indices replicated over partitions
    # replicated weights
    iota = inp("iota", [128, 1], F32)
    emb_s = inp("emb_s", [54, 16])
    emb_i = inp("emb_i", [32, 128, 64])
    emb_f = inp("emb_f", [33, 16])
    wc1p = inp("wc1p", [7, 128, 512])
    bc1t = inp("bc1t", [128, 4], F32)
    wc2 = inp("wc2", [4, 128, 384])
    bc2t = inp("bc2t", [128, 3], F32)
    wc3 = inp("wc3", [3, 128, 512])
    bc3t = inp("bc3t", [128, 4], F32)
    wa1a = inp("wa1a", [2, 128, 128])
    nwa1b = inp("nwa1b", [4, 128, 128])        # -Wa1[256:768] k-tiled
    nba1r = inp("nba1r", [1, 128])             # -ba1 as a row
    wa2 = inp("wa2", [128, 1])
    wihg = inp("wihg", [2, 128, 1024])         # Wih^T k-tiled, gate-scaled
    whhg = inp("whhg", [2, 128, 1024])         # Whh^T k-tiled, gate-scaled
    bgr = inp("bgr", [1, 1024])                # (bih+bhh) gate-scaled row
    wt1 = inp("wt1", [2, 128, 128])
    bt1t = inp("bt1t", [128, 1], F32)
    wt2 = inp("wt2", [128, 1])
    bt2r = inp("bt2r", [128, 1], F32)
    id16 = inp("id16", [128, 128])             # bf16 identity
    out = nc.dram_tensor("out", [NF, BL], F32, kind="ExternalOutput")

    with tile.TileContext(nc) as tc:
        with (
            tc.tile_pool(name="const", bufs=1) as cpool,
            tc.tile_pool(name="big", bufs=1) as bigpool,
            tc.tile_pool(name="state", bufs=1) as state,
            tc.tile_pool(name="ps", bufs=4, space="PSUM") as ps,
            tc.tile_pool(name="psg", bufs=1, space="PSUM") as psg,
            tc.tile_pool(name="pssc", bufs=1, space="PSUM") as pssc,
            tc.tile_pool(name="pswt", bufs=1, space="PSUM") as pswt,
        ):
            # ---- persistent constants (issued from Act HWDGE; SP is busy
            # streaming xdt and big xtb transfers block their issuing seq) ----
            def ld(ap_dram, shape, dt=BF, tag=None):
                t_ = cpool.tile(shape, dt, tag=tag or ap_dram.name)
                nc.scalar.dma_start(t_[:], ap_dram)
                return t_

            def ldk(dram, nt, m, pool, dt=BF):
                t_ = pool.tile([128, nt, m], dt, tag=dram.name)
                nc.scalar.dma_start(
                    t_[:, :, :], dram[:, :, :].rearrange("n p m -> p n m"))
                return t_

            ones1 = cpool.tile([1, BL], BF, tag="ones1")
            nc.vector.memset(ones1[:], 1.0)

            # ---- persistent big tensors ----
            # xtb via Pool SWDGE in halves: cheap issue, parallel queues
            xtb_s = bigpool.tile([128, 3, BL, D], BF, tag="xtb")
            for k in range(3):
                for hb in range(2):
                    nc.gpsimd.dma_start(
                        xtb_s[:, k, hb * 32:(hb + 1) * 32, :],
                        xtb[k, :, hb * 32:(hb + 1) * 32, :])
            P_s = bigpool.tile([128, T, BL], BF, tag="P")

            # ---- persistent state ----
            outs17 = state.tile([128, 2, NF + 1, BL], BF, tag="outs")
            c32 = state.tile([128, 2, BL], F32, tag="c32")
            cb = state.tile([128, 2, BL], BF, tag="cb")
            e_sb = state.tile([64, 368], BF, tag="esb")
            nc.vector.memset(e_sb[:, T:368], 0.0)
            w_t = state.tile([128, 3, BL], BF, tag="wt")
            h1 = state.tile([128, 2, 512], BF, tag="h1")

            # ================= prologue =================
            with (
                tc.tile_pool(name="pro", bufs=1) as pro,
                tc.tile_pool(name="proh", bufs=2) as work,
                tc.tile_pool(name="xchunk", bufs=2) as xchunk,
            ):
                idx_s = pro.tile([128, 3, BL], F32, tag="idx")
                nc.scalar.dma_start(idx_s[:], idxr[:, :, :])
                iota_s = pro.tile([128, 1], F32, tag="iota")
                nc.scalar.dma_start(iota_s[:], iota[:, :])
                st0_s = ldk(st0, 4, BL, pro)
                embs_s = pro.tile([54, 16], BF, tag="embs")
                nc.scalar.dma_start(embs_s[:], emb_s[:, :])
                embf_s = pro.tile([33, 16], BF, tag="embf")
                nc.scalar.dma_start(embf_s[:], emb_f[:, :])
                embi_s = ldk(emb_i, 32, 64, pro)
                wa1a_s = ldk(wa1a, 2, 128, pro)
                wc1p_s = ldk(wc1p, 7, 512, pro)
                bc1_s = pro.tile([128, 4], F32, tag="bc1")
                nc.scalar.dma_start(bc1_s[:], bc1t[:, :])
                wc2_s = ldk(wc2, 4, 384, pro)
                bc2_s = pro.tile([128, 3], F32, tag="bc2")
                nc.scalar.dma_start(bc2_s[:], bc2t[:, :])
                wc3_s = ldk(wc3, 3, 512, pro)
                bc3_s = pro.tile([128, 4], F32, tag="bc3")
                nc.scalar.dma_start(bc3_s[:], bc3t[:, :])
                # decoder weights (needed from ~step 0 on; issue after the
                # embedding/cond-MLP path consts)
                wa1b_s = ldk(nwa1b, 4, 128, cpool)
                nba1_s = ld(nba1r[:, :], [1, 128])
                wa2_s = ld(wa2[:, :], [128, 1])
                id16_s = ld(id16[:, :], [128, 128])
                wihg_s = ldk(wihg, 2, 1024, cpool)
                whhg_s = ldk(whhg, 2, 1024, cpool)
                bgr_s = ld(bgr[:, :], [1, 1024])
                wt1_s = ldk(wt1, 2, 128, cpool)
                bt1_s = ld(bt1t[:, :], [128, 1], F32)
                wt2_s = ld(wt2[:, :], [128, 1])
                bt2_s = ld(bt2r[:, :], [128, 1], F32)

                # embeddings -> xcond tiles 0..2; thought -> 3..6
                xcond = pro.tile([128, 7, BL], BF, tag="xcond")
                nc.vector.memset(xcond[:, 0:3, :], 0.0)
                for k in range(4):
                    nc.vector.tensor_copy(xcond[:, 3 + k, :], st0_s[:, k, :])

                def onehot_embed(col, table_s, nt, width, out_slice):
                    pe_out = ps.tile([width, BL], F32, tag="sm")
                    for k in range(nt):
                        oh = work.tile([128, BL], BF, tag="oh")
                        nc.vector.scalar_tensor_tensor(
                            oh[:], idx_s[:, col, :], float(-128 * k),
                            iota_s[:, :].broadcast_to((128, BL)),
                            op0=ALU.add, op1=ALU.is_equal)
                        kk = table_s.shape[0] if nt == 1 else 128
                        lhs = table_s[:, k, :] if nt > 1 else table_s[:, :]
                        nc.tensor.matmul(pe_out[:], lhs[:kk] if nt == 1 else lhs,
                                         oh[:kk] if nt == 1 else oh[:],
                                         start=(k == 0), stop=(k == nt - 1))
                    nc.scalar.activation(out_slice, pe_out[:], AF.Copy)

                onehot_embed(0, embs_s, 1, 16, xcond[0:16, 0, :])
                onehot_embed(1, embi_s, 32, 64, xcond[0:64, 1, :])
                onehot_embed(2, embf_s, 1, 16, xcond[0:16, 2, :])

                # conditioning MLP
                def mlp_layer(w_s, nk, x_s, mt, bias_s, relu, out_t):
                    for m in range(mt):
                        pe_o = ps.tile([128, BL], F32, tag="sm")
                        for k in range(nk):
                            nc.tensor.matmul(
                                pe_o[:], w_s[:, k, m * 128:(m + 1) * 128],
                                x_s[:, k, :],
                                start=(k == 0), stop=(k == nk - 1))
                        nc.scalar.activation(
                            out_t[:, m, :], pe_o[:],
                            AF.Relu if relu else AF.Identity,
                            bias=bias_s[:, m:m + 1])

                ct1 = pro.tile([128, 4, BL], BF, tag="ct1")
                mlp_layer(wc1p_s, 7, xcond, 4, bc1_s, True, ct1)
                ct2 = pro.tile([128, 3, BL], BF, tag="ct2")
                mlp_layer(wc2_s, 4, ct1, 3, bc2_s, True, ct2)
                ct3f = pro.tile([128, 4, BL], F32, tag="ct3f")
                mlp_layer(wc3_s, 3, ct2, 4, bc3_s, False, ct3f)

                # initial state: h0 -> outs17 slot 0, c0 -> c32 + cb
                nc.scalar.activation(outs17[:, :, 0, :], ct3f[:, 0:2, :],
                                     AF.Copy)
                nc.scalar.activation(cb[:, :, :], ct3f[:, 2:4, :], AF.Copy)
                nc.vector.tensor_copy(c32[:, :, :], ct3f[:, 2:4, :])

                # P precompute: P[f, t, b] = sum_d X[t,b,d] Wa1a[d,f]
                # xdt streamed in big staged pieces (few DMAs; SP issues),
                # matmul'd in 8-t psum sub-chunks.
                PT = 46
                CH = 8
                cc = 0
                t0 = 0
                while t0 < T:
                    pn = min(PT, T - t0)
                    xc = xchunk.tile([128, 2, PT, BL], BF, tag="xc")
                    for k in range(2):
                        nc.sync.dma_start(xc[:, k, :pn, :],
                                          xdt[k, :, t0:t0 + pn, :])
                    u0 = 0
                    while u0 < pn:
                        tn = min(CH, pn - u0)
                        pe_p = ps.tile([128, CH * BL], F32, tag="sm")
                        for k in range(2):
                            nc.tensor.matmul(
                                pe_p[:, :tn * BL], wa1a_s[:, k, :],
                                xc[:, k, u0:u0 + tn, :].rearrange(
                                    "p t b -> p (t b)"),
                                start=(k == 0), stop=(k == 1))
                        dst = P_s[:, t0 + u0:t0 + u0 + tn, :].rearrange(
                            "p t b -> p (t b)")
                        if cc % 2 == 0:
                            nc.scalar.activation(dst, pe_p[:, :tn * BL],
                                                 AF.Copy)
                        else:
                            nc.vector.tensor_copy(dst, pe_p[:, :tn * BL])
                        cc += 1
                        u0 += tn
                    t0 += pn

            # ================= decoder steps =================
            _dec_cm = tc.tile_pool(name="dec", bufs=1)
            _mch_cm = tc.tile_pool(name="mch", bufs=3)
            work = _dec_cm.__enter__()
            mpool = _mch_cm.__enter__()
            for s in range(NF):
                hs = [outs17[:, 0, s, :], outs17[:, 1, s, :],
                      cb[:, 0, :], cb[:, 1, :]]
                # nq = -(Wa1b^T [h;c] + ba1) via negated weights + bias fold.
                # c-dependent terms first: cb is ready before h each step.
                nq_ps = ps.tile([128, BL], F32, tag="sm")
                nq_mms = [(nba1_s[0:1, :], ones1[0:1, :]),
                          (wa1b_s[:, 2, :], hs[2]),
                          (wa1b_s[:, 3, :], hs[3]),
                          (wa1b_s[:, 0, :], hs[0]),
                          (wa1b_s[:, 1, :], hs[1])]
                prev = None
                for gi, (wm, xm) in enumerate(nq_mms):
                    mm = nc.tensor.matmul(nq_ps[:], wm, xm,
                                          start=(gi == 0), stop=(gi == 4))
                    if prev is not None:
                        add_dep_helper(mm.ins, prev.ins, sync=False,
                                       reason="psum accum order")
                    prev = mm
                nq_sb = work.tile([128, BL], BF, tag="nqsb")
                nc.scalar.activation(nq_sb[:], nq_ps[:], AF.Copy)

                # max phase: M = max(P, nq) ; scores[t] = wa2^T M[:, t, :]
                sc_ps = pssc.tile([64, T], F32, tag="sc")
                chunks = [(i * DCH, DCH, nc.vector) for i in range(4)]
                chunks.append((TDVE, DCH, nc.vector))
                chunks.append((TDVE + DCH, T - TDVE - DCH, nc.vector))
                for t0, tn, eng in chunks:
                    mc = mpool.tile([128, DCH, BL], BF, tag="mc")
                    eng.tensor_tensor(
                        mc[:, 0:tn, :], P_s[:, t0:t0 + tn, :],
                        nq_sb[:].unsqueeze(1).broadcast_to((128, tn, BL)),
                        op=ALU.max)
                    for jj in range(tn):
                        nc.tensor.matmul(sc_ps[:, t0 + jj:t0 + jj + 1],
                                         mc[:, jj, :], wa2_s[:, :],
                                         start=True, stop=True)

                # softmax over t (scores are O(1), skip max subtraction)
                zz = work.tile([64, 1], F32, tag="zz")
                nc.scalar.activation(e_sb[:, 0:T], sc_ps[:, :], AF.Exp,
                                     accum_out=zz[:])
                zrec = work.tile([64, 1], F32, tag="zrec")
                nc.vector.reciprocal(zrec[:], zz[:])
                nc.vector.tensor_scalar_mul(e_sb[:, 0:T], e_sb[:, 0:T],
                                            zrec[:, 0:1])

                # transpose e -> w_t [t-part, b]
                wt_ps = pswt.tile([128, 3, BL], BF, tag="wtp")
                nc.tensor.transpose(wt_ps[:, 0, :], e_sb[:, 0:128],
                                    id16_s[0:64, 0:64])
                nc.tensor.transpose(wt_ps[:, 1, :], e_sb[:, 128:256],
                                    id16_s[0:64, 0:64])
                nc.tensor.transpose(wt_ps[0:112, 2, :], e_sb[:, 256:368],
                                    id16_s[0:64, 0:64])
                nc.scalar.activation(w_t[:, 0:2, :], wt_ps[:, 0:2, :],
                                     AF.Copy)
                nc.scalar.activation(w_t[0:112, 2, :], wt_ps[0:112, 2, :],
                                     AF.Copy)

                # context: inp[d, b] = sum_t X[t, b, d] * w_t[t, b]
                v_ps = ps.tile([128, 2, BL], F32, tag="sm")
                prev = None
                groups = [(0, 128), (1, 128), (2, 109)]
                for b in range(BL):
                    for dm in range(2):
                        for gi, (g, kn) in enumerate(groups):
                            mm = nc.tensor.matmul(
                                v_ps[:, dm, b:b + 1],
                                xtb_s[0:kn, g, b, dm * 128:(dm + 1) * 128],
                                w_t[0:kn, g, b:b + 1],
                                start=(gi == 0), stop=(gi == 2))
                            if prev is not None:
                                add_dep_helper(mm.ins, prev.ins, sync=False,
                                               reason="psum accum order")
                            prev = mm
                inpT = work.tile([128, 2, BL], BF, tag="inpT")
                nc.scalar.activation(inpT[:], v_ps[:], AF.Copy)

                # gates (gate-scaled weights; bias folded): g_ps = -x|-2x
                g_ps = psg.tile([128, 512], F32, tag="g")
                prev = None
                for m in range(8):
                    sl = g_ps[:, m * 64:(m + 1) * 64]
                    # h-dependent terms first; inpT (latest) last
                    gmms = [(bgr_s[0:1, m * 128:(m + 1) * 128], ones1[0:1, :]),
                            (whhg_s[:, 0, m * 128:(m + 1) * 128], hs[0]),
                            (whhg_s[:, 1, m * 128:(m + 1) * 128], hs[1]),
                            (wihg_s[:, 0, m * 128:(m + 1) * 128], inpT[:, 0, :]),
                            (wihg_s[:, 1, m * 128:(m + 1) * 128], inpT[:, 1, :])]
                    for gi, (wm, xm) in enumerate(gmms):
                        mm = nc.tensor.matmul(sl, wm, xm, start=(gi == 0),
                                              stop=(gi == 4))
                        if prev is not None:
                            add_dep_helper(mm.ins, prev.ins, sync=False,
                                           reason="psum accum order")
                        prev = mm

                # ey = exp(g_ps); r = 1/(1+ey): sigma for i,f,o; tanh_g = 2r-1
                ey = work.tile([128, 512], F32, tag="ey")
                nc.scalar.activation(ey[:], g_ps[:], AF.Exp)
                nc.vector.tensor_scalar_add(ey[:], ey[:], 1.0)
                nc.vector.reciprocal(ey[:], ey[:])
                gt = work.tile([128, 2, BL], F32, tag="gt")
                nc.vector.tensor_scalar(
                    out=gt[:],
                    in0=ey[:, 256:384].rearrange("p (k b) -> p k b", k=2),
                    scalar1=2.0, scalar2=-1.0, op0=ALU.mult, op1=ALU.add)

                # c = sigma_f * c + sigma_i * tanh_g ; h = sigma_o * tanh(c)
                t1 = work.tile([128, 2, BL], F32, tag="t1")
                nc.vector.tensor_tensor(
                    t1[:], ey[:, 128:256].rearrange("p (k b) -> p k b", k=2),
                    c32[:], op=ALU.mult)
                t2 = work.tile([128, 2, BL], F32, tag="t2")
                nc.vector.tensor_tensor(
                    t2[:], ey[:, 0:128].rearrange("p (k b) -> p k b", k=2),
                    gt[:], op=ALU.mult)
                nc.vector.tensor_tensor(c32[:], t1[:], t2[:], op=ALU.add)
                ec = work.tile([128, 2, BL], F32, tag="ec")
                nc.scalar.activation(ec[:], c32[:], AF.Exp, scale=-2.0)
                nc.vector.tensor_scalar_add(ec[:], ec[:], 1.0)
                nc.vector.reciprocal(ec[:], ec[:])
                nc.vector.tensor_scalar(out=ec[:], in0=ec[:], scalar1=2.0,
                                        scalar2=-1.0, op0=ALU.mult,
                                        op1=ALU.add)
                nc.vector.tensor_tensor(
                    outs17[:, :, s + 1, :],
                    ey[:, 384:512].rearrange("p (k b) -> p k b", k=2),
                    ec[:], op=ALU.mult)
                nc.scalar.activation(cb[:], c32[:], AF.Copy)

            # ---- final MLP: td = relu(outs@Wt1+bt1)@Wt2 + bt2 ----
            for half in range(2):
                f_ps = psg.tile([128, 512], F32, tag="g")
                for k in range(2):
                    nc.tensor.matmul(
                        f_ps[:],
                        wt1_s[:, k, :],
                        outs17[:, k, 1 + half * 8:1 + (half + 1) * 8,
                               :].rearrange("p s b -> p (s b)"),
                        start=(k == 0), stop=(k == 1))
                nc.scalar.activation(h1[:, half, :], f_ps[:], AF.Relu,
                                     bias=bt1_s[:, :])
            td_ps = ps.tile([128, 8], F32, tag="sm")
            h1f = h1[:, :, :].rearrange("p h x -> p (h x)")
            for j in range(8):
                nc.tensor.matmul(td_ps[:, j:j + 1],
                                 h1f[:, 128 * j:128 * (j + 1)],
                                 wt2_s[:, :], start=True, stop=True)
            td_sb = work.tile([128, 8], F32, tag="tdsb")
            nc.vector.tensor_scalar_add(td_sb[:], td_ps[:], bt2_s[:, 0:1])
            nc.sync.dma_start(
                out.rearrange("(j two) b -> (two b) j", two=2), td_sb[:, :])
            _mch_cm.__exit__(None, None, None)
            _dec_cm.__exit__(None, None, None)

    nc.compile()
    return nc


_NC = None


def _get_nc():
    global _NC
    if _NC is None:
        _NC = build_bass()
    return _NC


def kernel(x_cat_static, state_h, state_c, outputs_encoder,
           emb_store, emb_item, emb_family,
           Wc1, bc1, Wc2, bc2, Wc3, bc3,
           Wa1, ba1, Wa2, ba2,
           Wt1, bt1, Wt2, bt2,
           Wih, Whh, bih, bhh):
    nc = _get_nc()

    # Wc1 rows: [store 16][item 64][family 16][thought 512] -> pad each
    # embedding block to its own 128-row K-tile, thought gets 4 tiles.
    wc1p = np.zeros((7, 128, 512), ml_dtypes.bfloat16)
    wc1p[0, :16] = _bf(Wc1[0:16])
    wc1p[1, :64] = _bf(Wc1[16:80])
    wc1p[2, :16] = _bf(Wc1[80:96])
    for k in range(4):
        wc1p[3 + k] = _bf(Wc1[96 + k * 128:96 + (k + 1) * 128])

    emb_i_t = np.zeros((32, 128, 64), ml_dtypes.bfloat16)
    ei = _bf(emb_item)
    for k in range(32):
        lo = k * 128
        hi = min(4036, lo + 128)
        if lo < 4036:
            emb_i_t[k, :hi - lo] = ei[lo:hi]

    # gate scaling: sigmoid gates (i,f,o) need exp(-x); tanh gate (g) exp(-2x)
    gsc = np.full(1024, -1.0, np.float32)
    gsc[512:768] = -2.0
    wih_g = (np.asarray(Wih, np.float32) * gsc[:, None]).T   # [256, 1024]
    whh_g = (np.asarray(Whh, np.float32) * gsc[:, None]).T
    bias_g = (_f32(bih) + _f32(bhh)) * gsc

    common = {
        "iota": np.arange(128, dtype=np.float32).reshape(128, 1),
        "emb_s": _bf(emb_store),
        "emb_i": emb_i_t,
        "emb_f": _bf(emb_family),
        "wc1p": wc1p,
        "bc1t": _f32(bc1).reshape(4, 128).T.copy(),
        "wc2": _ktile(_bf(Wc2), 512, 512),
        "bc2t": _f32(bc2).reshape(3, 128).T.copy(),
        "wc3": _ktile(_bf(Wc3), 384, 384),
        "bc3t": _f32(bc3).reshape(4, 128).T.copy(),
        "wa1a": _ktile(_bf(Wa1[:256]), 256, 256),
        "nwa1b": _ktile(_bf(-np.asarray(Wa1[256:], np.float32)), 512, 512),
        "nba1r": _bf(-_f32(ba1)).reshape(1, 128),
        "wa2": _bf(Wa2),
        "wihg": _ktile(_bf(wih_g), 256, 256),
        "whhg": _ktile(_bf(whh_g), 256, 256),
        "bgr": _bf(bias_g).reshape(1, 1024),
        "wt1": _ktile(_bf(Wt1), 256, 256),
        "bt1t": _f32(bt1).reshape(128, 1),
        "wt2": _bf(Wt2),
        "bt2r": np.full((128, 1), float(np.asarray(bt2).reshape(-1)[0]),
                        np.float32),
        "id16": np.eye(128, dtype=ml_dtypes.bfloat16),
    }

    oe = np.asarray(outputs_encoder)
    th = np.concatenate([np.asarray(state_h)[0], np.asarray(state_c)[0]],
                        axis=-1)  # [B, 512]
    xc = np.asarray(x_cat_static)

    in_maps = []
    for c in range(N_CORES):
        b0 = c * BL
        shb = _bf(oe[:, b0:b0 + BL, :])                # [T, BL, D]
        xtb_t = np.zeros((3, 128, BL, D), ml_dtypes.bfloat16)
        xtb_t[0] = shb[0:128]
        xtb_t[1] = shb[128:256]
        xtb_t[2, 0:109] = shb[256:365]
        xdt_t = np.ascontiguousarray(
            shb.transpose(2, 0, 1).reshape(2, 128, T, BL))
        st_t = np.ascontiguousarray(
            _bf(th[b0:b0 + BL]).T.reshape(4, 128, BL))
        idxr = np.broadcast_to(
            xc[b0:b0 + BL].T.astype(np.float32)[None, :, :],
            (128, 3, BL)).copy()
        m = dict(common)
        m.update({"xdt": xdt_t, "xtb": xtb_t, "st0": st_t, "idxr": idxr})
        in_maps.append(m)

    kw = {}
    if os.environ.get("KTRACE", "") == "1":
        kw = dict(trace=True, trace_cores=[0])
    res = run_bass_kernel_spmd(nc, in_maps, list(range(N_CORES)), **kw)
    if res.exec_time_ns is not None:
        print("HW exec time:", res.exec_time_ns, "ns  (mean",
              res.mean_exec_time_ns, ")", flush=True)
    outs = [res.results[c]["out"] for c in range(N_CORES)]
    return np.concatenate(outs, axis=1).astype(np.float32)


# revision 24
# speedup vs baseline: 1.7857x; 1.0262x over previous
import sys, os
sys.path.insert(0, '/opt/trn_rl_repo')
import numpy as np
import ml_dtypes
import concourse.bass as bass
import concourse.bacc as bacc
import concourse.mybir as mybir
import concourse.tile as tile
from concourse.tile import add_dep_helper
from concourse.bass_utils import run_bass_kernel_spmd

BF = mybir.dt.bfloat16
F32 = mybir.dt.float32
F8 = mybir.dt.float8e4
AF = mybir.ActivationFunctionType
ALU = mybir.AluOpType
AX = mybir.AxisListType

N_CORES = 8
B = 512
BL = B // N_CORES      # 64 batch rows per core
T = 365
D = 256
FA = 128               # attention MLP hidden
NF = 16                # forecast steps
NR = 256               # decoder hidden

# max-phase split: DVE handles t[0:TDVE) in 4 chunks, Pool t[TDVE:365)
TDVE = 264
DCH = TDVE // 4        # 66


def _bf(x):
    return np.ascontiguousarray(x).astype(ml_dtypes.bfloat16)


def _f32(x):
    return np.ascontiguousarray(x).astype(np.float32)


def _ktile(w, k_total, pad_to):
    """[K, M] -> [ntiles, 128, M] zero-padded on K."""
    K, M = w.shape
    assert K == k_total
    nt = (pad_to + 127) // 128
    out = np.zeros((nt, 128, M), w.dtype)
    for i in range(nt):
        lo = i * 128
        hi = min(K, lo + 128)
        if lo < K:
            out[i, :hi - lo] = w[lo:hi]
    return out


def build_bass():
    nc = bacc.Bacc("TRN2", target_bir_lowering=False, debug=False,
                   num_devices=N_CORES)

    def inp(name, shape, dt=BF):
        return nc.dram_tensor(name, shape, dt, kind="ExternalInput")

    # per-core sharded tensors
    xdt = inp("xdt", [2, 128, T, BL], F8)          # X^T: xdt[k,p,t,b] = X[t,b,128k+p]
    xtb = inp("xtb", [3, 128, BL, D])          # X t-grouped: [t0:128|128:256|256:365]
    st0 = inp("st0", [4, 128, BL])             # [h0;c0]^T k-tiled
    idxr = inp("idxr", [128, 3, BL], F32)      # indices replicated over partitions
    # replicated weights
    iota = inp("iota", [128, 1], F32)
    emb_s = inp("emb_s", [54, 16])
    emb_i = inp("emb_i", [32, 128, 64])
    emb_f = inp("emb_f", [33, 16])
    wc1p = inp("wc1p", [7, 128, 512])
    bc1t = inp("bc1t", [128, 4], F32)
    wc2 = inp("wc2", [4, 128, 384])
    bc2t = inp("bc2t", [128, 3], F32)
    wc3 = inp("wc3", [3, 128, 512])
    bc3t = inp("bc3t", [128, 4], F32)
    wa1a = inp("wa1a", [2, 128, 128], F8)
    nwa1b = inp("nwa1b", [4, 128, 128])        # -Wa1[256:768] k-tiled
    nba1r = inp("nba1r", [1, 128])             # -ba1 as a row
    wa2 = inp("wa2", [128, 1])
    wihg = inp("wihg", [2, 128, 1024])         # Wih^T k-tiled, gate-scaled
    whhg = inp("whhg", [2, 128, 1024])         # Whh^T k-tiled, gate-scaled
    bgr = inp("bgr", [1, 1024])                # (bih+bhh) gate-scaled row
    wt1 = inp("wt1", [2, 128, 128])
    bt1t = inp("bt1t", [128, 1], F32)
    wt2 = inp("wt2", [128, 1])
    bt2r = inp("bt2r", [128, 1], F32)
    id16 = inp("id16", [128, 128])             # bf16 identity
    out = nc.dram_tensor("out", [NF, BL], F32, kind="ExternalOutput")

    with tile.TileContext(nc) as tc:
        with (
            tc.tile_pool(name="const", bufs=1) as cpool,
            tc.tile_pool(name="big", bufs=1) as bigpool,
            tc.tile_pool(name="state", bufs=1) as state,
            tc.tile_pool(name="ps", bufs=4, space="PSUM") as ps,
            tc.tile_pool(name="psg", bufs=1, space="PSUM") as psg,
            tc.tile_pool(name="pssc", bufs=1, space="PSUM") as pssc,
            tc.tile_pool(name="pswt", bufs=1, space="PSUM") as pswt,
        ):
            # ---- persistent constants (issued from Act HWDGE; SP is busy
            # streaming xdt and big xtb transfers block their issuing seq) ----
            def ld(ap_dram, shape, dt=BF, tag=None):
                t_ = cpool.tile(shape, dt, tag=tag or ap_dram.name)
                nc.scalar.dma_start(t_[:], ap_dram)
                return t_

            def ldk(dram, nt, m, pool, dt=BF):
                t_ = pool.tile([128, nt, m], dt, tag=dram.name)
                nc.scalar.dma_start(
                    t_[:, :, :], dram[:, :, :].rearrange("n p m -> p n m"))
                return t_

            ones1 = cpool.tile([1, BL], BF, tag="ones1")
            nc.vector.memset(ones1[:], 1.0)

            # ---- persistent big tensors ----
            # xtb via Pool SWDGE in halves: cheap issue, parallel queues
            xtb_s = bigpool.tile([128, 3, BL, D], BF, tag="xtb")
            for k in range(3):
                for hb in range(2):
                    nc.gpsimd.dma_start(
                        xtb_s[:, k, hb * 32:(hb + 1) * 32, :],
                        xtb[k, :, hb * 32:(hb + 1) * 32, :])
            P_s = bigpool.tile([128, T, BL], BF, tag="P")

            # ---- persistent state ----
            outs17 = state.tile([128, 2, NF + 1, BL], BF, tag="outs")
            c32 = state.tile([128, 2, BL], F32, tag="c32")
            cb = state.tile([128, 2, BL], BF, tag="cb")
            e_sb = state.tile([64, 368], BF, tag="esb")
            nc.vector.memset(e_sb[:, T:368], 0.0)
            w_t = state.tile([128, 3, BL], BF, tag="wt")
            h1 = state.tile([128, 2, 512], BF, tag="h1")

            # ================= prologue =================
            with (
                tc.tile_pool(name="pro", bufs=1) as pro,
                tc.tile_pool(name="proh", bufs=2) as work,
                tc.tile_pool(name="xchunk", bufs=2) as xchunk,
            ):
                idx_s = pro.tile([128, 3, BL], F32, tag="idx")
                nc.scalar.dma_start(idx_s[:], idxr[:, :, :])
                iota_s = pro.tile([128, 1], F32, tag="iota")
                nc.scalar.dma_start(iota_s[:], iota[:, :])
                st0_s = ldk(st0, 4, BL, pro)
                embs_s = pro.tile([54, 16], BF, tag="embs")
                nc.scalar.dma_start(embs_s[:], emb_s[:, :])
                embf_s = pro.tile([33, 16], BF, tag="embf")
                nc.scalar.dma_start(embf_s[:], emb_f[:, :])
                embi_s = ldk(emb_i, 32, 64, pro)
                wa1a_s = ldk(wa1a, 2, 128, pro, dt=F8)
                wc1p_s = ldk(wc1p, 7, 512, pro)
                bc1_s = pro.tile([128, 4], F32, tag="bc1")
                nc.scalar.dma_start(bc1_s[:], bc1t[:, :])
                wc2_s = ldk(wc2, 4, 384, pro)
                bc2_s = pro.tile([128, 3], F32, tag="bc2")
                nc.scalar.dma_start(bc2_s[:], bc2t[:, :])
                wc3_s = ldk(wc3, 3, 512, pro)
                bc3_s = pro.tile([128, 4], F32, tag="bc3")
                nc.scalar.dma_start(bc3_s[:], bc3t[:, :])
                # decoder weights (needed from ~step 0 on; issue after the
                # embedding/cond-MLP path consts)
                wa1b_s = ldk(nwa1b, 4, 128, cpool)
                nba1_s = ld(nba1r[:, :], [1, 128])
                wa2_s = ld(wa2[:, :], [128, 1])
                id16_s = ld(id16[:, :], [128, 128])
                wihg_s = ldk(wihg, 2, 1024, cpool)
                whhg_s = ldk(whhg, 2, 1024, cpool)
                bgr_s = ld(bgr[:, :], [1, 1024])
                wt1_s = ldk(wt1, 2, 128, cpool)
                bt1_s = ld(bt1t[:, :], [128, 1], F32)
                wt2_s = ld(wt2[:, :], [128, 1])
                bt2_s = ld(bt2r[:, :], [128, 1], F32)

                # embeddings -> xcond tiles 0..2; thought -> 3..6
                xcond = pro.tile([128, 7, BL], BF, tag="xcond")
                nc.vector.memset(xcond[:, 0:3, :], 0.0)
                for k in range(4):
                    nc.vector.tensor_copy(xcond[:, 3 + k, :], st0_s[:, k, :])

                def onehot_embed(col, table_s, nt, width, out_slice):
                    pe_out = ps.tile([width, BL], F32, tag="sm")
                    for k in range(nt):
                        oh = work.tile([128, BL], BF, tag="oh")
                        nc.vector.scalar_tensor_tensor(
                            oh[:], idx_s[:, col, :], float(-128 * k),
                            iota_s[:, :].broadcast_to((128, BL)),
                            op0=ALU.add, op1=ALU.is_equal)
                        kk = table_s.shape[0] if nt == 1 else 128
                        lhs = table_s[:, k, :] if nt > 1 else table_s[:, :]
                        nc.tensor.matmul(pe_out[:], lhs[:kk] if nt == 1 else lhs,
                                         oh[:kk] if nt == 1 else oh[:],
                                         start=(k == 0), stop=(k == nt - 1))
                    nc.scalar.activation(out_slice, pe_out[:], AF.Copy)

                onehot_embed(0, embs_s, 1, 16, xcond[0:16, 0, :])
                onehot_embed(1, embi_s, 32, 64, xcond[0:64, 1, :])
                onehot_embed(2, embf_s, 1, 16, xcond[0:16, 2, :])

                # conditioning MLP
                def mlp_layer(w_s, nk, x_s, mt, bias_s, relu, out_t):
                    for m in range(mt):
                        pe_o = ps.tile([128, BL], F32, tag="sm")
                        for k in range(nk):
                            nc.tensor.matmul(
                                pe_o[:], w_s[:, k, m * 128:(m + 1) * 128],
                                x_s[:, k, :],
                                start=(k == 0), stop=(k == nk - 1))
                        nc.scalar.activation(
                            out_t[:, m, :], pe_o[:],
                            AF.Relu if relu else AF.Identity,
                            bias=bias_s[:, m:m + 1])

                ct1 = pro.tile([128, 4, BL], BF, tag="ct1")
                mlp_layer(wc1p_s, 7, xcond, 4, bc1_s, True, ct1)
                ct2 = pro.tile([128, 3, BL], BF, tag="ct2")
                mlp_layer(wc2_s, 4, ct1, 3, bc2_s, True, ct2)
                ct3f = pro.tile([128, 4, BL], F32, tag="ct3f")
                mlp_layer(wc3_s, 3, ct2, 4, bc3_s, False, ct3f)

                # initial state: h0 -> outs17 slot 0, c0 -> c32 + cb
                nc.scalar.activation(outs17[:, :, 0, :], ct3f[:, 0:2, :],
                                     AF.Copy)
                nc.scalar.activation(cb[:, :, :], ct3f[:, 2:4, :], AF.Copy)
                nc.vector.tensor_copy(c32[:, :, :], ct3f[:, 2:4, :])

                # P precompute: P[f, t, b] = sum_d X[t,b,d] Wa1a[d,f]
                # xdt streamed in big staged pieces (few DMAs; SP issues),
                # matmul'd in 8-t psum sub-chunks.
                PT = 46
                CH = 8
                cc = 0
                t0 = 0
                while t0 < T:
                    pn = min(PT, T - t0)
                    xc = xchunk.tile([128, 2, PT, BL], F8, tag="xc")
                    for k in range(2):
                        nc.sync.dma_start(xc[:, k, :pn, :],
                                          xdt[k, :, t0:t0 + pn, :])
                    u0 = 0
                    while u0 < pn:
                        tn = min(CH, pn - u0)
                        pe_p = ps.tile([128, CH * BL], F32, tag="sm")
                        for k in range(2):
                            nc.tensor.matmul(
                                pe_p[:, :tn * BL], wa1a_s[:, k, :],
                                xc[:, k, u0:u0 + tn, :].rearrange(
                                    "p t b -> p (t b)"),
                                start=(k == 0), stop=(k == 1))
                        dst = P_s[:, t0 + u0:t0 + u0 + tn, :].rearrange(
                            "p t b -> p (t b)")
                        if cc % 2 == 0:
                            nc.scalar.activation(dst, pe_p[:, :tn * BL],
                                                 AF.Copy)
                        else:
                            nc.vector.tensor_copy(dst, pe_p[:, :tn * BL])
                        cc += 1
                        u0 += tn
                    t0 += pn

            # ================= decoder steps =================
            _dec_cm = tc.tile_pool(name="dec", bufs=1)
            _mch_cm = tc.tile_pool(name="mch", bufs=3)
            work = _dec_cm.__enter__()
            mpool = _mch_cm.__enter__()
            for s in range(NF):
                hs = [outs17[:, 0, s, :], outs17[:, 1, s, :],
                      cb[:, 0, :], cb[:, 1, :]]
                # nq = -(Wa1b^T [h;c] + ba1) via negated weights + bias fold.
                # c-dependent terms first: cb is ready before h each step.
                nq_ps = ps.tile([128, BL], F32, tag="sm")
                nq_mms = [(nba1_s[0:1, :], ones1[0:1, :]),
                          (wa1b_s[:, 2, :], hs[2]),
                          (wa1b_s[:, 3, :], hs[3]),
                          (wa1b_s[:, 0, :], hs[0]),
                          (wa1b_s[:, 1, :], hs[1])]
                prev = None
                for gi, (wm, xm) in enumerate(nq_mms):
                    mm = nc.tensor.matmul(nq_ps[:], wm, xm,
                                          start=(gi == 0), stop=(gi == 4))
                    if prev is not None:
                        add_dep_helper(mm.ins, prev.ins, sync=False,
                                       reason="psum accum order")
                    prev = mm
                nq_sb = work.tile([128, BL], BF, tag="nqsb")
                nc.scalar.activation(nq_sb[:], nq_ps[:], AF.Copy)

                # max phase: M = max(P, nq) ; scores[t] = wa2^T M[:, t, :]
                sc_ps = pssc.tile([64, T], F32, tag="sc")
                chunks = [(i * DCH, DCH, nc.vector) for i in range(4)]
                chunks.append((TDVE, DCH, nc.vector))
                chunks.append((TDVE + DCH, T - TDVE - DCH, nc.vector))
                for t0, tn, eng in chunks:
                    mc = mpool.tile([128, DCH, BL], BF, tag="mc")
                    eng.tensor_tensor(
                        mc[:, 0:tn, :], P_s[:, t0:t0 + tn, :],
                        nq_sb[:].unsqueeze(1).broadcast_to((128, tn, BL)),
                        op=ALU.max)
                    for jj in range(tn):
                        nc.tensor.matmul(sc_ps[:, t0 + jj:t0 + jj + 1],
                                         mc[:, jj, :], wa2_s[:, :],
                                         start=True, stop=True)

                # softmax over t (scores are O(1), skip max subtraction)
                zz = work.tile([64, 1], F32, tag="zz")
                nc.scalar.activation(e_sb[:, 0:T], sc_ps[:, :], AF.Exp,
                                     accum_out=zz[:])
                zrec = work.tile([64, 1], F32, tag="zrec")
                nc.vector.reciprocal(zrec[:], zz[:])
                nc.vector.tensor_scalar_mul(e_sb[:, 0:T], e_sb[:, 0:T],
                                            zrec[:, 0:1])

                # transpose e -> w_t [t-part, b]
                wt_ps = pswt.tile([128, 3, BL], BF, tag="wtp")
                nc.tensor.transpose(wt_ps[:, 0, :], e_sb[:, 0:128],
                                    id16_s[0:64, 0:64])
                nc.tensor.transpose(wt_ps[:, 1, :], e_sb[:, 128:256],
                                    id16_s[0:64, 0:64])
                nc.tensor.transpose(wt_ps[0:112, 2, :], e_sb[:, 256:368],
                                    id16_s[0:64, 0:64])
                nc.scalar.activation(w_t[:, 0:2, :], wt_ps[:, 0:2, :],
                                     AF.Copy)
                nc.scalar.activation(w_t[0:112, 2, :], wt_ps[0:112, 2, :],
                                     AF.Copy)

                # context: inp[d, b] = sum_t X[t, b, d] * w_t[t, b]
                v_ps = ps.tile([128, 2, BL], F32, tag="sm")
                prev = None
                groups = [(0, 128), (1, 128), (2, 109)]
                for b in range(BL):
                    for dm in range(2):
                        for gi, (g, kn) in enumerate(groups):
                            mm = nc.tensor.matmul(
                                v_ps[:, dm, b:b + 1],
                                xtb_s[0:kn, g, b, dm * 128:(dm + 1) * 128],
                                w_t[0:kn, g, b:b + 1],
                                start=(gi == 0), stop=(gi == 2))
                            if prev is not None:
                                add_dep_helper(mm.ins, prev.ins, sync=False,
                                               reason="psum accum order")
                            prev = mm
                inpT = work.tile([128, 2, BL], BF, tag="inpT")
                nc.scalar.activation(inpT[:], v_ps[:], AF.Copy)

                # gates (gate-scaled weights; bias folded): g_ps = -x|-2x
                g_ps = psg.tile([128, 512], F32, tag="g")
                prev = None
                for m in range(8):
                    sl = g_ps[:, m * 64:(m + 1) * 64]
                    # h-dependent terms first; inpT (latest) last
                    gmms = [(bgr_s[0:1, m * 128:(m + 1) * 128], ones1[0:1, :]),
                            (whhg_s[:, 0, m * 128:(m + 1) * 128], hs[0]),
                            (whhg_s[:, 1, m * 128:(m + 1) * 128], hs[1]),
                            (wihg_s[:, 0, m * 128:(m + 1) * 128], inpT[:, 0, :]),
                            (wihg_s[:, 1, m * 128:(m + 1) * 128], inpT[:, 1, :])]
                    for gi, (wm, xm) in enumerate(gmms):
                        mm = nc.tensor.matmul(sl, wm, xm, start=(gi == 0),
                                              stop=(gi == 4))
                        if prev is not None:
                            add_dep_helper(mm.ins, prev.ins, sync=False,
                                           reason="psum accum order")
                        prev = mm

                # ey = exp(g_ps); r = 1/(1+ey): sigma for i,f,o; tanh_g = 2r-1
                ey = work.tile([128, 512], F32, tag="ey")
                nc.scalar.activation(ey[:], g_ps[:], AF.Exp)
                nc.vector.tensor_scalar_add(ey[:], ey[:], 1.0)
                nc.vector.reciprocal(ey[:], ey[:])
                gt = work.tile([128, 2, BL], F32, tag="gt")
                nc.vector.tensor_scalar(
                    out=gt[:],
                    in0=ey[:, 256:384].rearrange("p (k b) -> p k b", k=2),
                    scalar1=2.0, scalar2=-1.0, op0=ALU.mult, op1=ALU.add)

                # c = sigma_f * c + sigma_i * tanh_g ; h = sigma_o * tanh(c)
                t1 = work.tile([128, 2, BL], F32, tag="t1")
                nc.vector.tensor_tensor(
                    t1[:], ey[:, 128:256].rearrange("p (k b) -> p k b", k=2),
                    c32[:], op=ALU.mult)
                t2 = work.tile([128, 2, BL], F32, tag="t2")
                nc.vector.tensor_tensor(
                    t2[:], ey[:, 0:128].rearrange("p (k b) -> p k b", k=2),
                    gt[:], op=ALU.mult)
                nc.vector.tensor_tensor(c32[:], t1[:], t2[:], op=ALU.add)
                ec = work.tile([128, 2, BL], F32, tag="ec")
                nc.scalar.activation(ec[:], c32[:], AF.Exp, scale=-2.0)
                nc.vector.tensor_scalar_add(ec[:], ec[:], 1.0)
                nc.vector.reciprocal(ec[:], ec[:])
                nc.vector.tensor_scalar(out=ec[:], in0=ec[:], scalar1=2.0,
                                        scalar2=-1.0, op0=ALU.mult,
                                        op1=ALU.add)
                nc.vector.tensor_tensor(
                    outs17[:, :, s + 1, :],
                    ey[:, 384:512].rearrange("p (k b) -> p k b", k=2),
                    ec[:], op=ALU.mult)
                nc.scalar.activation(cb[:], c32[:], AF.Copy)

            # ---- final MLP: td = relu(outs@Wt1+bt1)@Wt2 + bt2 ----
            for half in range(2):
                f_ps = psg.tile([128, 512], F32, tag="g")
                for k in range(2):
                    nc.tensor.matmul(
                        f_ps[:],
                        wt1_s[:, k, :],
                        outs17[:, k, 1 + half * 8:1 + (half + 1) * 8,
                               :].rearrange("p s b -> p (s b)"),
                        start=(k == 0), stop=(k == 1))
                nc.scalar.activation(h1[:, half, :], f_ps[:], AF.Relu,
                                     bias=bt1_s[:, :])
            td_ps = ps.tile([128, 8], F32, tag="sm")
            h1f = h1[:, :, :].rearrange("p h x -> p (h x)")
            for j in range(8):
                nc.tensor.matmul(td_ps[:, j:j + 1],
                                 h1f[:, 128 * j:128 * (j + 1)],
                                 wt2_s[:, :], start=True, stop=True)
            td_sb = work.tile([128, 8], F32, tag="tdsb")
            nc.vector.tensor_scalar_add(td_sb[:], td_ps[:], bt2_s[:, 0:1])
            nc.sync.dma_start(
                out.rearrange("(j two) b -> (two b) j", two=2), td_sb[:, :])
            _mch_cm.__exit__(None, None, None)
            _dec_cm.__exit__(None, None, None)

    nc.compile()
    return nc


_NC = None


def _get_nc():
    global _NC
    if _NC is None:
        _NC = build_bass()
    return _NC


def kernel(x_cat_static, state_h, state_c, outputs_encoder,
           emb_store, emb_item, emb_family,
           Wc1, bc1, Wc2, bc2, Wc3, bc3,
           Wa1, ba1, Wa2, ba2,
           Wt1, bt1, Wt2, bt2,
           Wih, Whh, bih, bhh):
    nc = _get_nc()

    # Wc1 rows: [store 16][item 64][family 16][thought 512] -> pad each
    # embedding block to its own 128-row K-tile, thought gets 4 tiles.
    wc1p = np.zeros((7, 128, 512), ml_dtypes.bfloat16)
    wc1p[0, :16] = _bf(Wc1[0:16])
    wc1p[1, :64] = _bf(Wc1[16:80])
    wc1p[2, :16] = _bf(Wc1[80:96])
    for k in range(4):
        wc1p[3 + k] = _bf(Wc1[96 + k * 128:96 + (k + 1) * 128])

    emb_i_t = np.zeros((32, 128, 64), ml_dtypes.bfloat16)
    ei = _bf(emb_item)
    for k in range(32):
        lo = k * 128
        hi = min(4036, lo + 128)
        if lo < 4036:
            emb_i_t[k, :hi - lo] = ei[lo:hi]

    # gate scaling: sigmoid gates (i,f,o) need exp(-x); tanh gate (g) exp(-2x)
    gsc = np.full(1024, -1.0, np.float32)
    gsc[512:768] = -2.0
    wih_g = (np.asarray(Wih, np.float32) * gsc[:, None]).T   # [256, 1024]
    whh_g = (np.asarray(Whh, np.float32) * gsc[:, None]).T
    bias_g = (_f32(bih) + _f32(bhh)) * gsc

    common = {
        "iota": np.arange(128, dtype=np.float32).reshape(128, 1),
        "emb_s": _bf(emb_store),
        "emb_i": emb_i_t,
        "emb_f": _bf(emb_family),
        "wc1p": wc1p,
        "bc1t": _f32(bc1).reshape(4, 128).T.copy(),
        "wc2": _ktile(_bf(Wc2), 512, 512),
        "bc2t": _f32(bc2).reshape(3, 128).T.copy(),
        "wc3": _ktile(_bf(Wc3), 384, 384),
        "bc3t": _f32(bc3).reshape(4, 128).T.copy(),
        "wa1a": _ktile(np.ascontiguousarray(Wa1[:256]).astype(
            ml_dtypes.float8_e4m3), 256, 256),
        "nwa1b": _ktile(_bf(-np.asarray(Wa1[256:], np.float32)), 512, 512),
        "nba1r": _bf(-_f32(ba1)).reshape(1, 128),
        "wa2": _bf(Wa2),
        "wihg": _ktile(_bf(wih_g), 256, 256),
        "whhg": _ktile(_bf(whh_g), 256, 256),
        "bgr": _bf(bias_g).reshape(1, 1024),
        "wt1": _ktile(_bf(Wt1), 256, 256),
        "bt1t": _f32(bt1).reshape(128, 1),
        "wt2": _bf(Wt2),
        "bt2r": np.full((128, 1), float(np.asarray(bt2).reshape(-1)[0]),
                        np.float32),
        "id16": np.eye(128, dtype=ml_dtypes.bfloat16),
    }

    oe = np.asarray(outputs_encoder)
    th = np.concatenate([np.asarray(state_h)[0], np.asarray(state_c)[0]],
                        axis=-1)  # [B, 512]
    xc = np.asarray(x_cat_static)

    in_maps = []
    for c in range(N_CORES):
        b0 = c * BL
        shb = _bf(oe[:, b0:b0 + BL, :])                # [T, BL, D]
        xtb_t = np.zeros((3, 128, BL, D), ml_dtypes.bfloat16)
        xtb_t[0] = shb[0:128]
        xtb_t[1] = shb[128:256]
        xtb_t[2, 0:109] = shb[256:365]
        xdt_t = np.ascontiguousarray(
            oe[:, b0:b0 + BL, :].transpose(2, 0, 1).reshape(
                2, 128, T, BL)).astype(ml_dtypes.float8_e4m3)
        st_t = np.ascontiguousarray(
            _bf(th[b0:b0 + BL]).T.reshape(4, 128, BL))
        idxr = np.broadcast_to(
            xc[b0:b0 + BL].T.astype(np.float32)[None, :, :],
            (128, 3, BL)).copy()
        m = dict(common)
        m.update({"xdt": xdt_t, "xtb": xtb_t, "st0": st_t, "idxr": idxr})
        in_maps.append(m)

    kw = {}
    if os.environ.get("KTRACE", "") == "1":
        kw = dict(trace=True, trace_cores=[0])
    res = run_bass_kernel_spmd(nc, in_maps, list(range(N_CORES)), **kw)
    if res.exec_time_ns is not None:
        print("HW exec time:", res.exec_time_ns, "ns  (mean",
              res.mean_exec_time_ns, ")", flush=True)
    outs = [res.results[c]["out"] for c in range(N_CORES)]
    return np.concatenate(outs, axis=1).astype(np.float32)


# revision 26
# speedup vs baseline: 1.9087x; 1.0689x over previous
import sys, os
sys.path.insert(0, '/opt/trn_rl_repo')
import numpy as np
import ml_dtypes
import concourse.bass as bass
import concourse.bacc as bacc
import concourse.mybir as mybir
import concourse.tile as tile
from concourse.tile import add_dep_helper
from concourse.bass_utils import run_bass_kernel_spmd

BF = mybir.dt.bfloat16
F32 = mybir.dt.float32
F8 = mybir.dt.float8e4
AF = mybir.ActivationFunctionType
ALU = mybir.AluOpType
AX = mybir.AxisListType

N_CORES = 8
B = 512
BL = B // N_CORES      # 64 batch rows per core
T = 365
D = 256
FA = 128               # attention MLP hidden
NF = 16                # forecast steps
NR = 256               # decoder hidden

# max-phase split: DVE handles t[0:TDVE) in 4 chunks, Pool t[TDVE:365)
TDVE = 264
DCH = TDVE // 4        # 66


def _bf(x):
    return np.ascontiguousarray(x).astype(ml_dtypes.bfloat16)


def _f32(x):
    return np.ascontiguousarray(x).astype(np.float32)


def _ktile(w, k_total, pad_to):
    """[K, M] -> [ntiles, 128, M] zero-padded on K."""
    K, M = w.shape
    assert K == k_total
    nt = (pad_to + 127) // 128
    out = np.zeros((nt, 128, M), w.dtype)
    for i in range(nt):
        lo = i * 128
        hi = min(K, lo + 128)
        if lo < K:
            out[i, :hi - lo] = w[lo:hi]
    return out


def build_bass():
    nc = bacc.Bacc("TRN2", target_bir_lowering=False, debug=False,
                   num_devices=N_CORES)

    def inp(name, shape, dt=BF):
        return nc.dram_tensor(name, shape, dt, kind="ExternalInput")

    # per-core sharded tensors
    xdt = inp("xdt", [2, 128, T, BL], F8)          # X^T: xdt[k,p,t,b] = X[t,b,128k+p]
    xtb = inp("xtb", [3, 128, BL, D])          # X t-grouped: [t0:128|128:256|256:365]
    st0 = inp("st0", [4, 128, BL])             # [h0;c0]^T k-tiled
    idxr = inp("idxr", [128, 3, BL], F32)      # indices replicated over partitions
    # replicated weights
    iota = inp("iota", [128, 1], F32)
    emb_s = inp("emb_s", [54, 16])
    emb_i = inp("emb_i", [32, 128, 64])
    emb_f = inp("emb_f", [33, 16])
    wc1p = inp("wc1p", [7, 128, 512])
    bc1t = inp("bc1t", [128, 4], F32)
    wc2 = inp("wc2", [4, 128, 384])
    bc2t = inp("bc2t", [128, 3], F32)
    wc3 = inp("wc3", [3, 128, 512])
    bc3t = inp("bc3t", [128, 4], F32)
    wa1a = inp("wa1a", [2, 128, 128], F8)
    nwa1b = inp("nwa1b", [4, 128, 128])        # -Wa1[256:768] k-tiled
    nba1r = inp("nba1r", [1, 128])             # -ba1 as a row
    wa2 = inp("wa2", [128, 1])
    wihg = inp("wihg", [2, 128, 1024])         # Wih^T k-tiled, gate-scaled
    whhg = inp("whhg", [2, 128, 1024])         # Whh^T k-tiled, gate-scaled
    bgr = inp("bgr", [1, 1024])                # (bih+bhh) gate-scaled row
    wt1 = inp("wt1", [2, 128, 128])
    bt1t = inp("bt1t", [128, 1], F32)
    wt2 = inp("wt2", [128, 1])
    bt2r = inp("bt2r", [128, 1], F32)
    id16 = inp("id16", [128, 128])             # bf16 identity
    out = nc.dram_tensor("out", [NF, BL], F32, kind="ExternalOutput")

    with tile.TileContext(nc) as tc:
        with (
            tc.tile_pool(name="const", bufs=1) as cpool,
            tc.tile_pool(name="big", bufs=1) as bigpool,
            tc.tile_pool(name="state", bufs=1) as state,
            tc.tile_pool(name="ps", bufs=4, space="PSUM") as ps,
            tc.tile_pool(name="psg", bufs=1, space="PSUM") as psg,
            tc.tile_pool(name="pssc", bufs=1, space="PSUM") as pssc,
            tc.tile_pool(name="pswt", bufs=1, space="PSUM") as pswt,
        ):
            # ---- persistent constants (issued from Act HWDGE; SP is busy
            # streaming xdt and big xtb transfers block their issuing seq) ----
            def ld(ap_dram, shape, dt=BF, tag=None):
                t_ = cpool.tile(shape, dt, tag=tag or ap_dram.name)
                nc.scalar.dma_start(t_[:], ap_dram)
                return t_

            def ldk(dram, nt, m, pool, dt=BF):
                t_ = pool.tile([128, nt, m], dt, tag=dram.name)
                nc.scalar.dma_start(
                    t_[:, :, :], dram[:, :, :].rearrange("n p m -> p n m"))
                return t_

            ones1 = cpool.tile([1, BL], BF, tag="ones1")
            nc.vector.memset(ones1[:], 1.0)

            # ---- persistent big tensors ----
            # xtb via Pool SWDGE in halves: cheap issue, parallel queues
            xtb_s = bigpool.tile([128, 3, BL, D], BF, tag="xtb")
            for k in range(3):
                for hb in range(2):
                    nc.gpsimd.dma_start(
                        xtb_s[:, k, hb * 32:(hb + 1) * 32, :],
                        xtb[k, :, hb * 32:(hb + 1) * 32, :])
            P_s = bigpool.tile([128, T, BL], BF, tag="P")

            # ---- persistent state ----
            outs17 = state.tile([128, 2, NF + 1, BL], BF, tag="outs")
            c32 = state.tile([128, 2, BL], F32, tag="c32")
            cb = state.tile([128, 2, BL], BF, tag="cb")
            e_sb0 = state.tile([32, 368], BF, tag="esb0")
            nc.vector.memset(e_sb0[:, T:368], 0.0)
            e_sb1 = state.tile([32, 368], BF, tag="esb1")
            nc.vector.memset(e_sb1[:, T:368], 0.0)
            h1 = state.tile([128, 2, 512], BF, tag="h1")

            # ================= prologue =================
            with (
                tc.tile_pool(name="pro", bufs=1) as pro,
                tc.tile_pool(name="proh", bufs=2) as work,
                tc.tile_pool(name="xchunk", bufs=2) as xchunk,
            ):
                idx_s = pro.tile([128, 3, BL], F32, tag="idx")
                nc.scalar.dma_start(idx_s[:], idxr[:, :, :])
                iota_s = pro.tile([128, 1], F32, tag="iota")
                nc.scalar.dma_start(iota_s[:], iota[:, :])
                st0_s = ldk(st0, 4, BL, pro)
                embs_s = pro.tile([54, 16], BF, tag="embs")
                nc.scalar.dma_start(embs_s[:], emb_s[:, :])
                embf_s = pro.tile([33, 16], BF, tag="embf")
                nc.scalar.dma_start(embf_s[:], emb_f[:, :])
                embi_s = ldk(emb_i, 32, 64, pro)
                wa1a_s = ldk(wa1a, 2, 128, pro, dt=F8)
                wc1p_s = ldk(wc1p, 7, 512, pro)
                bc1_s = pro.tile([128, 4], F32, tag="bc1")
                nc.scalar.dma_start(bc1_s[:], bc1t[:, :])
                wc2_s = ldk(wc2, 4, 384, pro)
                bc2_s = pro.tile([128, 3], F32, tag="bc2")
                nc.scalar.dma_start(bc2_s[:], bc2t[:, :])
                wc3_s = ldk(wc3, 3, 512, pro)
                bc3_s = pro.tile([128, 4], F32, tag="bc3")
                nc.scalar.dma_start(bc3_s[:], bc3t[:, :])
                # decoder weights (needed from ~step 0 on; issue after the
                # embedding/cond-MLP path consts)
                wa1b_s = ldk(nwa1b, 4, 128, cpool)
                nba1_s = ld(nba1r[:, :], [1, 128])
                wa2_s = ld(wa2[:, :], [128, 1])
                id16_s = ld(id16[:, :], [128, 128])
                wihg_s = ldk(wihg, 2, 1024, cpool)
                whhg_s = ldk(whhg, 2, 1024, cpool)
                bgr_s = ld(bgr[:, :], [1, 1024])
                wt1_s = ldk(wt1, 2, 128, cpool)
                bt1_s = ld(bt1t[:, :], [128, 1], F32)
                wt2_s = ld(wt2[:, :], [128, 1])
                bt2_s = ld(bt2r[:, :], [128, 1], F32)

                # embeddings -> xcond tiles 0..2; thought -> 3..6
                xcond = pro.tile([128, 7, BL], BF, tag="xcond")
                nc.vector.memset(xcond[:, 0:3, :], 0.0)
                for k in range(4):
                    nc.vector.tensor_copy(xcond[:, 3 + k, :], st0_s[:, k, :])

                def onehot_embed(col, table_s, nt, width, out_slice):
                    pe_out = ps.tile([width, BL], F32, tag="sm")
                    for k in range(nt):
                        oh = work.tile([128, BL], BF, tag="oh")
                        nc.vector.scalar_tensor_tensor(
                            oh[:], idx_s[:, col, :], float(-128 * k),
                            iota_s[:, :].broadcast_to((128, BL)),
                            op0=ALU.add, op1=ALU.is_equal)
                        kk = table_s.shape[0] if nt == 1 else 128
                        lhs = table_s[:, k, :] if nt > 1 else table_s[:, :]
                        nc.tensor.matmul(pe_out[:], lhs[:kk] if nt == 1 else lhs,
                                         oh[:kk] if nt == 1 else oh[:],
                                         start=(k == 0), stop=(k == nt - 1))
                    nc.scalar.activation(out_slice, pe_out[:], AF.Copy)

                onehot_embed(0, embs_s, 1, 16, xcond[0:16, 0, :])
                onehot_embed(1, embi_s, 32, 64, xcond[0:64, 1, :])
                onehot_embed(2, embf_s, 1, 16, xcond[0:16, 2, :])

                # conditioning MLP
                def mlp_layer(w_s, nk, x_s, mt, bias_s, relu, out_t):
                    for m in range(mt):
                        pe_o = ps.tile([128, BL], F32, tag="sm")
                        for k in range(nk):
                            nc.tensor.matmul(
                                pe_o[:], w_s[:, k, m * 128:(m + 1) * 128],
                                x_s[:, k, :],
                                start=(k == 0), stop=(k == nk - 1))
                        nc.scalar.activation(
                            out_t[:, m, :], pe_o[:],
                            AF.Relu if relu else AF.Identity,
                            bias=bias_s[:, m:m + 1])

                ct1 = pro.tile([128, 4, BL], BF, tag="ct1")
                mlp_layer(wc1p_s, 7, xcond, 4, bc1_s, True, ct1)
                ct2 = pro.tile([128, 3, BL], BF, tag="ct2")
                mlp_layer(wc2_s, 4, ct1, 3, bc2_s, True, ct2)
                ct3f = pro.tile([128, 4, BL], F32, tag="ct3f")
                mlp_layer(wc3_s, 3, ct2, 4, bc3_s, False, ct3f)

                # initial state: h0 -> outs17 slot 0, c0 -> c32 + cb
                nc.scalar.activation(outs17[:, :, 0, :], ct3f[:, 0:2, :],
                                     AF.Copy)
                nc.scalar.activation(cb[:, :, :], ct3f[:, 2:4, :], AF.Copy)
                nc.vector.tensor_copy(c32[:, :, :], ct3f[:, 2:4, :])

                # P precompute: P[f, t, b] = sum_d X[t,b,d] Wa1a[d,f]
                # xdt streamed in big staged pieces (few DMAs; SP issues),
                # matmul'd in 8-t psum sub-chunks.
                PT = 46
                CH = 8
                cc = 0
                t0 = 0
                while t0 < T:
                    pn = min(PT, T - t0)
                    xc = xchunk.tile([128, 2, PT, BL], F8, tag="xc")
                    for k in range(2):
                        nc.sync.dma_start(xc[:, k, :pn, :],
                                          xdt[k, :, t0:t0 + pn, :])
                    u0 = 0
                    while u0 < pn:
                        tn = min(CH, pn - u0)
                        pe_p = ps.tile([128, CH * BL], F32, tag="sm")
                        for k in range(2):
                            nc.tensor.matmul(
                                pe_p[:, :tn * BL], wa1a_s[:, k, :],
                                xc[:, k, u0:u0 + tn, :].rearrange(
                                    "p t b -> p (t b)"),
                                start=(k == 0), stop=(k == 1))
                        dst = P_s[:, t0 + u0:t0 + u0 + tn, :].rearrange(
                            "p t b -> p (t b)")
                        if cc % 2 == 0:
                            nc.scalar.activation(dst, pe_p[:, :tn * BL],
                                                 AF.Copy)
                        else:
                            nc.vector.tensor_copy(dst, pe_p[:, :tn * BL])
                        cc += 1
                        u0 += tn
                    t0 += pn

            # ================= decoder steps =================
            # Two anti-phase batch halves: while one half runs its DVE max
            # phase, the other half runs softmax/context/gates/LSTM.
            _dec_cm = tc.tile_pool(name="dec", bufs=1)
            _mch_cm = tc.tile_pool(name="mch", bufs=3)
            work = _dec_cm.__enter__()
            mpool = _mch_cm.__enter__()
            HB = 32
            TC3 = [(0, 122), (122, 122), (244, 121)]
            groups = [(0, 128), (1, 128), (2, 109)]
            e_half = [e_sb0, e_sb1]

            def hslice(h):
                return slice(h * HB, (h + 1) * HB)

            def chain(mm, prev):
                if prev is not None:
                    add_dep_helper(mm.ins, prev.ins, sync=False,
                                   reason="psum accum order")
                return mm

            def a_nq(s, h):
                hsl = hslice(h)
                hs = [outs17[:, 0, s, hsl], outs17[:, 1, s, hsl],
                      cb[:, 0, hsl], cb[:, 1, hsl]]
                nq_ps = ps.tile([128, HB], F32, tag="sm")
                mms = [(nba1_s[0:1, :], ones1[0:1, 0:HB]),
                       (wa1b_s[:, 2, :], hs[2]), (wa1b_s[:, 3, :], hs[3]),
                       (wa1b_s[:, 0, :], hs[0]), (wa1b_s[:, 1, :], hs[1])]
                prev = None
                for gi, (wm, xm) in enumerate(mms):
                    prev = chain(nc.tensor.matmul(
                        nq_ps[:], wm, xm, start=(gi == 0), stop=(gi == 4)),
                        prev)
                nq_sb = work.tile([128, HB], BF, tag=f"nq{h}")
                nc.scalar.activation(nq_sb[:], nq_ps[:], AF.Copy)
                return nq_sb

            def b_max(s, h, nq_sb):
                hsl = hslice(h)
                sc_ps = pssc.tile([HB, T], F32, tag="sc")
                for t0, tn in TC3:
                    mc = mpool.tile([128, TC3[0][1], HB], BF, tag="mc")
                    nc.vector.tensor_tensor(
                        mc[:, 0:tn, :], P_s[:, t0:t0 + tn, hsl],
                        nq_sb[:].unsqueeze(1).broadcast_to((128, tn, HB)),
                        op=ALU.max)
                    for jj in range(tn):
                        nc.tensor.matmul(sc_ps[:, t0 + jj:t0 + jj + 1],
                                         mc[:, jj, :], wa2_s[:, :],
                                         start=True, stop=True)
                return sc_ps

            def c_soft(s, h, sc_ps):
                esb = e_half[h]
                zz = work.tile([HB, 1], F32, tag=f"zz{h}")
                nc.scalar.activation(esb[:, 0:T], sc_ps[:, :], AF.Exp,
                                     accum_out=zz[:])
                zrec = work.tile([HB, 1], F32, tag=f"zr{h}")
                nc.vector.reciprocal(zrec[:], zz[:])
                nc.vector.tensor_scalar_mul(esb[:, 0:T], esb[:, 0:T],
                                            zrec[:, 0:1])
                wt_ps = pswt.tile([128, 3, HB], BF, tag="wtp")
                nc.tensor.transpose(wt_ps[:, 0, :], esb[:, 0:128],
                                    id16_s[0:HB, 0:HB])
                nc.tensor.transpose(wt_ps[:, 1, :], esb[:, 128:256],
                                    id16_s[0:HB, 0:HB])
                nc.tensor.transpose(wt_ps[0:112, 2, :], esb[:, 256:368],
                                    id16_s[0:HB, 0:HB])
                w_th = work.tile([128, 3, HB], BF, tag=f"wt{h}")
                nc.scalar.activation(w_th[:, 0:2, :], wt_ps[:, 0:2, :],
                                     AF.Copy)
                nc.scalar.activation(w_th[0:112, 2, :], wt_ps[0:112, 2, :],
                                     AF.Copy)
                return w_th

            def d_ctx(s, h, w_th):
                v_ps = ps.tile([128, 2, HB], F32, tag="sm")
                prev = None
                for b in range(HB):
                    gb = h * HB + b
                    for dm in range(2):
                        for gi, (g, kn) in enumerate(groups):
                            prev = chain(nc.tensor.matmul(
                                v_ps[:, dm, b:b + 1],
                                xtb_s[0:kn, g, gb, dm * 128:(dm + 1) * 128],
                                w_th[0:kn, g, b:b + 1],
                                start=(gi == 0), stop=(gi == 2)), prev)
                inpT = work.tile([128, 2, HB], BF, tag=f"inp{h}")
                nc.scalar.activation(inpT[:], v_ps[:], AF.Copy)
                return inpT

            def e_gates(s, h, inpT):
                hsl = hslice(h)
                hs0 = outs17[:, 0, s, hsl]
                hs1 = outs17[:, 1, s, hsl]
                g_ps = psg.tile([128, 8 * HB], F32, tag="g")
                prev = None
                for m in range(8):
                    sl_ = g_ps[:, m * HB:(m + 1) * HB]
                    gmms = [(bgr_s[0:1, m * 128:(m + 1) * 128],
                             ones1[0:1, 0:HB]),
                            (whhg_s[:, 0, m * 128:(m + 1) * 128], hs0),
                            (whhg_s[:, 1, m * 128:(m + 1) * 128], hs1),
                            (wihg_s[:, 0, m * 128:(m + 1) * 128],
                             inpT[:, 0, :]),
                            (wihg_s[:, 1, m * 128:(m + 1) * 128],
                             inpT[:, 1, :])]
                    for gi, (wm, xm) in enumerate(gmms):
                        prev = chain(nc.tensor.matmul(
                            sl_, wm, xm, start=(gi == 0), stop=(gi == 4)),
                            prev)
                ey = work.tile([128, 8 * HB], F32, tag=f"ey{h}")
                nc.scalar.activation(ey[:], g_ps[:], AF.Exp)
                return ey

            def f1_lstm(s, h, ey):
                # sigma = 1/(1+ey) for i,f,o; tanh_g = 2r-1; c update
                hsl = hslice(h)
                nc.gpsimd.tensor_scalar_add(ey[:], ey[:], 1.0)
                nc.vector.reciprocal(ey[:], ey[:])
                gt = work.tile([128, 2, HB], F32, tag=f"gt{h}")
                nc.gpsimd.tensor_scalar(
                    out=gt[:],
                    in0=ey[:, 4 * HB:6 * HB].rearrange("p (k b) -> p k b",
                                                       k=2),
                    scalar1=2.0, scalar2=-1.0, op0=ALU.mult, op1=ALU.add)
                t1 = work.tile([128, 2, HB], F32, tag=f"t1{h}")
                nc.vector.tensor_tensor(
                    t1[:],
                    ey[:, 2 * HB:4 * HB].rearrange("p (k b) -> p k b", k=2),
                    c32[:, :, hsl], op=ALU.mult)
                t2 = work.tile([128, 2, HB], F32, tag=f"t2{h}")
                nc.vector.tensor_tensor(
                    t2[:],
                    ey[:, 0:2 * HB].rearrange("p (k b) -> p k b", k=2),
                    gt[:], op=ALU.mult)
                nc.vector.tensor_tensor(c32[:, :, hsl], t1[:], t2[:],
                                        op=ALU.add)
                nc.gpsimd.tensor_copy(cb[:, :, hsl], c32[:, :, hsl])

            def f2_lstm(s, h, ey):
                # h = sigma_o * tanh(c) with tanh via exp(-2c)
                hsl = hslice(h)
                ec = work.tile([128, 2, HB], F32, tag=f"ec{h}")
                nc.scalar.activation(ec[:], c32[:, :, hsl], AF.Exp,
                                     scale=-2.0)
                nc.gpsimd.tensor_scalar_add(ec[:], ec[:], 1.0)
                nc.vector.reciprocal(ec[:], ec[:])
                nc.gpsimd.tensor_scalar(out=ec[:], in0=ec[:], scalar1=2.0,
                                        scalar2=-1.0, op0=ALU.mult,
                                        op1=ALU.add)
                nc.vector.tensor_tensor(
                    outs17[:, :, s + 1, hsl],
                    ey[:, 6 * HB:8 * HB].rearrange("p (k b) -> p k b", k=2),
                    ec[:], op=ALU.mult)

            nq0 = a_nq(0, 0)
            ey_prev = [None, None]
            for s in range(NF):
                sc0 = b_max(s, 0, nq0)
                if s > 0:
                    f1_lstm(s - 1, 1, ey_prev[1])
                wt0 = c_soft(s, 0, sc0)
                if s > 0:
                    f2_lstm(s - 1, 1, ey_prev[1])
                nq1 = a_nq(s, 1)
                inp0 = d_ctx(s, 0, wt0)
                ey0 = e_gates(s, 0, inp0)
                sc1 = b_max(s, 1, nq1)
                f1_lstm(s, 0, ey0)
                wt1h = c_soft(s, 1, sc1)
                f2_lstm(s, 0, ey0)
                if s + 1 < NF:
                    nq0 = a_nq(s + 1, 0)
                inp1 = d_ctx(s, 1, wt1h)
                ey_prev[1] = e_gates(s, 1, inp1)
            f1_lstm(NF - 1, 1, ey_prev[1])
            f2_lstm(NF - 1, 1, ey_prev[1])

            # ---- final MLP: td = relu(outs@Wt1+bt1)@Wt2 + bt2 ----
            for half in range(2):
                f_ps = psg.tile([128, 512], F32, tag="g")
                for k in range(2):
                    nc.tensor.matmul(
                        f_ps[:],
                        wt1_s[:, k, :],
                        outs17[:, k, 1 + half * 8:1 + (half + 1) * 8,
                               :].rearrange("p s b -> p (s b)"),
                        start=(k == 0), stop=(k == 1))
                nc.scalar.activation(h1[:, half, :], f_ps[:], AF.Relu,
                                     bias=bt1_s[:, :])
            td_ps = ps.tile([128, 8], F32, tag="sm")
            h1f = h1[:, :, :].rearrange("p h x -> p (h x)")
            for j in range(8):
                nc.tensor.matmul(td_ps[:, j:j + 1],
                                 h1f[:, 128 * j:128 * (j + 1)],
                                 wt2_s[:, :], start=True, stop=True)
            td_sb = work.tile([128, 8], F32, tag="tdsb")
            nc.vector.tensor_scalar_add(td_sb[:], td_ps[:], bt2_s[:, 0:1])
            nc.sync.dma_start(
                out.rearrange("(j two) b -> (two b) j", two=2), td_sb[:, :])
            _mch_cm.__exit__(None, None, None)
            _dec_cm.__exit__(None, None, None)

    nc.compile()
    return nc


_NC = None


def _get_nc():
    global _NC
    if _NC is None:
        _NC = build_bass()
    return _NC


def kernel(x_cat_static, state_h, state_c, outputs_encoder,
           emb_store, emb_item, emb_family,
           Wc1, bc1, Wc2, bc2, Wc3, bc3,
           Wa1, ba1, Wa2, ba2,
           Wt1, bt1, Wt2, bt2,
           Wih, Whh, bih, bhh):
    nc = _get_nc()

    # Wc1 rows: [store 16][item 64][family 16][thought 512] -> pad each
    # embedding block to its own 128-row K-tile, thought gets 4 tiles.
    wc1p = np.zeros((7, 128, 512), ml_dtypes.bfloat16)
    wc1p[0, :16] = _bf(Wc1[0:16])
    wc1p[1, :64] = _bf(Wc1[16:80])
    wc1p[2, :16] = _bf(Wc1[80:96])
    for k in range(4):
        wc1p[3 + k] = _bf(Wc1[96 + k * 128:96 + (k + 1) * 128])

    emb_i_t = np.zeros((32, 128, 64), ml_dtypes.bfloat16)
    ei = _bf(emb_item)
    for k in range(32):
        lo = k * 128
        hi = min(4036, lo + 128)
        if lo < 4036:
            emb_i_t[k, :hi - lo] = ei[lo:hi]

    # gate scaling: sigmoid gates (i,f,o) need exp(-x); tanh gate (g) exp(-2x)
    gsc = np.full(1024, -1.0, np.float32)
    gsc[512:768] = -2.0
    wih_g = (np.asarray(Wih, np.float32) * gsc[:, None]).T   # [256, 1024]
    whh_g = (np.asarray(Whh, np.float32) * gsc[:, None]).T
    bias_g = (_f32(bih) + _f32(bhh)) * gsc

    common = {
        "iota": np.arange(128, dtype=np.float32).reshape(128, 1),
        "emb_s": _bf(emb_store),
        "emb_i": emb_i_t,
        "emb_f": _bf(emb_family),
        "wc1p": wc1p,
        "bc1t": _f32(bc1).reshape(4, 128).T.copy(),
        "wc2": _ktile(_bf(Wc2), 512, 512),
        "bc2t": _f32(bc2).reshape(3, 128).T.copy(),
        "wc3": _ktile(_bf(Wc3), 384, 384),
        "bc3t": _f32(bc3).reshape(4, 128).T.copy(),
        "wa1a": _ktile(np.ascontiguousarray(Wa1[:256]).astype(
            ml_dtypes.float8_e4m3), 256, 256),
        "nwa1b": _ktile(_bf(-np.asarray(Wa1[256:], np.float32)), 512, 512),
        "nba1r": _bf(-_f32(ba1)).reshape(1, 128),
        "wa2": _bf(Wa2),
        "wihg": _ktile(_bf(wih_g), 256, 256),
        "whhg": _ktile(_bf(whh_g), 256, 256),
        "bgr": _bf(bias_g).reshape(1, 1024),
        "wt1": _ktile(_bf(Wt1), 256, 256),
        "bt1t": _f32(bt1).reshape(128, 1),
        "wt2": _bf(Wt2),
        "bt2r": np.full((128, 1), float(np.asarray(bt2).reshape(-1)[0]),
                        np.float32),
        "id16": np.eye(128, dtype=ml_dtypes.bfloat16),
    }

    oe = np.asarray(outputs_encoder)
    th = np.concatenate([np.asarray(state_h)[0], np.asarray(state_c)[0]],
                        axis=-1)  # [B, 512]
    xc = np.asarray(x_cat_static)

    in_maps = []
    for c in range(N_CORES):
        b0 = c * BL
        shb = _bf(oe[:, b0:b0 + BL, :])                # [T, BL, D]
        xtb_t = np.zeros((3, 128, BL, D), ml_dtypes.bfloat16)
        xtb_t[0] = shb[0:128]
        xtb_t[1] = shb[128:256]
        xtb_t[2, 0:109] = shb[256:365]
        xdt_t = np.ascontiguousarray(
            oe[:, b0:b0 + BL, :].transpose(2, 0, 1).reshape(
                2, 128, T, BL)).astype(ml_dtypes.float8_e4m3)
        st_t = np.ascontiguousarray(
            _bf(th[b0:b0 + BL]).T.reshape(4, 128, BL))
        idxr = np.broadcast_to(
            xc[b0:b0 + BL].T.astype(np.float32)[None, :, :],
            (128, 3, BL)).copy()
        m = dict(common)
        m.update({"xdt": xdt_t, "xtb": xtb_t, "st0": st_t, "idxr": idxr})
        in_maps.append(m)

    kw = {}
    if os.environ.get("KTRACE", "") == "1":
        kw = dict(trace=True, trace_cores=[0])
    res = run_bass_kernel_spmd(nc, in_maps, list(range(N_CORES)), **kw)
    if res.exec_time_ns is not None:
        print("HW exec time:", res.exec_time_ns, "ns  (mean",
              res.mean_exec_time_ns, ")", flush=True)
    outs = [res.results[c]["out"] for c in range(N_CORES)]
    return np.concatenate(outs, axis=1).astype(np.float32)


# revision 27
# speedup vs baseline: 2.0537x; 1.0759x over previous
import sys, os
sys.path.insert(0, '/opt/trn_rl_repo')
import numpy as np
import ml_dtypes
import concourse.bass as bass
import concourse.bacc as bacc
import concourse.mybir as mybir
import concourse.tile as tile
from concourse.tile import add_dep_helper
from concourse.bass_utils import run_bass_kernel_spmd

BF = mybir.dt.bfloat16
F32 = mybir.dt.float32
F8 = mybir.dt.float8e4
AF = mybir.ActivationFunctionType
ALU = mybir.AluOpType
AX = mybir.AxisListType

N_CORES = 8
B = 512
BL = B // N_CORES      # 64 batch rows per core
T = 365
D = 256
FA = 128               # attention MLP hidden
NF = 16                # forecast steps
NR = 256               # decoder hidden

# max-phase split: DVE handles t[0:TDVE) in 4 chunks, Pool t[TDVE:365)
TDVE = 264
DCH = TDVE // 4        # 66


def _bf(x):
    return np.ascontiguousarray(x).astype(ml_dtypes.bfloat16)


def _f32(x):
    return np.ascontiguousarray(x).astype(np.float32)


def _ktile(w, k_total, pad_to):
    """[K, M] -> [ntiles, 128, M] zero-padded on K."""
    K, M = w.shape
    assert K == k_total
    nt = (pad_to + 127) // 128
    out = np.zeros((nt, 128, M), w.dtype)
    for i in range(nt):
        lo = i * 128
        hi = min(K, lo + 128)
        if lo < K:
            out[i, :hi - lo] = w[lo:hi]
    return out


def build_bass():
    nc = bacc.Bacc("TRN2", target_bir_lowering=False, debug=False,
                   num_devices=N_CORES)

    def inp(name, shape, dt=BF):
        return nc.dram_tensor(name, shape, dt, kind="ExternalInput")

    # per-core sharded tensors
    xdt = inp("xdt", [2, 128, T, BL], F8)          # X^T: xdt[k,p,t,b] = X[t,b,128k+p]
    xtb = inp("xtb", [3, 128, BL, D], F8)          # X t-grouped: [t0:128|128:256|256:365]
    st0 = inp("st0", [4, 128, BL])             # [h0;c0]^T k-tiled
    idxr = inp("idxr", [128, 3, BL], F32)      # indices replicated over partitions
    # replicated weights
    iota = inp("iota", [128, 1], F32)
    emb_s = inp("emb_s", [54, 16])
    emb_i = inp("emb_i", [128, 32, 64])
    emb_f = inp("emb_f", [33, 16])
    wc1p = inp("wc1p", [7, 128, 512])
    bc1t = inp("bc1t", [128, 4], F32)
    wc2 = inp("wc2", [4, 128, 384])
    bc2t = inp("bc2t", [128, 3], F32)
    wc3 = inp("wc3", [3, 128, 512])
    bc3t = inp("bc3t", [128, 4], F32)
    wa1a = inp("wa1a", [2, 128, 128], F8)
    nwa1b = inp("nwa1b", [4, 128, 128])        # -Wa1[256:768] k-tiled
    nba1r = inp("nba1r", [1, 128])             # -ba1 as a row
    wa2 = inp("wa2", [128, 1])
    wihg = inp("wihg", [2, 128, 1024])         # Wih^T k-tiled, gate-scaled
    whhg = inp("whhg", [2, 128, 1024])         # Whh^T k-tiled, gate-scaled
    bgr = inp("bgr", [1, 1024])                # (bih+bhh) gate-scaled row
    wt1 = inp("wt1", [2, 128, 128])
    bt1t = inp("bt1t", [128, 1], F32)
    wt2 = inp("wt2", [128, 1])
    bt2r = inp("bt2r", [128, 1], F32)
    id16 = inp("id16", [128, 128])             # bf16 identity
    out = nc.dram_tensor("out", [NF, BL], F32, kind="ExternalOutput")

    with tile.TileContext(nc) as tc:
        with (
            tc.tile_pool(name="const", bufs=1) as cpool,
            tc.tile_pool(name="big", bufs=1) as bigpool,
            tc.tile_pool(name="state", bufs=1) as state,
            tc.tile_pool(name="ps", bufs=4, space="PSUM") as ps,
            tc.tile_pool(name="psg", bufs=1, space="PSUM") as psg,
            tc.tile_pool(name="pssc", bufs=1, space="PSUM") as pssc,
            tc.tile_pool(name="pswt", bufs=1, space="PSUM") as pswt,
        ):
            # ---- persistent constants (issued from Act HWDGE; SP is busy
            # streaming xdt and big xtb transfers block their issuing seq) ----
            def ld(ap_dram, shape, dt=BF, tag=None):
                t_ = cpool.tile(shape, dt, tag=tag or ap_dram.name)
                nc.scalar.dma_start(t_[:], ap_dram)
                return t_

            def ldk(dram, nt, m, pool, dt=BF):
                t_ = pool.tile([128, nt, m], dt, tag=dram.name)
                nc.scalar.dma_start(
                    t_[:, :, :], dram[:, :, :].rearrange("n p m -> p n m"))
                return t_

            ones1 = cpool.tile([1, BL], BF, tag="ones1")
            nc.vector.memset(ones1[:], 1.0)

            # ---- persistent big tensors ----
            xtb_s = bigpool.tile([128, 3, BL, D], F8, tag="xtb")
            P_s = bigpool.tile([128, T, BL], BF, tag="P")

            # ---- persistent state ----
            outs17 = state.tile([128, 2, NF + 1, BL], BF, tag="outs")
            c32 = state.tile([128, 2, BL], F32, tag="c32")
            cb = state.tile([128, 2, BL], BF, tag="cb")
            e_sb0 = state.tile([32, 368], BF, tag="esb0")
            nc.vector.memset(e_sb0[:, T:368], 0.0)
            e_sb1 = state.tile([32, 368], BF, tag="esb1")
            nc.vector.memset(e_sb1[:, T:368], 0.0)
            h1 = state.tile([128, 2, 512], BF, tag="h1")

            # ================= prologue =================
            with (
                tc.tile_pool(name="pro", bufs=1) as pro,
                tc.tile_pool(name="proh", bufs=2) as work,
                tc.tile_pool(name="xchunk", bufs=2) as xchunk,
            ):
                idx_s = pro.tile([128, 3, BL], F32, tag="idx")
                nc.scalar.dma_start(idx_s[:], idxr[:, :, :])
                iota_s = pro.tile([128, 1], F32, tag="iota")
                nc.scalar.dma_start(iota_s[:], iota[:, :])
                st0_s = ldk(st0, 4, BL, pro)
                embs_s = pro.tile([54, 16], BF, tag="embs")
                nc.scalar.dma_start(embs_s[:], emb_s[:, :])
                embf_s = pro.tile([33, 16], BF, tag="embf")
                nc.scalar.dma_start(embf_s[:], emb_f[:, :])
                embi_s = pro.tile([128, 32, 64], BF, tag="embi")
                nc.scalar.dma_start(embi_s[:, :, :], emb_i[:, :, :])
                wa1a_s = ldk(wa1a, 2, 128, pro, dt=F8)
                wc1p_s = ldk(wc1p, 7, 512, pro)
                bc1_s = pro.tile([128, 4], F32, tag="bc1")
                nc.scalar.dma_start(bc1_s[:], bc1t[:, :])
                wc2_s = ldk(wc2, 4, 384, pro)
                bc2_s = pro.tile([128, 3], F32, tag="bc2")
                nc.scalar.dma_start(bc2_s[:], bc2t[:, :])
                wc3_s = ldk(wc3, 3, 512, pro)
                bc3_s = pro.tile([128, 4], F32, tag="bc3")
                nc.scalar.dma_start(bc3_s[:], bc3t[:, :])
                # decoder weights (needed from ~step 0 on); issued from SP
                # so the Act seq stays free for compute
                def lds(ap_dram, shape, dt=BF, tag=None):
                    t_ = cpool.tile(shape, dt, tag=tag or ap_dram.name)
                    nc.sync.dma_start(t_[:], ap_dram)
                    return t_

                def ldks(dram, nt, m):
                    t_ = cpool.tile([128, nt, m], BF, tag=dram.name)
                    nc.sync.dma_start(
                        t_[:, :, :], dram[:, :, :].rearrange("n p m -> p n m"))
                    return t_

                wa1b_s = ldks(nwa1b, 4, 128)
                nba1_s = lds(nba1r[:, :], [1, 128])
                wa2_s = lds(wa2[:, :], [128, 1])
                id16_s = lds(id16[:, :], [128, 128])
                # xtb via Pool SWDGE after the score-path inputs
                for k in range(3):
                    for hb in range(2):
                        nc.gpsimd.dma_start(
                            xtb_s[:, k, hb * 32:(hb + 1) * 32, :],
                            xtb[k, :, hb * 32:(hb + 1) * 32, :])
                wihg_s = ldks(wihg, 2, 1024)
                whhg_s = ldks(whhg, 2, 1024)
                bgr_s = lds(bgr[:, :], [1, 1024])
                wt1_s = ldks(wt1, 2, 128)
                bt1_s = lds(bt1t[:, :], [128, 1], F32)
                wt2_s = lds(wt2[:, :], [128, 1])
                bt2_s = lds(bt2r[:, :], [128, 1], F32)

                # embeddings -> xcond tiles 0..2; thought -> 3..6
                xcond = pro.tile([128, 7, BL], BF, tag="xcond")
                nc.vector.memset(xcond[:, 0:3, :], 0.0)
                for k in range(4):
                    nc.vector.tensor_copy(xcond[:, 3 + k, :], st0_s[:, k, :])

                def onehot_embed(col, table_s, nt, width, out_slice):
                    pe_out = ps.tile([width, BL], F32, tag="sm")
                    for k in range(nt):
                        oh = work.tile([128, BL], BF, tag="oh")
                        nc.vector.scalar_tensor_tensor(
                            oh[:], idx_s[:, col, :], float(-128 * k),
                            iota_s[:, :].broadcast_to((128, BL)),
                            op0=ALU.add, op1=ALU.is_equal)
                        kk = table_s.shape[0] if nt == 1 else 128
                        lhs = table_s[:, k, :] if nt > 1 else table_s[:, :]
                        nc.tensor.matmul(pe_out[:], lhs[:kk] if nt == 1 else lhs,
                                         oh[:kk] if nt == 1 else oh[:],
                                         start=(k == 0), stop=(k == nt - 1))
                    nc.scalar.activation(out_slice, pe_out[:], AF.Copy)

                onehot_embed(0, embs_s, 1, 16, xcond[0:16, 0, :])
                onehot_embed(1, embi_s, 32, 64, xcond[0:64, 1, :])
                onehot_embed(2, embf_s, 1, 16, xcond[0:16, 2, :])

                # conditioning MLP
                def mlp_layer(w_s, nk, x_s, mt, bias_s, relu, out_t):
                    for m in range(mt):
                        pe_o = ps.tile([128, BL], F32, tag="sm")
                        for k in range(nk):
                            nc.tensor.matmul(
                                pe_o[:], w_s[:, k, m * 128:(m + 1) * 128],
                                x_s[:, k, :],
                                start=(k == 0), stop=(k == nk - 1))
                        nc.scalar.activation(
                            out_t[:, m, :], pe_o[:],
                            AF.Relu if relu else AF.Identity,
                            bias=bias_s[:, m:m + 1])

                ct1 = pro.tile([128, 4, BL], BF, tag="ct1")
                mlp_layer(wc1p_s, 7, xcond, 4, bc1_s, True, ct1)
                ct2 = pro.tile([128, 3, BL], BF, tag="ct2")
                mlp_layer(wc2_s, 4, ct1, 3, bc2_s, True, ct2)
                ct3f = pro.tile([128, 4, BL], F32, tag="ct3f")
                mlp_layer(wc3_s, 3, ct2, 4, bc3_s, False, ct3f)

                # initial state: h0 -> outs17 slot 0, c0 -> c32 + cb
                nc.scalar.activation(outs17[:, :, 0, :], ct3f[:, 0:2, :],
                                     AF.Copy)
                nc.scalar.activation(cb[:, :, :], ct3f[:, 2:4, :], AF.Copy)
                nc.vector.tensor_copy(c32[:, :, :], ct3f[:, 2:4, :])

                # P precompute: P[f, t, b] = sum_d X[t,b,d] Wa1a[d,f]
                # xdt streamed in big staged pieces (few DMAs; SP issues),
                # matmul'd in 8-t psum sub-chunks.
                PT = 46
                CH = 8
                cc = 0
                t0 = 0
                while t0 < T:
                    pn = min(PT, T - t0)
                    xc = xchunk.tile([128, 2, PT, BL], F8, tag="xc")
                    for k in range(2):
                        nc.sync.dma_start(xc[:, k, :pn, :],
                                          xdt[k, :, t0:t0 + pn, :])
                    u0 = 0
                    while u0 < pn:
                        tn = min(CH, pn - u0)
                        pe_p = ps.tile([128, CH * BL], F32, tag="sm")
                        for k in range(2):
                            nc.tensor.matmul(
                                pe_p[:, :tn * BL], wa1a_s[:, k, :],
                                xc[:, k, u0:u0 + tn, :].rearrange(
                                    "p t b -> p (t b)"),
                                start=(k == 0), stop=(k == 1))
                        dst = P_s[:, t0 + u0:t0 + u0 + tn, :].rearrange(
                            "p t b -> p (t b)")
                        if cc % 2 == 0:
                            nc.scalar.activation(dst, pe_p[:, :tn * BL],
                                                 AF.Copy)
                        else:
                            nc.vector.tensor_copy(dst, pe_p[:, :tn * BL])
                        cc += 1
                        u0 += tn
                    t0 += pn

            # ================= decoder steps =================
            # Two anti-phase batch halves: while one half runs its DVE max
            # phase, the other half runs softmax/context/gates/LSTM.
            _dec_cm = tc.tile_pool(name="dec", bufs=1)
            _mch_cm = tc.tile_pool(name="mch", bufs=3)
            work = _dec_cm.__enter__()
            mpool = _mch_cm.__enter__()
            HB = 32
            TC3 = [(0, 122), (122, 122), (244, 121)]
            groups = [(0, 128), (1, 128), (2, 109)]
            e_half = [e_sb0, e_sb1]

            def hslice(h):
                return slice(h * HB, (h + 1) * HB)

            def chain(mm, prev):
                if prev is not None:
                    add_dep_helper(mm.ins, prev.ins, sync=False,
                                   reason="psum accum order")
                return mm

            def a_nq(s, h):
                hsl = hslice(h)
                hs = [outs17[:, 0, s, hsl], outs17[:, 1, s, hsl],
                      cb[:, 0, hsl], cb[:, 1, hsl]]
                nq_ps = ps.tile([128, HB], F32, tag="sm")
                mms = [(nba1_s[0:1, :], ones1[0:1, 0:HB]),
                       (wa1b_s[:, 2, :], hs[2]), (wa1b_s[:, 3, :], hs[3]),
                       (wa1b_s[:, 0, :], hs[0]), (wa1b_s[:, 1, :], hs[1])]
                prev = None
                for gi, (wm, xm) in enumerate(mms):
                    prev = chain(nc.tensor.matmul(
                        nq_ps[:], wm, xm, start=(gi == 0), stop=(gi == 4)),
                        prev)
                nq_sb = work.tile([128, HB], BF, tag=f"nq{h}")
                nc.vector.tensor_copy(nq_sb[:], nq_ps[:])
                return nq_sb

            def b_max(s, h, nq_sb):
                hsl = hslice(h)
                sc_ps = pssc.tile([HB, T], F32, tag="sc")
                for t0, tn in TC3:
                    mc = mpool.tile([128, TC3[0][1], HB], BF, tag="mc")
                    nc.vector.tensor_tensor(
                        mc[:, 0:tn, :], P_s[:, t0:t0 + tn, hsl],
                        nq_sb[:].unsqueeze(1).broadcast_to((128, tn, HB)),
                        op=ALU.max)
                    for jj in range(tn):
                        nc.tensor.matmul(sc_ps[:, t0 + jj:t0 + jj + 1],
                                         mc[:, jj, :], wa2_s[:, :],
                                         start=True, stop=True)
                return sc_ps

            def c_soft(s, h, sc_ps):
                esb = e_half[h]
                zz = work.tile([HB, 1], F32, tag=f"zz{h}")
                nc.scalar.activation(esb[:, 0:T], sc_ps[:, :], AF.Exp,
                                     accum_out=zz[:])
                zrec = work.tile([HB, 1], F32, tag=f"zr{h}")
                nc.vector.reciprocal(zrec[:], zz[:])
                nc.vector.tensor_scalar_mul(esb[:, 0:T], esb[:, 0:T],
                                            zrec[:, 0:1])
                wt_ps = pswt.tile([128, 3, HB], BF, tag="wtp")
                nc.tensor.transpose(wt_ps[:, 0, :], esb[:, 0:128],
                                    id16_s[0:HB, 0:HB])
                nc.tensor.transpose(wt_ps[:, 1, :], esb[:, 128:256],
                                    id16_s[0:HB, 0:HB])
                nc.tensor.transpose(wt_ps[0:112, 2, :], esb[:, 256:368],
                                    id16_s[0:HB, 0:HB])
                w_th = work.tile([128, 3, HB], BF, tag=f"wt{h}")
                nc.scalar.activation(w_th[:, 0:2, :], wt_ps[:, 0:2, :],
                                     AF.Copy)
                nc.scalar.activation(w_th[0:112, 2, :], wt_ps[0:112, 2, :],
                                     AF.Copy)
                return w_th

            def d_ctx(s, h, w_th):
                v_ps = ps.tile([128, 2, HB], F32, tag="sm")
                prev = None
                for b in range(HB):
                    gb = h * HB + b
                    for dm in range(2):
                        for gi, (g, kn) in enumerate(groups):
                            prev = chain(nc.tensor.matmul(
                                v_ps[:, dm, b:b + 1],
                                xtb_s[0:kn, g, gb, dm * 128:(dm + 1) * 128],
                                w_th[0:kn, g, b:b + 1],
                                start=(gi == 0), stop=(gi == 2)), prev)
                inpT = work.tile([128, 2, HB], BF, tag=f"inp{h}")
                nc.scalar.activation(inpT[:], v_ps[:], AF.Copy)
                return inpT

            def e_gates(s, h, inpT):
                hsl = hslice(h)
                hs0 = outs17[:, 0, s, hsl]
                hs1 = outs17[:, 1, s, hsl]
                g_ps = psg.tile([128, 8 * HB], F32, tag="g")
                prev = None
                for m in range(8):
                    sl_ = g_ps[:, m * HB:(m + 1) * HB]
                    gmms = [(bgr_s[0:1, m * 128:(m + 1) * 128],
                             ones1[0:1, 0:HB]),
                            (whhg_s[:, 0, m * 128:(m + 1) * 128], hs0),
                            (whhg_s[:, 1, m * 128:(m + 1) * 128], hs1),
                            (wihg_s[:, 0, m * 128:(m + 1) * 128],
                             inpT[:, 0, :]),
                            (wihg_s[:, 1, m * 128:(m + 1) * 128],
                             inpT[:, 1, :])]
                    for gi, (wm, xm) in enumerate(gmms):
                        prev = chain(nc.tensor.matmul(
                            sl_, wm, xm, start=(gi == 0), stop=(gi == 4)),
                            prev)
                ey = work.tile([128, 8 * HB], F32, tag=f"ey{h}")
                nc.scalar.activation(ey[:], g_ps[:], AF.Exp)
                return ey

            def f1_lstm(s, h, ey):
                # sigma = 1/(1+ey) for i,f,o; tanh_g = 2r-1; c update
                hsl = hslice(h)
                nc.gpsimd.tensor_scalar_add(ey[:], ey[:], 1.0)
                nc.vector.reciprocal(ey[:], ey[:])
                gt = work.tile([128, 2, HB], F32, tag=f"gt{h}")
                nc.gpsimd.tensor_scalar(
                    out=gt[:],
                    in0=ey[:, 4 * HB:6 * HB].rearrange("p (k b) -> p k b",
                                                       k=2),
                    scalar1=2.0, scalar2=-1.0, op0=ALU.mult, op1=ALU.add)
                t1 = work.tile([128, 2, HB], F32, tag=f"t1{h}")
                nc.vector.tensor_tensor(
                    t1[:],
                    ey[:, 2 * HB:4 * HB].rearrange("p (k b) -> p k b", k=2),
                    c32[:, :, hsl], op=ALU.mult)
                t2 = work.tile([128, 2, HB], F32, tag=f"t2{h}")
                nc.vector.tensor_tensor(
                    t2[:],
                    ey[:, 0:2 * HB].rearrange("p (k b) -> p k b", k=2),
                    gt[:], op=ALU.mult)
                nc.vector.tensor_tensor(c32[:, :, hsl], t1[:], t2[:],
                                        op=ALU.add)
                nc.gpsimd.tensor_copy(cb[:, :, hsl], c32[:, :, hsl])

            def f2_lstm(s, h, ey):
                # h = sigma_o * tanh(c) with tanh via exp(-2c)
                hsl = hslice(h)
                ec = work.tile([128, 2, HB], F32, tag=f"ec{h}")
                nc.scalar.activation(ec[:], c32[:, :, hsl], AF.Exp,
                                     scale=-2.0)
                nc.vector.tensor_scalar_add(ec[:], ec[:], 1.0)
                nc.vector.reciprocal(ec[:], ec[:])
                nc.vector.tensor_scalar(out=ec[:], in0=ec[:], scalar1=2.0,
                                        scalar2=-1.0, op0=ALU.mult,
                                        op1=ALU.add)
                nc.vector.tensor_tensor(
                    outs17[:, :, s + 1, hsl],
                    ey[:, 6 * HB:8 * HB].rearrange("p (k b) -> p k b", k=2),
                    ec[:], op=ALU.mult)

            nq0 = a_nq(0, 0)
            ey_prev = [None, None]
            for s in range(NF):
                sc0 = b_max(s, 0, nq0)
                if s > 0:
                    f1_lstm(s - 1, 1, ey_prev[1])
                wt0 = c_soft(s, 0, sc0)
                if s > 0:
                    f2_lstm(s - 1, 1, ey_prev[1])
                nq1 = a_nq(s, 1)
                inp0 = d_ctx(s, 0, wt0)
                ey0 = e_gates(s, 0, inp0)
                sc1 = b_max(s, 1, nq1)
                f1_lstm(s, 0, ey0)
                wt1h = c_soft(s, 1, sc1)
                f2_lstm(s, 0, ey0)
                if s + 1 < NF:
                    nq0 = a_nq(s + 1, 0)
                inp1 = d_ctx(s, 1, wt1h)
                ey_prev[1] = e_gates(s, 1, inp1)
            f1_lstm(NF - 1, 1, ey_prev[1])
            f2_lstm(NF - 1, 1, ey_prev[1])

            # ---- final MLP: td = relu(outs@Wt1+bt1)@Wt2 + bt2 ----
            for half in range(2):
                f_ps = psg.tile([128, 512], F32, tag="g")
                for k in range(2):
                    nc.tensor.matmul(
                        f_ps[:],
                        wt1_s[:, k, :],
                        outs17[:, k, 1 + half * 8:1 + (half + 1) * 8,
                               :].rearrange("p s b -> p (s b)"),
                        start=(k == 0), stop=(k == 1))
                nc.scalar.activation(h1[:, half, :], f_ps[:], AF.Relu,
                                     bias=bt1_s[:, :])
            td_ps = ps.tile([128, 8], F32, tag="sm")
            h1f = h1[:, :, :].rearrange("p h x -> p (h x)")
            for j in range(8):
                nc.tensor.matmul(td_ps[:, j:j + 1],
                                 h1f[:, 128 * j:128 * (j + 1)],
                                 wt2_s[:, :], start=True, stop=True)
            td_sb = work.tile([128, 8], F32, tag="tdsb")
            nc.vector.tensor_scalar_add(td_sb[:], td_ps[:], bt2_s[:, 0:1])
            nc.sync.dma_start(
                out.rearrange("(j two) b -> (two b) j", two=2), td_sb[:, :])
            _mch_cm.__exit__(None, None, None)
            _dec_cm.__exit__(None, None, None)

    nc.compile()
    return nc


_NC = None


def _get_nc():
    global _NC
    if _NC is None:
        _NC = build_bass()
    return _NC


def kernel(x_cat_static, state_h, state_c, outputs_encoder,
           emb_store, emb_item, emb_family,
           Wc1, bc1, Wc2, bc2, Wc3, bc3,
           Wa1, ba1, Wa2, ba2,
           Wt1, bt1, Wt2, bt2,
           Wih, Whh, bih, bhh):
    nc = _get_nc()

    # Wc1 rows: [store 16][item 64][family 16][thought 512] -> pad each
    # embedding block to its own 128-row K-tile, thought gets 4 tiles.
    wc1p = np.zeros((7, 128, 512), ml_dtypes.bfloat16)
    wc1p[0, :16] = _bf(Wc1[0:16])
    wc1p[1, :64] = _bf(Wc1[16:80])
    wc1p[2, :16] = _bf(Wc1[80:96])
    for k in range(4):
        wc1p[3 + k] = _bf(Wc1[96 + k * 128:96 + (k + 1) * 128])

    emb_i_t = np.zeros((32, 128, 64), ml_dtypes.bfloat16)
    ei = _bf(emb_item)
    for k in range(32):
        lo = k * 128
        hi = min(4036, lo + 128)
        if lo < 4036:
            emb_i_t[k, :hi - lo] = ei[lo:hi]
    emb_i_t = np.ascontiguousarray(emb_i_t.transpose(1, 0, 2))

    # gate scaling: sigmoid gates (i,f,o) need exp(-x); tanh gate (g) exp(-2x)
    gsc = np.full(1024, -1.0, np.float32)
    gsc[512:768] = -2.0
    wih_g = (np.asarray(Wih, np.float32) * gsc[:, None]).T   # [256, 1024]
    whh_g = (np.asarray(Whh, np.float32) * gsc[:, None]).T
    bias_g = (_f32(bih) + _f32(bhh)) * gsc

    common = {
        "iota": np.arange(128, dtype=np.float32).reshape(128, 1),
        "emb_s": _bf(emb_store),
        "emb_i": emb_i_t,
        "emb_f": _bf(emb_family),
        "wc1p": wc1p,
        "bc1t": _f32(bc1).reshape(4, 128).T.copy(),
        "wc2": _ktile(_bf(Wc2), 512, 512),
        "bc2t": _f32(bc2).reshape(3, 128).T.copy(),
        "wc3": _ktile(_bf(Wc3), 384, 384),
        "bc3t": _f32(bc3).reshape(4, 128).T.copy(),
        "wa1a": _ktile(np.ascontiguousarray(Wa1[:256]).astype(
            ml_dtypes.float8_e4m3), 256, 256),
        "nwa1b": _ktile(_bf(-np.asarray(Wa1[256:], np.float32)), 512, 512),
        "nba1r": _bf(-_f32(ba1)).reshape(1, 128),
        "wa2": _bf(Wa2),
        "wihg": _ktile(_bf(wih_g), 256, 256),
        "whhg": _ktile(_bf(whh_g), 256, 256),
        "bgr": _bf(bias_g).reshape(1, 1024),
        "wt1": _ktile(_bf(Wt1), 256, 256),
        "bt1t": _f32(bt1).reshape(128, 1),
        "wt2": _bf(Wt2),
        "bt2r": np.full((128, 1), float(np.asarray(bt2).reshape(-1)[0]),
                        np.float32),
        "id16": np.eye(128, dtype=ml_dtypes.bfloat16),
    }

    oe = np.asarray(outputs_encoder)
    th = np.concatenate([np.asarray(state_h)[0], np.asarray(state_c)[0]],
                        axis=-1)  # [B, 512]
    xc = np.asarray(x_cat_static)

    in_maps = []
    for c in range(N_CORES):
        b0 = c * BL
        shb = np.ascontiguousarray(oe[:, b0:b0 + BL, :]).astype(
            ml_dtypes.float8_e4m3)                     # [T, BL, D]
        xtb_t = np.zeros((3, 128, BL, D), ml_dtypes.float8_e4m3)
        xtb_t[0] = shb[0:128]
        xtb_t[1] = shb[128:256]
        xtb_t[2, 0:109] = shb[256:365]
        xdt_t = np.ascontiguousarray(
            oe[:, b0:b0 + BL, :].transpose(2, 0, 1).reshape(
                2, 128, T, BL)).astype(ml_dtypes.float8_e4m3)
        st_t = np.ascontiguousarray(
            _bf(th[b0:b0 + BL]).T.reshape(4, 128, BL))
        idxr = np.broadcast_to(
            xc[b0:b0 + BL].T.astype(np.float32)[None, :, :],
            (128, 3, BL)).copy()
        m = dict(common)
        m.update({"xdt": xdt_t, "xtb": xtb_t, "st0": st_t, "idxr": idxr})
        in_maps.append(m)

    kw = {}
    if os.environ.get("KTRACE", "") == "1":
        kw = dict(trace=True, trace_cores=[0])
    res = run_bass_kernel_spmd(nc, in_maps, list(range(N_CORES)), **kw)
    if res.exec_time_ns is not None:
        print("HW exec time:", res.exec_time_ns, "ns  (mean",
              res.mean_exec_time_ns, ")", flush=True)
    outs = [res.results[c]["out"] for c in range(N_CORES)]
    return np.concatenate(outs, axis=1).astype(np.float32)


# revision 28
# speedup vs baseline: 2.1592x; 1.0514x over previous
import sys, os
sys.path.insert(0, '/opt/trn_rl_repo')
import numpy as np
import ml_dtypes
import concourse.bass as bass
import concourse.bacc as bacc
import concourse.mybir as mybir
import concourse.tile as tile
from concourse.tile import add_dep_helper
from concourse.bass_utils import run_bass_kernel_spmd

BF = mybir.dt.bfloat16
F32 = mybir.dt.float32
F8 = mybir.dt.float8e4
AF = mybir.ActivationFunctionType
ALU = mybir.AluOpType
AX = mybir.AxisListType

N_CORES = 8
B = 512
BL = B // N_CORES      # 64 batch rows per core
T = 365
D = 256
FA = 128               # attention MLP hidden
NF = 16                # forecast steps
NR = 256               # decoder hidden

# max-phase split: DVE handles t[0:TDVE) in 4 chunks, Pool t[TDVE:365)
TDVE = 264
DCH = TDVE // 4        # 66


def _bf(x):
    return np.ascontiguousarray(x).astype(ml_dtypes.bfloat16)


def _f32(x):
    return np.ascontiguousarray(x).astype(np.float32)


def _ktile(w, k_total, pad_to):
    """[K, M] -> [ntiles, 128, M] zero-padded on K."""
    K, M = w.shape
    assert K == k_total
    nt = (pad_to + 127) // 128
    out = np.zeros((nt, 128, M), w.dtype)
    for i in range(nt):
        lo = i * 128
        hi = min(K, lo + 128)
        if lo < K:
            out[i, :hi - lo] = w[lo:hi]
    return out


def build_bass():
    nc = bacc.Bacc("TRN2", target_bir_lowering=False, debug=False,
                   num_devices=N_CORES)

    def inp(name, shape, dt=BF):
        return nc.dram_tensor(name, shape, dt, kind="ExternalInput")

    # per-core sharded tensors
    xdt = inp("xdt", [2, 128, T, BL], F8)          # X^T: xdt[k,p,t,b] = X[t,b,128k+p]
    xtb = inp("xtb", [3, 128, BL, D], F8)          # X t-grouped: [t0:128|128:256|256:365]
    st0 = inp("st0", [4, 128, BL])             # [h0;c0]^T k-tiled
    idxr = inp("idxr", [128, 3, BL], F32)      # indices replicated over partitions
    # replicated weights
    iota = inp("iota", [128, 1], F32)
    emb_s = inp("emb_s", [54, 16])
    emb_i = inp("emb_i", [128, 32, 64])
    emb_f = inp("emb_f", [33, 16])
    wc1p = inp("wc1p", [7, 128, 512])
    bc1t = inp("bc1t", [128, 4], F32)
    wc2 = inp("wc2", [4, 128, 384])
    bc2t = inp("bc2t", [128, 3], F32)
    wc3 = inp("wc3", [3, 128, 512])
    bc3t = inp("bc3t", [128, 4], F32)
    wa1a = inp("wa1a", [2, 128, 128], F8)
    nwa1b = inp("nwa1b", [4, 128, 128])        # -Wa1[256:768] k-tiled
    nba1r = inp("nba1r", [1, 128])             # -ba1 as a row
    wa2 = inp("wa2", [128, 1])
    wihg = inp("wihg", [2, 128, 1024])         # Wih^T k-tiled, gate-scaled
    whhg = inp("whhg", [2, 128, 1024])         # Whh^T k-tiled, gate-scaled
    bgr = inp("bgr", [1, 1024])                # (bih+bhh) gate-scaled row
    wt1 = inp("wt1", [2, 128, 128])
    bt1t = inp("bt1t", [128, 1], F32)
    wt2 = inp("wt2", [128, 1])
    bt2r = inp("bt2r", [128, 1], F32)
    id16 = inp("id16", [128, 128])             # bf16 identity
    out = nc.dram_tensor("out", [NF, BL], F32, kind="ExternalOutput")

    with tile.TileContext(nc) as tc:
        with (
            tc.tile_pool(name="const", bufs=1) as cpool,
            tc.tile_pool(name="big", bufs=1) as bigpool,
            tc.tile_pool(name="state", bufs=1) as state,
            tc.tile_pool(name="ps", bufs=4, space="PSUM") as ps,
            tc.tile_pool(name="psg", bufs=1, space="PSUM") as psg,
            tc.tile_pool(name="pssc", bufs=1, space="PSUM") as pssc,
            tc.tile_pool(name="pswt", bufs=1, space="PSUM") as pswt,
        ):
            # ---- persistent constants (issued from Act HWDGE; SP is busy
            # streaming xdt and big xtb transfers block their issuing seq) ----
            def ld(ap_dram, shape, dt=BF, tag=None):
                t_ = cpool.tile(shape, dt, tag=tag or ap_dram.name)
                nc.scalar.dma_start(t_[:], ap_dram)
                return t_

            def ldk(dram, nt, m, pool, dt=BF):
                t_ = pool.tile([128, nt, m], dt, tag=dram.name)
                nc.scalar.dma_start(
                    t_[:, :, :], dram[:, :, :].rearrange("n p m -> p n m"))
                return t_

            ones1 = cpool.tile([1, BL], BF, tag="ones1")
            nc.vector.memset(ones1[:], 1.0)

            # ---- persistent big tensors ----
            xtb_s = bigpool.tile([128, 3, BL, D], F8, tag="xtb")
            P_s = bigpool.tile([128, T, BL], BF, tag="P")

            # ---- persistent state ----
            outs17 = state.tile([128, 2, NF + 1, BL], BF, tag="outs")
            c32 = state.tile([128, 2, BL], F32, tag="c32")
            cb = state.tile([128, 2, BL], BF, tag="cb")
            e_sb0 = state.tile([32, 368], BF, tag="esb0")
            nc.vector.memset(e_sb0[:, T:368], 0.0)
            e_sb1 = state.tile([32, 368], BF, tag="esb1")
            nc.vector.memset(e_sb1[:, T:368], 0.0)
            h1 = state.tile([128, 2, 512], BF, tag="h1")

            # ================= prologue =================
            with (
                tc.tile_pool(name="pro", bufs=1) as pro,
                tc.tile_pool(name="proh", bufs=2) as work,
                tc.tile_pool(name="xchunk", bufs=8) as xchunk,
            ):
                idx_s = pro.tile([128, 3, BL], F32, tag="idx")
                nc.scalar.dma_start(idx_s[:], idxr[:, :, :])
                iota_s = pro.tile([128, 1], F32, tag="iota")
                nc.scalar.dma_start(iota_s[:], iota[:, :])
                st0_s = ldk(st0, 4, BL, pro)
                embs_s = pro.tile([54, 16], BF, tag="embs")
                nc.scalar.dma_start(embs_s[:], emb_s[:, :])
                embf_s = pro.tile([33, 16], BF, tag="embf")
                nc.scalar.dma_start(embf_s[:], emb_f[:, :])
                embi_s = pro.tile([128, 32, 64], BF, tag="embi")
                nc.scalar.dma_start(embi_s[:, :, :], emb_i[:, :, :])
                wa1a_s = ldk(wa1a, 2, 128, pro, dt=F8)
                wc1p_s = ldk(wc1p, 7, 512, pro)
                bc1_s = pro.tile([128, 4], F32, tag="bc1")
                nc.scalar.dma_start(bc1_s[:], bc1t[:, :])
                wc2_s = ldk(wc2, 4, 384, pro)
                bc2_s = pro.tile([128, 3], F32, tag="bc2")
                nc.scalar.dma_start(bc2_s[:], bc2t[:, :])
                wc3_s = ldk(wc3, 3, 512, pro)
                bc3_s = pro.tile([128, 4], F32, tag="bc3")
                nc.scalar.dma_start(bc3_s[:], bc3t[:, :])
                # embeddings -> xcond tiles 0..2; thought -> 3..6
                xcond = pro.tile([128, 7, BL], BF, tag="xcond")
                nc.vector.memset(xcond[:, 0:3, :], 0.0)
                for k in range(4):
                    nc.vector.tensor_copy(xcond[:, 3 + k, :], st0_s[:, k, :])

                def onehot_embed(col, table_s, nt, width, out_slice):
                    pe_out = ps.tile([width, BL], F32, tag="sm")
                    for k in range(nt):
                        oh = work.tile([128, BL], BF, tag="oh")
                        nc.vector.scalar_tensor_tensor(
                            oh[:], idx_s[:, col, :], float(-128 * k),
                            iota_s[:, :].broadcast_to((128, BL)),
                            op0=ALU.add, op1=ALU.is_equal)
                        kk = table_s.shape[0] if nt == 1 else 128
                        lhs = table_s[:, k, :] if nt > 1 else table_s[:, :]
                        nc.tensor.matmul(pe_out[:], lhs[:kk] if nt == 1 else lhs,
                                         oh[:kk] if nt == 1 else oh[:],
                                         start=(k == 0), stop=(k == nt - 1))
                    nc.scalar.activation(out_slice, pe_out[:], AF.Copy)

                onehot_embed(0, embs_s, 1, 16, xcond[0:16, 0, :])
                onehot_embed(1, embi_s, 32, 64, xcond[0:64, 1, :])
                onehot_embed(2, embf_s, 1, 16, xcond[0:16, 2, :])

                # conditioning MLP
                def mlp_layer(w_s, nk, x_s, mt, bias_s, relu, out_t):
                    for m in range(mt):
                        pe_o = ps.tile([128, BL], F32, tag="sm")
                        for k in range(nk):
                            nc.tensor.matmul(
                                pe_o[:], w_s[:, k, m * 128:(m + 1) * 128],
                                x_s[:, k, :],
                                start=(k == 0), stop=(k == nk - 1))
                        nc.scalar.activation(
                            out_t[:, m, :], pe_o[:],
                            AF.Relu if relu else AF.Identity,
                            bias=bias_s[:, m:m + 1])

                ct1 = pro.tile([128, 4, BL], BF, tag="ct1")
                mlp_layer(wc1p_s, 7, xcond, 4, bc1_s, True, ct1)
                ct2 = pro.tile([128, 3, BL], BF, tag="ct2")
                mlp_layer(wc2_s, 4, ct1, 3, bc2_s, True, ct2)
                ct3f = pro.tile([128, 4, BL], F32, tag="ct3f")
                mlp_layer(wc3_s, 3, ct2, 4, bc3_s, False, ct3f)

                # initial state: h0 -> outs17 slot 0, c0 -> c32 + cb
                nc.scalar.activation(outs17[:, :, 0, :], ct3f[:, 0:2, :],
                                     AF.Copy)
                nc.scalar.activation(cb[:, :, :], ct3f[:, 2:4, :], AF.Copy)
                nc.vector.tensor_copy(c32[:, :, :], ct3f[:, 2:4, :])

                # P precompute: P[f, t, b] = sum_d X[t,b,d] Wa1a[d,f]
                # xdt streamed in big staged pieces (few DMAs; SP issues),
                # matmul'd in 8-t psum sub-chunks.
                PT = 46
                CH = 8
                cc = 0
                t0 = 0
                while t0 < T:
                    pn = min(PT, T - t0)
                    xc = xchunk.tile([128, 2, PT, BL], F8, tag="xc")
                    for k in range(2):
                        nc.sync.dma_start(xc[:, k, :pn, :],
                                          xdt[k, :, t0:t0 + pn, :])
                    u0 = 0
                    while u0 < pn:
                        tn = min(CH, pn - u0)
                        pe_p = ps.tile([128, CH * BL], F32, tag="sm")
                        for k in range(2):
                            nc.tensor.matmul(
                                pe_p[:, :tn * BL], wa1a_s[:, k, :],
                                xc[:, k, u0:u0 + tn, :].rearrange(
                                    "p t b -> p (t b)"),
                                start=(k == 0), stop=(k == 1))
                        dst = P_s[:, t0 + u0:t0 + u0 + tn, :].rearrange(
                            "p t b -> p (t b)")
                        if cc % 2 == 0:
                            nc.scalar.activation(dst, pe_p[:, :tn * BL],
                                                 AF.Copy)
                        else:
                            nc.vector.tensor_copy(dst, pe_p[:, :tn * BL])
                        cc += 1
                        u0 += tn
                    t0 += pn

                # SP queue after the xdt pieces: step-0 nq/score weights,
                # then xtb (H0 batch-half first), then gate/final weights.
                def lds(ap_dram, shape, dt=BF, tag=None):
                    t_ = cpool.tile(shape, dt, tag=tag or ap_dram.name)
                    nc.sync.dma_start(t_[:], ap_dram)
                    return t_

                def ldks(dram, nt, m):
                    t_ = cpool.tile([128, nt, m], BF, tag=dram.name)
                    nc.sync.dma_start(
                        t_[:, :, :], dram[:, :, :].rearrange("n p m -> p n m"))
                    return t_

                wa1b_s = ldks(nwa1b, 4, 128)
                nba1_s = lds(nba1r[:, :], [1, 128])
                wa2_s = lds(wa2[:, :], [128, 1])
                id16_s = lds(id16[:, :], [128, 128])
                for k in range(3):
                    nc.sync.dma_start(xtb_s[:, k, 0:32, :],
                                      xtb[k, :, 0:32, :])
                wihg_s = ldks(wihg, 2, 1024)
                whhg_s = ldks(whhg, 2, 1024)
                bgr_s = lds(bgr[:, :], [1, 1024])
                for k in range(3):
                    nc.sync.dma_start(xtb_s[:, k, 32:64, :],
                                      xtb[k, :, 32:64, :])
                wt1_s = ldks(wt1, 2, 128)
                bt1_s = lds(bt1t[:, :], [128, 1], F32)
                wt2_s = lds(wt2[:, :], [128, 1])
                bt2_s = lds(bt2r[:, :], [128, 1], F32)

            # ================= decoder steps =================
            # Two anti-phase batch halves: while one half runs its DVE max
            # phase, the other half runs softmax/context/gates/LSTM.
            _dec_cm = tc.tile_pool(name="dec", bufs=1)
            _mch_cm = tc.tile_pool(name="mch", bufs=3)
            work = _dec_cm.__enter__()
            mpool = _mch_cm.__enter__()
            HB = 32
            TC3 = [(0, 122), (122, 122), (244, 121)]
            groups = [(0, 128), (1, 128), (2, 109)]
            e_half = [e_sb0, e_sb1]

            def hslice(h):
                return slice(h * HB, (h + 1) * HB)

            def chain(mm, prev):
                if prev is not None:
                    add_dep_helper(mm.ins, prev.ins, sync=False,
                                   reason="psum accum order")
                return mm

            def a_nq(s, h):
                hsl = hslice(h)
                hs = [outs17[:, 0, s, hsl], outs17[:, 1, s, hsl],
                      cb[:, 0, hsl], cb[:, 1, hsl]]
                nq_ps = ps.tile([128, HB], F32, tag="sm")
                mms = [(nba1_s[0:1, :], ones1[0:1, 0:HB]),
                       (wa1b_s[:, 2, :], hs[2]), (wa1b_s[:, 3, :], hs[3]),
                       (wa1b_s[:, 0, :], hs[0]), (wa1b_s[:, 1, :], hs[1])]
                prev = None
                for gi, (wm, xm) in enumerate(mms):
                    prev = chain(nc.tensor.matmul(
                        nq_ps[:], wm, xm, start=(gi == 0), stop=(gi == 4)),
                        prev)
                nq_sb = work.tile([128, HB], BF, tag=f"nq{h}")
                nc.vector.tensor_copy(nq_sb[:], nq_ps[:])
                return nq_sb

            def b_max(s, h, nq_sb):
                hsl = hslice(h)
                sc_ps = pssc.tile([HB, T], F32, tag="sc")
                for t0, tn in TC3:
                    mc = mpool.tile([128, TC3[0][1], HB], BF, tag="mc")
                    nc.vector.tensor_tensor(
                        mc[:, 0:tn, :], P_s[:, t0:t0 + tn, hsl],
                        nq_sb[:].unsqueeze(1).broadcast_to((128, tn, HB)),
                        op=ALU.max)
                    for jj in range(tn):
                        nc.tensor.matmul(sc_ps[:, t0 + jj:t0 + jj + 1],
                                         mc[:, jj, :], wa2_s[:, :],
                                         start=True, stop=True)
                return sc_ps

            def c_soft(s, h, sc_ps):
                esb = e_half[h]
                zz = work.tile([HB, 1], F32, tag=f"zz{h}")
                nc.scalar.activation(esb[:, 0:T], sc_ps[:, :], AF.Exp,
                                     accum_out=zz[:])
                zrec = work.tile([HB, 1], F32, tag=f"zr{h}")
                nc.vector.reciprocal(zrec[:], zz[:])
                nc.vector.tensor_scalar_mul(esb[:, 0:T], esb[:, 0:T],
                                            zrec[:, 0:1])
                wt_ps = pswt.tile([128, 3, HB], BF, tag="wtp")
                nc.tensor.transpose(wt_ps[:, 0, :], esb[:, 0:128],
                                    id16_s[0:HB, 0:HB])
                nc.tensor.transpose(wt_ps[:, 1, :], esb[:, 128:256],
                                    id16_s[0:HB, 0:HB])
                nc.tensor.transpose(wt_ps[0:112, 2, :], esb[:, 256:368],
                                    id16_s[0:HB, 0:HB])
                w_th = work.tile([128, 3, HB], BF, tag=f"wt{h}")
                nc.scalar.activation(w_th[:, 0:2, :], wt_ps[:, 0:2, :],
                                     AF.Copy)
                nc.scalar.activation(w_th[0:112, 2, :], wt_ps[0:112, 2, :],
                                     AF.Copy)
                return w_th

            def d_ctx(s, h, w_th):
                v_ps = ps.tile([128, 2, HB], F32, tag="sm")
                prev = None
                for b in range(HB):
                    gb = h * HB + b
                    for dm in range(2):
                        for gi, (g, kn) in enumerate(groups):
                            prev = chain(nc.tensor.matmul(
                                v_ps[:, dm, b:b + 1],
                                xtb_s[0:kn, g, gb, dm * 128:(dm + 1) * 128],
                                w_th[0:kn, g, b:b + 1],
                                start=(gi == 0), stop=(gi == 2)), prev)
                inpT = work.tile([128, 2, HB], BF, tag=f"inp{h}")
                nc.scalar.activation(inpT[:], v_ps[:], AF.Copy)
                return inpT

            def e_gates(s, h, inpT):
                hsl = hslice(h)
                hs0 = outs17[:, 0, s, hsl]
                hs1 = outs17[:, 1, s, hsl]
                g_ps = psg.tile([128, 8 * HB], F32, tag="g")
                prev = None
                for m in range(8):
                    sl_ = g_ps[:, m * HB:(m + 1) * HB]
                    gmms = [(bgr_s[0:1, m * 128:(m + 1) * 128],
                             ones1[0:1, 0:HB]),
                            (whhg_s[:, 0, m * 128:(m + 1) * 128], hs0),
                            (whhg_s[:, 1, m * 128:(m + 1) * 128], hs1),
                            (wihg_s[:, 0, m * 128:(m + 1) * 128],
                             inpT[:, 0, :]),
                            (wihg_s[:, 1, m * 128:(m + 1) * 128],
                             inpT[:, 1, :])]
                    for gi, (wm, xm) in enumerate(gmms):
                        prev = chain(nc.tensor.matmul(
                            sl_, wm, xm, start=(gi == 0), stop=(gi == 4)),
                            prev)
                ey = work.tile([128, 8 * HB], F32, tag=f"ey{h}")
                nc.scalar.activation(ey[:], g_ps[:], AF.Exp)
                return ey

            def f1_lstm(s, h, ey):
                # sigma = 1/(1+ey) for i,f,o; tanh_g = 2r-1; c update
                hsl = hslice(h)
                nc.gpsimd.tensor_scalar_add(ey[:], ey[:], 1.0)
                nc.vector.reciprocal(ey[:], ey[:])
                gt = work.tile([128, 2, HB], F32, tag=f"gt{h}")
                nc.gpsimd.tensor_scalar(
                    out=gt[:],
                    in0=ey[:, 4 * HB:6 * HB].rearrange("p (k b) -> p k b",
                                                       k=2),
                    scalar1=2.0, scalar2=-1.0, op0=ALU.mult, op1=ALU.add)
                t1 = work.tile([128, 2, HB], F32, tag=f"t1{h}")
                nc.vector.tensor_tensor(
                    t1[:],
                    ey[:, 2 * HB:4 * HB].rearrange("p (k b) -> p k b", k=2),
                    c32[:, :, hsl], op=ALU.mult)
                t2 = work.tile([128, 2, HB], F32, tag=f"t2{h}")
                nc.vector.tensor_tensor(
                    t2[:],
                    ey[:, 0:2 * HB].rearrange("p (k b) -> p k b", k=2),
                    gt[:], op=ALU.mult)
                nc.vector.tensor_tensor(c32[:, :, hsl], t1[:], t2[:],
                                        op=ALU.add)
                nc.gpsimd.tensor_copy(cb[:, :, hsl], c32[:, :, hsl])

            def f2_lstm(s, h, ey):
                # h = sigma_o * tanh(c) with tanh via exp(-2c)
                hsl = hslice(h)
                ec = work.tile([128, 2, HB], F32, tag=f"ec{h}")
                nc.scalar.activation(ec[:], c32[:, :, hsl], AF.Exp,
                                     scale=-2.0)
                nc.vector.tensor_scalar_add(ec[:], ec[:], 1.0)
                nc.vector.reciprocal(ec[:], ec[:])
                nc.vector.tensor_scalar(out=ec[:], in0=ec[:], scalar1=2.0,
                                        scalar2=-1.0, op0=ALU.mult,
                                        op1=ALU.add)
                nc.vector.tensor_tensor(
                    outs17[:, :, s + 1, hsl],
                    ey[:, 6 * HB:8 * HB].rearrange("p (k b) -> p k b", k=2),
                    ec[:], op=ALU.mult)

            nq0 = a_nq(0, 0)
            ey_prev = [None, None]
            for s in range(NF):
                sc0 = b_max(s, 0, nq0)
                if s > 0:
                    f1_lstm(s - 1, 1, ey_prev[1])
                wt0 = c_soft(s, 0, sc0)
                if s > 0:
                    f2_lstm(s - 1, 1, ey_prev[1])
                nq1 = a_nq(s, 1)
                inp0 = d_ctx(s, 0, wt0)
                ey0 = e_gates(s, 0, inp0)
                sc1 = b_max(s, 1, nq1)
                f1_lstm(s, 0, ey0)
                wt1h = c_soft(s, 1, sc1)
                f2_lstm(s, 0, ey0)
                if s + 1 < NF:
                    nq0 = a_nq(s + 1, 0)
                inp1 = d_ctx(s, 1, wt1h)
                ey_prev[1] = e_gates(s, 1, inp1)
            f1_lstm(NF - 1, 1, ey_prev[1])
            f2_lstm(NF - 1, 1, ey_prev[1])

            # ---- final MLP: td = relu(outs@Wt1+bt1)@Wt2 + bt2 ----
            for half in range(2):
                f_ps = psg.tile([128, 512], F32, tag="g")
                for k in range(2):
                    nc.tensor.matmul(
                        f_ps[:],
                        wt1_s[:, k, :],
                        outs17[:, k, 1 + half * 8:1 + (half + 1) * 8,
                               :].rearrange("p s b -> p (s b)"),
                        start=(k == 0), stop=(k == 1))
                nc.scalar.activation(h1[:, half, :], f_ps[:], AF.Relu,
                                     bias=bt1_s[:, :])
            td_ps = ps.tile([128, 8], F32, tag="sm")
            h1f = h1[:, :, :].rearrange("p h x -> p (h x)")
            for j in range(8):
                nc.tensor.matmul(td_ps[:, j:j + 1],
                                 h1f[:, 128 * j:128 * (j + 1)],
                                 wt2_s[:, :], start=True, stop=True)
            td_sb = work.tile([128, 8], F32, tag="tdsb")
            nc.vector.tensor_scalar_add(td_sb[:], td_ps[:], bt2_s[:, 0:1])
            nc.sync.dma_start(
                out.rearrange("(j two) b -> (two b) j", two=2), td_sb[:, :])
            _mch_cm.__exit__(None, None, None)
            _dec_cm.__exit__(None, None, None)

    nc.compile()
    return nc


_NC = None


def _get_nc():
    global _NC
    if _NC is None:
        _NC = build_bass()
    return _NC


def kernel(x_cat_static, state_h, state_c, outputs_encoder,
           emb_store, emb_item, emb_family,
           Wc1, bc1, Wc2, bc2, Wc3, bc3,
           Wa1, ba1, Wa2, ba2,
           Wt1, bt1, Wt2, bt2,
           Wih, Whh, bih, bhh):
    nc = _get_nc()

    # Wc1 rows: [store 16][item 64][family 16][thought 512] -> pad each
    # embedding block to its own 128-row K-tile, thought gets 4 tiles.
    wc1p = np.zeros((7, 128, 512), ml_dtypes.bfloat16)
    wc1p[0, :16] = _bf(Wc1[0:16])
    wc1p[1, :64] = _bf(Wc1[16:80])
    wc1p[2, :16] = _bf(Wc1[80:96])
    for k in range(4):
        wc1p[3 + k] = _bf(Wc1[96 + k * 128:96 + (k + 1) * 128])

    emb_i_t = np.zeros((32, 128, 64), ml_dtypes.bfloat16)
    ei = _bf(emb_item)
    for k in range(32):
        lo = k * 128
        hi = min(4036, lo + 128)
        if lo < 4036:
            emb_i_t[k, :hi - lo] = ei[lo:hi]
    emb_i_t = np.ascontiguousarray(emb_i_t.transpose(1, 0, 2))

    # gate scaling: sigmoid gates (i,f,o) need exp(-x); tanh gate (g) exp(-2x)
    gsc = np.full(1024, -1.0, np.float32)
    gsc[512:768] = -2.0
    wih_g = (np.asarray(Wih, np.float32) * gsc[:, None]).T   # [256, 1024]
    whh_g = (np.asarray(Whh, np.float32) * gsc[:, None]).T
    bias_g = (_f32(bih) + _f32(bhh)) * gsc

    common = {
        "iota": np.arange(128, dtype=np.float32).reshape(128, 1),
        "emb_s": _bf(emb_store),
        "emb_i": emb_i_t,
        "emb_f": _bf(emb_family),
        "wc1p": wc1p,
        "bc1t": _f32(bc1).reshape(4, 128).T.copy(),
        "wc2": _ktile(_bf(Wc2), 512, 512),
        "bc2t": _f32(bc2).reshape(3, 128).T.copy(),
        "wc3": _ktile(_bf(Wc3), 384, 384),
        "bc3t": _f32(bc3).reshape(4, 128).T.copy(),
        "wa1a": _ktile(np.ascontiguousarray(Wa1[:256]).astype(
            ml_dtypes.float8_e4m3), 256, 256),
        "nwa1b": _ktile(_bf(-np.asarray(Wa1[256:], np.float32)), 512, 512),
        "nba1r": _bf(-_f32(ba1)).reshape(1, 128),
        "wa2": _bf(Wa2),
        "wihg": _ktile(_bf(wih_g), 256, 256),
        "whhg": _ktile(_bf(whh_g), 256, 256),
        "bgr": _bf(bias_g).reshape(1, 1024),
        "wt1": _ktile(_bf(Wt1), 256, 256),
        "bt1t": _f32(bt1).reshape(128, 1),
        "wt2": _bf(Wt2),
        "bt2r": np.full((128, 1), float(np.asarray(bt2).reshape(-1)[0]),
                        np.float32),
        "id16": np.eye(128, dtype=ml_dtypes.bfloat16),
    }

    oe = np.asarray(outputs_encoder)
    th = np.concatenate([np.asarray(state_h)[0], np.asarray(state_c)[0]],
                        axis=-1)  # [B, 512]
    xc = np.asarray(x_cat_static)

    in_maps = []
    for c in range(N_CORES):
        b0 = c * BL
        shb = np.ascontiguousarray(oe[:, b0:b0 + BL, :]).astype(
            ml_dtypes.float8_e4m3)                     # [T, BL, D]
        xtb_t = np.zeros((3, 128, BL, D), ml_dtypes.float8_e4m3)
        xtb_t[0] = shb[0:128]
        xtb_t[1] = shb[128:256]
        xtb_t[2, 0:109] = shb[256:365]
        xdt_t = np.ascontiguousarray(
            oe[:, b0:b0 + BL, :].transpose(2, 0, 1).reshape(
                2, 128, T, BL)).astype(ml_dtypes.float8_e4m3)
        st_t = np.ascontiguousarray(
            _bf(th[b0:b0 + BL]).T.reshape(4, 128, BL))
        idxr = np.broadcast_to(
            xc[b0:b0 + BL].T.astype(np.float32)[None, :, :],
            (128, 3, BL)).copy()
        m = dict(common)
        m.update({"xdt": xdt_t, "xtb": xtb_t, "st0": st_t, "idxr": idxr})
        in_maps.append(m)

    kw = {}
    if os.environ.get("KTRACE", "") == "1":
        kw = dict(trace=True, trace_cores=[0])
    res = run_bass_kernel_spmd(nc, in_maps, list(range(N_CORES)), **kw)
    if res.exec_time_ns is not None:
        print("HW exec time:", res.exec_time_ns, "ns  (mean",
              res.mean_exec_time_ns, ")", flush=True)
    outs = [res.results[c]["out"] for c in range(N_CORES)]
    return np.concatenate(outs, axis=1).astype(np.float32)


# revision 31
# speedup vs baseline: 2.2026x; 1.0201x over previous
import sys, os
sys.path.insert(0, '/opt/trn_rl_repo')
import numpy as np
import ml_dtypes
import concourse.bass as bass
import concourse.bacc as bacc
import concourse.mybir as mybir
import concourse.tile as tile
from concourse.tile import add_dep_helper
from concourse.bass_utils import run_bass_kernel_spmd

BF = mybir.dt.bfloat16
F32 = mybir.dt.float32
F8 = mybir.dt.float8e4
AF = mybir.ActivationFunctionType
ALU = mybir.AluOpType
AX = mybir.AxisListType

N_CORES = 8
B = 512
BL = B // N_CORES      # 64 batch rows per core
T = 365
D = 256
FA = 128               # attention MLP hidden
NF = 16                # forecast steps
NR = 256               # decoder hidden

# max-phase split: DVE handles t[0:TDVE) in 4 chunks, Pool t[TDVE:365)
TDVE = 264
DCH = TDVE // 4        # 66


def _bf(x):
    return np.ascontiguousarray(x).astype(ml_dtypes.bfloat16)


def _f32(x):
    return np.ascontiguousarray(x).astype(np.float32)


def _ktile(w, k_total, pad_to):
    """[K, M] -> [ntiles, 128, M] zero-padded on K."""
    K, M = w.shape
    assert K == k_total
    nt = (pad_to + 127) // 128
    out = np.zeros((nt, 128, M), w.dtype)
    for i in range(nt):
        lo = i * 128
        hi = min(K, lo + 128)
        if lo < K:
            out[i, :hi - lo] = w[lo:hi]
    return out


def build_bass():
    nc = bacc.Bacc("TRN2", target_bir_lowering=False, debug=False,
                   num_devices=N_CORES)

    def inp(name, shape, dt=BF):
        return nc.dram_tensor(name, shape, dt, kind="ExternalInput")

    # per-core sharded tensors
    xdt = inp("xdt", [2, 128, T, BL], F8)          # X^T: xdt[k,p,t,b] = X[t,b,128k+p]
    xtb = inp("xtb", [3, 128, BL, D], F8)          # X t-grouped: [t0:128|128:256|256:365]
    st0 = inp("st0", [4, 128, BL])             # [h0;c0]^T k-tiled
    idxr = inp("idxr", [128, 3, BL], F32)      # indices replicated over partitions
    # replicated weights
    iota = inp("iota", [128, 1], F32)
    emb_s = inp("emb_s", [54, 16])
    emb_i = inp("emb_i", [128, 32, 64])
    emb_f = inp("emb_f", [33, 16])
    wc1p = inp("wc1p", [7, 128, 512])
    bc1t = inp("bc1t", [128, 4], F32)
    wc2 = inp("wc2", [4, 128, 384])
    bc2t = inp("bc2t", [128, 3], F32)
    wc3 = inp("wc3", [3, 128, 512])
    bc3t = inp("bc3t", [128, 4], F32)
    wa1a = inp("wa1a", [2, 128, 128], F8)
    nwa1b = inp("nwa1b", [4, 128, 128])        # -Wa1[256:768] k-tiled
    nba1c = inp("nba1c", [128, 1], F32)        # -ba1 as a column
    wa2 = inp("wa2", [128, 1])
    wihg = inp("wihg", [2, 128, 1024])         # Wih^T k-tiled, gate-scaled
    whhg = inp("whhg", [2, 128, 1024])         # Whh^T k-tiled, gate-scaled
    bgt2 = inp("bgt2", [128, 8], F32)          # gate-scaled bias, m-tiled
    wt1 = inp("wt1", [2, 128, 128])
    bt1t = inp("bt1t", [128, 1], F32)
    wt2 = inp("wt2", [128, 1])
    bt2r = inp("bt2r", [128, 1], F32)
    id16 = inp("id16", [128, 128])             # bf16 identity
    out = nc.dram_tensor("out", [NF, BL], F32, kind="ExternalOutput")

    with tile.TileContext(nc) as tc:
        with (
            tc.tile_pool(name="const", bufs=1) as cpool,
            tc.tile_pool(name="big", bufs=1) as bigpool,
            tc.tile_pool(name="state", bufs=1) as state,
            tc.tile_pool(name="ps", bufs=2, space="PSUM") as ps,
            tc.tile_pool(name="pps", bufs=2, space="PSUM") as pps,
            tc.tile_pool(name="psg", bufs=1, space="PSUM") as psg,
            tc.tile_pool(name="pssc", bufs=1, space="PSUM") as pssc,
            tc.tile_pool(name="pswt", bufs=1, space="PSUM") as pswt,
        ):
            # ---- persistent constants (issued from Act HWDGE; SP is busy
            # streaming xdt and big xtb transfers block their issuing seq) ----
            def ld(ap_dram, shape, dt=BF, tag=None):
                t_ = cpool.tile(shape, dt, tag=tag or ap_dram.name)
                nc.scalar.dma_start(t_[:], ap_dram)
                return t_

            def ldk(dram, nt, m, pool, dt=BF):
                t_ = pool.tile([128, nt, m], dt, tag=dram.name)
                nc.scalar.dma_start(
                    t_[:, :, :], dram[:, :, :].rearrange("n p m -> p n m"))
                return t_

            ones1 = cpool.tile([1, BL], BF, tag="ones1")
            nc.vector.memset(ones1[:], 1.0)

            # ---- persistent big tensors ----
            xtb_s = bigpool.tile([128, 3, BL, D], F8, tag="xtb")
            P_s = bigpool.tile([128, T, BL], BF, tag="P")

            # ---- persistent state ----
            outs17 = state.tile([128, 2, NF + 1, BL], BF, tag="outs")
            c32 = state.tile([128, 2, BL], F32, tag="c32")
            cb = state.tile([128, 2, BL], BF, tag="cb")
            e_sb0 = state.tile([32, 368], BF, tag="esb0")
            nc.vector.memset(e_sb0[:, T:368], 0.0)
            e_sb1 = state.tile([32, 368], BF, tag="esb1")
            nc.vector.memset(e_sb1[:, T:368], 0.0)
            h1 = state.tile([128, 2, 512], BF, tag="h1")

            # ================= prologue =================
            with (
                tc.tile_pool(name="pro", bufs=1) as pro,
                tc.tile_pool(name="proh", bufs=2) as work,
                tc.tile_pool(name="xchunk", bufs=8) as xchunk,
            ):
                idx_s = pro.tile([128, 3, BL], F32, tag="idx")
                nc.scalar.dma_start(idx_s[:], idxr[:, :, :])
                iota_s = pro.tile([128, 1], F32, tag="iota")
                nc.scalar.dma_start(iota_s[:], iota[:, :])
                embi_s = pro.tile([128, 32, 64], BF, tag="embi")
                nc.scalar.dma_start(embi_s[:, :, :], emb_i[:, :, :])
                wa1a_s = ldk(wa1a, 2, 128, pro, dt=F8)
                st0_s = ldk(st0, 4, BL, pro)
                embs_s = pro.tile([54, 16], BF, tag="embs")
                nc.scalar.dma_start(embs_s[:], emb_s[:, :])
                embf_s = pro.tile([33, 16], BF, tag="embf")
                nc.scalar.dma_start(embf_s[:], emb_f[:, :])
                wc1p_s = ldk(wc1p, 7, 512, pro)
                bc1_s = pro.tile([128, 4], F32, tag="bc1")
                nc.scalar.dma_start(bc1_s[:], bc1t[:, :])
                wc2_s = ldk(wc2, 4, 384, pro)
                bc2_s = pro.tile([128, 3], F32, tag="bc2")
                nc.scalar.dma_start(bc2_s[:], bc2t[:, :])
                wc3_s = ldk(wc3, 3, 512, pro)
                bc3_s = pro.tile([128, 4], F32, tag="bc3")
                nc.scalar.dma_start(bc3_s[:], bc3t[:, :])
                # embeddings -> xcond tiles 0..2; thought -> 3..6
                xcond = pro.tile([128, 7, BL], BF, tag="xcond")
                nc.vector.memset(xcond[:, 0:3, :], 0.0)
                for k in range(4):
                    nc.vector.tensor_copy(xcond[:, 3 + k, :], st0_s[:, k, :])

                def onehot_embed(col, table_s, nt, width, out_slice):
                    pe_out = ps.tile([width, BL], F32, tag="sm")
                    for k in range(nt):
                        oh = work.tile([128, BL], BF, tag="oh")
                        nc.vector.scalar_tensor_tensor(
                            oh[:], idx_s[:, col, :], float(-128 * k),
                            iota_s[:, :].broadcast_to((128, BL)),
                            op0=ALU.add, op1=ALU.is_equal)
                        kk = table_s.shape[0] if nt == 1 else 128
                        lhs = table_s[:, k, :] if nt > 1 else table_s[:, :]
                        nc.tensor.matmul(pe_out[:], lhs[:kk] if nt == 1 else lhs,
                                         oh[:kk] if nt == 1 else oh[:],
                                         start=(k == 0), stop=(k == nt - 1))
                    nc.vector.tensor_copy(out_slice, pe_out[:])

                onehot_embed(0, embs_s, 1, 16, xcond[0:16, 0, :])
                onehot_embed(1, embi_s, 32, 64, xcond[0:64, 1, :])
                onehot_embed(2, embf_s, 1, 16, xcond[0:16, 2, :])

                # conditioning MLP
                def mlp_layer(w_s, nk, x_s, mt, bias_s, relu, out_t):
                    for m in range(mt):
                        pe_o = ps.tile([128, BL], F32, tag="sm")
                        for k in range(nk):
                            nc.tensor.matmul(
                                pe_o[:], w_s[:, k, m * 128:(m + 1) * 128],
                                x_s[:, k, :],
                                start=(k == 0), stop=(k == nk - 1))
                        if relu:
                            nc.vector.tensor_scalar(
                                out=out_t[:, m, :], in0=pe_o[:],
                                scalar1=bias_s[:, m:m + 1], scalar2=0.0,
                                op0=ALU.add, op1=ALU.max)
                        else:
                            nc.vector.tensor_scalar_add(
                                out_t[:, m, :], pe_o[:],
                                bias_s[:, m:m + 1])

                ct1 = pro.tile([128, 4, BL], BF, tag="ct1")
                mlp_layer(wc1p_s, 7, xcond, 4, bc1_s, True, ct1)
                ct2 = pro.tile([128, 3, BL], BF, tag="ct2")
                mlp_layer(wc2_s, 4, ct1, 3, bc2_s, True, ct2)
                ct3f = pro.tile([128, 4, BL], F32, tag="ct3f")
                mlp_layer(wc3_s, 3, ct2, 4, bc3_s, False, ct3f)

                # initial state: h0 -> outs17 slot 0, c0 -> c32 + cb
                nc.vector.tensor_copy(outs17[:, :, 0, :], ct3f[:, 0:2, :])
                nc.vector.tensor_copy(cb[:, :, :], ct3f[:, 2:4, :])
                nc.vector.tensor_copy(c32[:, :, :], ct3f[:, 2:4, :])

                # P precompute: P[f, t, b] = sum_d X[t,b,d] Wa1a[d,f]
                # xdt streamed in big staged pieces (few DMAs; SP issues),
                # matmul'd in 8-t psum sub-chunks.
                PT = 46
                CH = 8
                cc = 0
                t0 = 0
                while t0 < T:
                    pn = min(PT, T - t0)
                    xc = xchunk.tile([128, 2, PT, BL], F8, tag="xc")
                    for k in range(2):
                        nc.sync.dma_start(xc[:, k, :pn, :],
                                          xdt[k, :, t0:t0 + pn, :])
                    u0 = 0
                    while u0 < pn:
                        tn = min(CH, pn - u0)
                        pe_p = pps.tile([128, CH * BL], F32, tag="pch")
                        for k in range(2):
                            nc.tensor.matmul(
                                pe_p[:, :tn * BL], wa1a_s[:, k, :],
                                xc[:, k, u0:u0 + tn, :].rearrange(
                                    "p t b -> p (t b)"),
                                start=(k == 0), stop=(k == 1))
                        dst = P_s[:, t0 + u0:t0 + u0 + tn, :].rearrange(
                            "p t b -> p (t b)")
                        if t0 + u0 < 180:
                            nc.scalar.activation(dst, pe_p[:, :tn * BL],
                                                 AF.Copy)
                        else:
                            nc.vector.tensor_copy(dst, pe_p[:, :tn * BL])
                        cc += 1
                        u0 += tn
                    t0 += pn

                # SP queue after the xdt pieces: step-0 nq/score weights,
                # then xtb (H0 batch-half first), then gate/final weights.
                def lds(ap_dram, shape, dt=BF, tag=None):
                    t_ = cpool.tile(shape, dt, tag=tag or ap_dram.name)
                    nc.sync.dma_start(t_[:], ap_dram)
                    return t_

                def ldks(dram, nt, m):
                    t_ = cpool.tile([128, nt, m], BF, tag=dram.name)
                    nc.sync.dma_start(
                        t_[:, :, :], dram[:, :, :].rearrange("n p m -> p n m"))
                    return t_

                wa1b_s = ldks(nwa1b, 4, 128)
                nba1_s = lds(nba1c[:, :], [128, 1], F32)
                wa2_s = lds(wa2[:, :], [128, 1])
                id16_s = lds(id16[:, :], [128, 128])
                for k in range(3):
                    nc.sync.dma_start(xtb_s[:, k, 0:32, :],
                                      xtb[k, :, 0:32, :])
                wihg_s = ldks(wihg, 2, 1024)
                whhg_s = ldks(whhg, 2, 1024)
                bgt_s = lds(bgt2[:, :], [128, 8], F32)
                for k in range(3):
                    nc.sync.dma_start(xtb_s[:, k, 32:64, :],
                                      xtb[k, :, 32:64, :])
                wt1_s = ldks(wt1, 2, 128)
                bt1_s = lds(bt1t[:, :], [128, 1], F32)
                wt2_s = lds(wt2[:, :], [128, 1])
                bt2_s = lds(bt2r[:, :], [128, 1], F32)
                # gate bias replicated over b once; psum gets pre-written
                # with it each step instead of a bias-row matmul
                bgrep = cpool.tile([128, 8, 32], F32, tag="bgrep")
                nc.vector.tensor_copy(
                    bgrep[:], bgt_s[:, :].unsqueeze(2).broadcast_to(
                        (128, 8, 32)))

            # ================= decoder steps =================
            # Two anti-phase batch halves: while one half runs its DVE max
            # phase, the other half runs softmax/context/gates/LSTM.
            _dec_cm = tc.tile_pool(name="dec", bufs=1)
            work = _dec_cm.__enter__()
            HB = 32
            TC3 = [(0, 122), (122, 122), (244, 121)]
            groups = [(0, 128), (1, 128), (2, 109)]
            e_half = [e_sb0, e_sb1]
            m0 = work.tile([128, T, 32], BF, tag="m0")
            m1 = work.tile([128, T, 32], BF, tag="m1")
            m_half = [m0, m1]

            def hslice(h):
                return slice(h * HB, (h + 1) * HB)

            def chain(mm, prev):
                if prev is not None:
                    add_dep_helper(mm.ins, prev.ins, sync=False,
                                   reason="psum accum order")
                return mm

            def a_nq(s, h):
                hsl = hslice(h)
                hs = [outs17[:, 0, s, hsl], outs17[:, 1, s, hsl],
                      cb[:, 0, hsl], cb[:, 1, hsl]]
                nq_ps = ps.tile([128, HB], F32, tag="sm")
                nc.scalar.activation(
                    nq_ps[:], nba1_s[:, :].broadcast_to((128, HB)), AF.Copy)
                mms = [(wa1b_s[:, 2, :], hs[2]), (wa1b_s[:, 3, :], hs[3]),
                       (wa1b_s[:, 0, :], hs[0]), (wa1b_s[:, 1, :], hs[1])]
                prev = None
                for gi, (wm, xm) in enumerate(mms):
                    prev = chain(nc.tensor.matmul(
                        nq_ps[:], wm, xm, start=False, stop=(gi == 3)),
                        prev)
                nq_sb = work.tile([128, HB], BF, tag=f"nq{h}")
                nc.vector.tensor_copy(nq_sb[:], nq_ps[:])
                q_sb = work.tile([128, HB], BF, tag=f"q{h}")
                nc.scalar.activation(q_sb[:], nq_ps[:], AF.Copy, scale=-1.0)
                return nq_sb, q_sb

            NBD = 24   # b-columns the DVE max covers; Act relu does the rest

            def b_max(s, h, nq_sb, q_sb):
                hsl = hslice(h)
                mh = m_half[h]
                sc_ps = pssc.tile([HB, T], F32, tag="sc")
                for b in range(NBD, HB):
                    nc.scalar.activation(
                        mh[:, 0:T, b], P_s[:, 0:T, h * HB + b], AF.Relu,
                        bias=q_sb[:, b:b + 1])
                for t0, tn in TC3:
                    nc.vector.tensor_tensor(
                        mh[:, t0:t0 + tn, 0:NBD],
                        P_s[:, t0:t0 + tn, h * HB:h * HB + NBD],
                        nq_sb[:, 0:NBD].unsqueeze(1).broadcast_to(
                            (128, tn, NBD)),
                        op=ALU.max)
                    for jj in range(tn):
                        nc.tensor.matmul(sc_ps[:, t0 + jj:t0 + jj + 1],
                                         mh[:, t0 + jj, :], wa2_s[:, :],
                                         start=True, stop=True)
                return sc_ps

            def c_soft(s, h, sc_ps):
                esb = e_half[h]
                zz = work.tile([HB, 1], F32, tag=f"zz{h}")
                nc.scalar.activation(esb[:, 0:T], sc_ps[:, :], AF.Exp,
                                     accum_out=zz[:])
                zrec = work.tile([HB, 1], F32, tag=f"zr{h}")
                nc.vector.reciprocal(zrec[:], zz[:])
                nc.vector.tensor_scalar_mul(esb[:, 0:T], esb[:, 0:T],
                                            zrec[:, 0:1])
                wt_ps = pswt.tile([128, 3, HB], BF, tag="wtp")
                nc.tensor.transpose(wt_ps[:, 0, :], esb[:, 0:128],
                                    id16_s[0:HB, 0:HB])
                nc.tensor.transpose(wt_ps[:, 1, :], esb[:, 128:256],
                                    id16_s[0:HB, 0:HB])
                nc.tensor.transpose(wt_ps[0:112, 2, :], esb[:, 256:368],
                                    id16_s[0:HB, 0:HB])
                w_th = work.tile([128, 3, HB], BF, tag=f"wt{h}")
                nc.scalar.activation(w_th[:, 0:2, :], wt_ps[:, 0:2, :],
                                     AF.Copy)
                nc.scalar.activation(w_th[0:112, 2, :], wt_ps[0:112, 2, :],
                                     AF.Copy)
                return w_th

            def d_ctx(s, h, w_th):
                v_ps = ps.tile([128, 2, HB], F32, tag="sm")
                prev = None
                for b in range(HB):
                    gb = h * HB + b
                    for dm in range(2):
                        for gi, (g, kn) in enumerate(groups):
                            prev = chain(nc.tensor.matmul(
                                v_ps[:, dm, b:b + 1],
                                xtb_s[0:kn, g, gb, dm * 128:(dm + 1) * 128],
                                w_th[0:kn, g, b:b + 1],
                                start=(gi == 0), stop=(gi == 2)), prev)
                inpT = work.tile([128, 2, HB], BF, tag=f"inp{h}")
                nc.scalar.activation(inpT[:], v_ps[:], AF.Copy)
                return inpT

            def e_gates(s, h, inpT):
                hsl = hslice(h)
                hs0 = outs17[:, 0, s, hsl]
                hs1 = outs17[:, 1, s, hsl]
                g_ps = psg.tile([128, 8 * HB], F32, tag="g")
                nc.scalar.activation(
                    g_ps[:].rearrange("p (m b) -> p m b", m=8), bgrep[:],
                    AF.Copy)
                prev = None
                for m in range(8):
                    sl_ = g_ps[:, m * HB:(m + 1) * HB]
                    gmms = [(whhg_s[:, 0, m * 128:(m + 1) * 128], hs0),
                            (whhg_s[:, 1, m * 128:(m + 1) * 128], hs1),
                            (wihg_s[:, 0, m * 128:(m + 1) * 128],
                             inpT[:, 0, :]),
                            (wihg_s[:, 1, m * 128:(m + 1) * 128],
                             inpT[:, 1, :])]
                    for gi, (wm, xm) in enumerate(gmms):
                        prev = chain(nc.tensor.matmul(
                            sl_, wm, xm, start=False, stop=(gi == 3)),
                            prev)
                ey = work.tile([128, 8 * HB], F32, tag=f"ey{h}")
                nc.scalar.activation(ey[:], g_ps[:], AF.Exp)
                return ey

            def f1_lstm(s, h, ey):
                # sigma = 1/(1+ey) for i,f,o; tanh_g = 2r-1; c update
                hsl = hslice(h)
                nc.gpsimd.tensor_scalar_add(ey[:], ey[:], 1.0)
                nc.vector.reciprocal(ey[:], ey[:])
                gt = work.tile([128, 2, HB], F32, tag=f"gt{h}")
                nc.gpsimd.tensor_scalar(
                    out=gt[:],
                    in0=ey[:, 4 * HB:6 * HB].rearrange("p (k b) -> p k b",
                                                       k=2),
                    scalar1=2.0, scalar2=-1.0, op0=ALU.mult, op1=ALU.add)
                t1 = work.tile([128, 2, HB], F32, tag=f"t1{h}")
                nc.vector.tensor_tensor(
                    t1[:],
                    ey[:, 2 * HB:4 * HB].rearrange("p (k b) -> p k b", k=2),
                    c32[:, :, hsl], op=ALU.mult)
                t2 = work.tile([128, 2, HB], F32, tag=f"t2{h}")
                nc.vector.tensor_tensor(
                    t2[:],
                    ey[:, 0:2 * HB].rearrange("p (k b) -> p k b", k=2),
                    gt[:], op=ALU.mult)
                nc.vector.tensor_tensor(c32[:, :, hsl], t1[:], t2[:],
                                        op=ALU.add)
                nc.gpsimd.tensor_copy(cb[:, :, hsl], c32[:, :, hsl])

            def f2_lstm(s, h, ey):
                # h = sigma_o * tanh(c) with tanh via exp(-2c)
                hsl = hslice(h)
                ec = work.tile([128, 2, HB], F32, tag=f"ec{h}")
                nc.scalar.activation(ec[:], c32[:, :, hsl], AF.Exp,
                                     scale=-2.0)
                nc.vector.tensor_scalar_add(ec[:], ec[:], 1.0)
                nc.vector.reciprocal(ec[:], ec[:])
                nc.vector.tensor_scalar(out=ec[:], in0=ec[:], scalar1=2.0,
                                        scalar2=-1.0, op0=ALU.mult,
                                        op1=ALU.add)
                nc.vector.tensor_tensor(
                    outs17[:, :, s + 1, hsl],
                    ey[:, 6 * HB:8 * HB].rearrange("p (k b) -> p k b", k=2),
                    ec[:], op=ALU.mult)

            nq0 = a_nq(0, 0)
            ey_prev = [None, None]
            for s in range(NF):
                sc0 = b_max(s, 0, *nq0)
                if s > 0:
                    f1_lstm(s - 1, 1, ey_prev[1])
                wt0 = c_soft(s, 0, sc0)
                if s > 0:
                    f2_lstm(s - 1, 1, ey_prev[1])
                nq1 = a_nq(s, 1)
                inp0 = d_ctx(s, 0, wt0)
                ey0 = e_gates(s, 0, inp0)
                sc1 = b_max(s, 1, *nq1)
                f1_lstm(s, 0, ey0)
                wt1h = c_soft(s, 1, sc1)
                f2_lstm(s, 0, ey0)
                if s + 1 < NF:
                    nq0 = a_nq(s + 1, 0)
                inp1 = d_ctx(s, 1, wt1h)
                ey_prev[1] = e_gates(s, 1, inp1)
            f1_lstm(NF - 1, 1, ey_prev[1])
            f2_lstm(NF - 1, 1, ey_prev[1])

            # ---- final MLP: td = relu(outs@Wt1+bt1)@Wt2 + bt2 ----
            for half in range(2):
                f_ps = psg.tile([128, 512], F32, tag="g")
                for k in range(2):
                    nc.tensor.matmul(
                        f_ps[:],
                        wt1_s[:, k, :],
                        outs17[:, k, 1 + half * 8:1 + (half + 1) * 8,
                               :].rearrange("p s b -> p (s b)"),
                        start=(k == 0), stop=(k == 1))
                nc.scalar.activation(h1[:, half, :], f_ps[:], AF.Relu,
                                     bias=bt1_s[:, :])
            td_ps = ps.tile([128, 8], F32, tag="sm")
            h1f = h1[:, :, :].rearrange("p h x -> p (h x)")
            for j in range(8):
                nc.tensor.matmul(td_ps[:, j:j + 1],
                                 h1f[:, 128 * j:128 * (j + 1)],
                                 wt2_s[:, :], start=True, stop=True)
            td_sb = work.tile([128, 8], F32, tag="tdsb")
            nc.vector.tensor_scalar_add(td_sb[:], td_ps[:], bt2_s[:, 0:1])
            nc.sync.dma_start(
                out.rearrange("(j two) b -> (two b) j", two=2), td_sb[:, :])
            _dec_cm.__exit__(None, None, None)

    nc.compile()
    return nc


_NC = None


def _get_nc():
    global _NC
    if _NC is None:
        _NC = build_bass()
    return _NC


def kernel(x_cat_static, state_h, state_c, outputs_encoder,
           emb_store, emb_item, emb_family,
           Wc1, bc1, Wc2, bc2, Wc3, bc3,
           Wa1, ba1, Wa2, ba2,
           Wt1, bt1, Wt2, bt2,
           Wih, Whh, bih, bhh):
    nc = _get_nc()

    # Wc1 rows: [store 16][item 64][family 16][thought 512] -> pad each
    # embedding block to its own 128-row K-tile, thought gets 4 tiles.
    wc1p = np.zeros((7, 128, 512), ml_dtypes.bfloat16)
    wc1p[0, :16] = _bf(Wc1[0:16])
    wc1p[1, :64] = _bf(Wc1[16:80])
    wc1p[2, :16] = _bf(Wc1[80:96])
    for k in range(4):
        wc1p[3 + k] = _bf(Wc1[96 + k * 128:96 + (k + 1) * 128])

    emb_i_t = np.zeros((32, 128, 64), ml_dtypes.bfloat16)
    ei = _bf(emb_item)
    for k in range(32):
        lo = k * 128
        hi = min(4036, lo + 128)
        if lo < 4036:
            emb_i_t[k, :hi - lo] = ei[lo:hi]
    emb_i_t = np.ascontiguousarray(emb_i_t.transpose(1, 0, 2))

    # gate scaling: sigmoid gates (i,f,o) need exp(-x); tanh gate (g) exp(-2x)
    gsc = np.full(1024, -1.0, np.float32)
    gsc[512:768] = -2.0
    wih_g = (np.asarray(Wih, np.float32) * gsc[:, None]).T   # [256, 1024]
    whh_g = (np.asarray(Whh, np.float32) * gsc[:, None]).T
    bias_g = (_f32(bih) + _f32(bhh)) * gsc

    common = {
        "iota": np.arange(128, dtype=np.float32).reshape(128, 1),
        "emb_s": _bf(emb_store),
        "emb_i": emb_i_t,
        "emb_f": _bf(emb_family),
        "wc1p": wc1p,
        "bc1t": _f32(bc1).reshape(4, 128).T.copy(),
        "wc2": _ktile(_bf(Wc2), 512, 512),
        "bc2t": _f32(bc2).reshape(3, 128).T.copy(),
        "wc3": _ktile(_bf(Wc3), 384, 384),
        "bc3t": _f32(bc3).reshape(4, 128).T.copy(),
        "wa1a": _ktile(np.ascontiguousarray(Wa1[:256]).astype(
            ml_dtypes.float8_e4m3), 256, 256),
        "nwa1b": _ktile(_bf(-np.asarray(Wa1[256:], np.float32)), 512, 512),
        "nba1c": (-_f32(ba1)).reshape(128, 1),
        "wa2": _bf(Wa2),
        "wihg": _ktile(_bf(wih_g), 256, 256),
        "whhg": _ktile(_bf(whh_g), 256, 256),
        "bgt2": np.ascontiguousarray(
            _f32(bias_g).reshape(8, 128).T),
        "wt1": _ktile(_bf(Wt1), 256, 256),
        "bt1t": _f32(bt1).reshape(128, 1),
        "wt2": _bf(Wt2),
        "bt2r": np.full((128, 1), float(np.asarray(bt2).reshape(-1)[0]),
                        np.float32),
        "id16": np.eye(128, dtype=ml_dtypes.bfloat16),
    }

    oe = np.asarray(outputs_encoder)
    th = np.concatenate([np.asarray(state_h)[0], np.asarray(state_c)[0]],
                        axis=-1)  # [B, 512]
    xc = np.asarray(x_cat_static)

    in_maps = []
    for c in range(N_CORES):
        b0 = c * BL
        shb = np.ascontiguousarray(oe[:, b0:b0 + BL, :]).astype(
            ml_dtypes.float8_e4m3)                     # [T, BL, D]
        xtb_t = np.zeros((3, 128, BL, D), ml_dtypes.float8_e4m3)
        xtb_t[0] = shb[0:128]
        xtb_t[1] = shb[128:256]
        xtb_t[2, 0:109] = shb[256:365]
        xdt_t = np.ascontiguousarray(
            oe[:, b0:b0 + BL, :].transpose(2, 0, 1).reshape(
                2, 128, T, BL)).astype(ml_dtypes.float8_e4m3)
        st_t = np.ascontiguousarray(
            _bf(th[b0:b0 + BL]).T.reshape(4, 128, BL))
        idxr = np.broadcast_to(
            xc[b0:b0 + BL].T.astype(np.float32)[None, :, :],
            (128, 3, BL)).copy()
        m = dict(common)
        m.update({"xdt": xdt_t, "xtb": xtb_t, "st0": st_t, "idxr": idxr})
        in_maps.append(m)

    kw = {}
    if os.environ.get("KTRACE", "") == "1":
        kw = dict(trace=True, trace_cores=[0])
    res = run_bass_kernel_spmd(nc, in_maps, list(range(N_CORES)), **kw)
    if res.exec_time_ns is not None:
        print("HW exec time:", res.exec_time_ns, "ns  (mean",
              res.mean_exec_time_ns, ")", flush=True)
    outs = [res.results[c]["out"] for c in range(N_CORES)]
    return np.concatenate(outs, axis=1).astype(np.float32)
